# revision 89
# baseline (speedup 1.0000x reference)
"""Trainium2 Bass kernel for nn_AttentionModule_7146825580577.

Strategy (see spec sharding_hint): pure data parallel over the batch dim
(8192 rows -> 1024 rows per core, 8 cores), weights replicated.

v2 schedule: the two 512-column batch tiles of each core are interleaved
inside every layer unit so the Tensor engine always has independent
matmul work queued (keeps the PE out of its low-clock pstates), weights
are streamed from HBM once (each chunk feeds both batch tiles), all
matmul operands are bf16, the gate sigmoid is computed via tanh (so the
scalar engine never has to swap activation tables), the logit sigmoid is
folded into the host-side preprocessing, and the rsqrt Newton iteration
runs on the otherwise-idle GPSIMD engine over PE-transposed stat tiles.

Device math (per core), in feature-transposed layout (features on SBUF
partitions, batch on the free dim):

  - All LayerNorms whose input is an affine function of a previous
    activation use host-side column-centered weights, so mean(y) == 0 by
    construction and only sum(y^2) is needed on device (computed by a
    ones-vector matmul on the PE, reduced over partitions).
  - seq_len==1 MHA reduces to out_proj(v_proj(kv)); both projections are
    fused on the host into a single 512x512 effective matrix. The self-
    attention residual (x + sa(x)) is folded into a single matmul with
    weights I + Wv@Wo.
  - The cross-attention pair average (a+b)/2 is a single concat-matmul.
  - The n2 LayerNorm (after gating) is folded into the fus_W1 matmul:
    gamma scales fold into the weights, betas fold into the bias; the
    per-sample mean correction is applied as zt = t*istd_bc - wrow_bc
    with both rows partition-broadcast on GPSIMD.
  - 1/sqrt(var+eps) uses the int32 bit trick + Newton-Raphson on GPSIMD,
    on PE-transposed [128, k] stat tiles so each op is tiny.
"""
import os
import sys

sys.path.insert(0, "/opt/trn_rl_repo")

import numpy as np

import concourse.bass as bass
import concourse.tile as tile
from concourse import bacc, mybir
from concourse.bass import ts
from concourse.bass_utils import run_bass_kernel_spmd
from concourse.masks import make_identity

D = 512
HID = 1024
B = 8192
NCORES = 8
BL = B // NCORES          # rows per core
EPS = 1e-5
MAGIC = 0x5F3759DF
F32 = mybir.dt.float32
I32 = mybir.dt.int32
FS = [10, 6, 15]          # logit dims per stream
NR_ITERS = int(os.environ.get("KERNEL_NR_ITERS", "2"))
MM_DT = {
    "f32r": mybir.dt.float32r,
    "f32": mybir.dt.float32,
    "bf16": mybir.dt.bfloat16,
}[os.environ.get("KERNEL_MM_DTYPE", "bf16")]
MM_IS_BF16 = MM_DT == mybir.dt.bfloat16
MF = MM_DT

F64 = np.float64


# --------------------------------------------------------------------------
# Host-side weight folding
# --------------------------------------------------------------------------

def _center_cols(W, b):
    W = np.asarray(W, F64)
    b = np.asarray(b, F64)
    return W - W.mean(axis=1, keepdims=True), b - b.mean()


def fold_weights(inp):
    g = lambda k: np.asarray(inp[k], dtype=F64)
    out = {}

    w_hp, b_hp = [], []
    for s in range(3):
        W, b = _center_cols(g("hp_W")[s], g("hp_b")[s])
        w_hp.append(W)
        b_hp.append(b)
    out["w_hp"] = np.stack(w_hp)
    out["b_hp"] = np.stack(b_hp)
    out["g_hp"], out["be_hp"] = g("hp_g"), g("hp_be")

    mhaW, mhab = g("mha_in_W"), g("mha_in_b")
    moW, mob = g("mha_out_W"), g("mha_out_b")
    Wv0, bv0 = mhaW[0][:, 2 * D:], mhab[0][2 * D:]
    Wr, br = _center_cols(np.eye(D) + Wv0 @ moW[0], bv0 @ moW[0] + mob[0])
    out["w_r"], out["b_r"] = Wr, br
    out["g_n1"], out["be_n1"] = g("n1_g"), g("n1_be")

    Wj, bj = [None] * 4, [None] * 4
    for j in (1, 2, 3):
        Wv, bv = mhaW[j][:, 2 * D:], mhab[j][2 * D:]
        Wj[j] = Wv @ moW[j]
        bj[j] = bv @ moW[j] + mob[j]
    # m_verb uses (inst_e @ W1, target_e @ W2); m_inst (verb @ W1, target @ W3);
    # m_target (verb @ W2, inst @ W3)
    # m is stored pre-halved (0.25 = average 0.5 x sigmoid-via-tanh 0.5):
    # t = e + sigmoid(pre)*m_avg = (e + m_tilde) + tanh(pre/2)*m_tilde
    # with m_tilde = 0.5*m_avg.
    mods = [(1, 2), (1, 3), (2, 3)]
    w_m, b_m = [], []
    for s in range(3):
        ja, jb = mods[s]
        w_m.append(np.concatenate([0.25 * Wj[ja], 0.25 * Wj[jb]], axis=0))
        b_m.append(0.25 * (bj[ja] + bj[jb]))
    out["w_m"] = np.stack(w_m)
    out["b_m"] = np.stack(b_m)

    out["w_g"] = g("gate_W")
    # tanh trick: sigmoid(x + b) = 0.5*tanh(0.5*x + 0.5*b) + 0.5
    out["b_g_half"] = 0.5 * g("gate_b")

    w_lp, b_lp = [], []
    for s, key in enumerate(["verb", "inst", "target"]):
        W, b = _center_cols(g(f"lp_W_{key}"), g(f"lp_b_{key}"))
        w_lp.append(W)
        b_lp.append(b)
    out["w_lp"] = w_lp
    out["b_lp"] = np.stack(b_lp)
    out["g_lp"], out["be_lp"] = g("lp_g"), g("lp_be")

    W1 = g("fus_W1")
    g2, be2 = g("n2_g"), g("n2_be")
    A1, negc = [], []
    bias_total = g("fus_b1").copy()
    for s in range(3):
        blk = W1[s * D:(s + 1) * D]
        A = g2[s][:, None] * blk
        c = blk.T @ g2[s]
        A1.append(A - A.mean(axis=1, keepdims=True))
        negc.append(-(c - c.mean()))
        bias_total += be2[s] @ blk
    L1 = []
    for s in range(3):
        off = 3 * D + s * (D // 2)
        blk = W1[off: off + D // 2]
        L1.append(blk - blk.mean(axis=1, keepdims=True))
    out["w_f1"] = np.stack(A1)
    out["negc_f1"] = np.stack(negc)
    out["w_f1l"] = np.stack(L1)
    out["b_f1"] = bias_total - bias_total.mean()
    out["g_f1"], out["be_f1"] = g("fus_g1"), g("fus_ge1")

    W2c, b2c = _center_cols(g("fus_W2"), g("fus_b2"))
    out["w_f2"], out["b_f2"] = W2c, b2c
    out["g_f2"], out["be_f2"] = g("fus_g2"), g("fus_ge2")
    return out


def _vec_pp(v, nk):
    """[.., nk*128] feature vector -> ACT per-partition layout [.., 128, nk]."""
    v = np.asarray(v, np.float32)
    return np.ascontiguousarray(v.reshape(v.shape[:-1] + (nk, 128)).swapaxes(-1, -2))


def _mf_np(v):
    """Host array in the matmul dtype (bf16 or fp32)."""
    if MM_IS_BF16:
        import ml_dtypes
        return np.ascontiguousarray(np.asarray(v, np.float32).astype(
            ml_dtypes.bfloat16))
    return np.ascontiguousarray(np.asarray(v, np.float32))


def device_arrays(fw):
    """Folded weights -> dict of arrays matching the DRAM tensor decls."""
    f32 = _mf_np
    dev = {}
    dev["w_hp"] = f32(fw["w_hp"].reshape(3, 8, 128, 512))
    dev["b_hp"] = _vec_pp(fw["b_hp"], 4)
    dev["w_r"] = f32(fw["w_r"].reshape(4, 128, 512))
    dev["b_r"] = _vec_pp(fw["b_r"], 4)
    dev["w_m"] = f32(fw["w_m"].reshape(3, 8, 128, 512))
    dev["b_m"] = _vec_pp(fw["b_m"], 4)
    dev["w_g"] = f32(fw["w_g"].reshape(3, 8, 128, 512))
    for s in range(3):
        dev[f"w_lp{s}"] = f32(fw["w_lp"][s])
    dev["b_lp"] = _vec_pp(fw["b_lp"], 2)
    dev["w_f1"] = f32(fw["w_f1"].reshape(3, 4, 128, 512))
    dev["negc_f1"] = f32(fw["negc_f1"][None])
    dev["w_f1l"] = f32(fw["w_f1l"].reshape(3, 2, 128, 512))
    dev["b_f1"] = _vec_pp(fw["b_f1"], 4)
    dev["w_f2"] = f32(fw["w_f2"].reshape(4, 128, 512))
    dev["b_f2"] = _vec_pp(fw["b_f2"], 4)
    for name in ("g_hp", "be_hp", "g_n1", "be_n1"):
        dev[name] = _vec_pp(fw[name], 4)
    dev["b_g_half"] = _vec_pp(fw["b_g_half"], 4)
    dev["g_lp"] = _vec_pp(fw["g_lp"], 2)
    dev["be_lp"] = _vec_pp(fw["be_lp"], 2)
    for name in ("g_f1", "be_f1", "g_f2", "be_f2"):
        dev[name] = _vec_pp(fw[name], 4)
    dev["ones_col"] = _mf_np(np.ones((128, 1), np.float32))
    dev["ones_row"] = _mf_np(np.ones((1, 128), np.float32))
    return dev


# --------------------------------------------------------------------------
# Device program
# --------------------------------------------------------------------------

ACT = mybir.ActivationFunctionType
ALU = mybir.AluOpType


def emit_program(tc, io):
    nc = tc.nc
    from contextlib import ExitStack
    ctx = ExitStack()

    # ---------------- pools ----------------
    P = lambda name, bufs, space="SBUF": ctx.enter_context(
        tc.tile_pool(name=name, bufs=bufs, space=space))
    const = P("const", 1)
    wpool = P("wchunk", 11)     # [128,2,512] MF weight pair chunks
    #                             (f1 holds 9 pairs + f2 prefetch)
    xpool = P("xchunk", 8)      # [128,2,512] MF input pair chunks
    #                             (all 8 live across both m-halves of a unit)
    sqp = P("sq", 7)            # [128,512] MF squares (live until deferred
    #                             stats matmuls run, a unit later)
    thp = P("th", 2)            # [128,512] MF gate tanh tiles
    zp = P("z", 2)              # [128,512] MF z = y*istd tiles
    up = P("u", 2)              # [128,512] MF gate (th+1)*m tiles
    yhp = P("yh", 6)            # [128,4,512] MF hp outputs (alive hp->r)
    ep = P("e", 6)              # [128,4,512] MF n1 outputs (alive r->gate)
    mp = P("m", 4)              # [128,4,512] MF pair-average (alive m->gate)
    tp = P("t", 6)              # [128,4,512] MF gate t; normalized in place,
    #                             alive until f1 consumes it
    lpo = P("l", 6)             # [128,2,512] MF lp outputs (alive ->f1)
    hp_ = P("h", 2)             # [128,4,512] MF f1 outputs
    op_ = P("o", 2)             # [128,4,512] MF f2 outputs
    lsp = P("ls", 1)            # [Fs,1024] MF host-sigmoided logits
    stf = P("stats_f32", 6)     # [1,512] F32 stat rows
    stb = P("stats_mf", 8)      # [1,512] MF istd/wrow rows (gate wrow rows
    #                             stay live until f1's rank-1 matmuls)
    nrp = P("nr", 4)            # [128,8] F32 NR tiles
    bcp = P("bc_sb", 2)         # [128,512] MF broadcast rows
    mm_ps = P("mm_ps", 5, "PSUM")
    st_ps = P("st_ps", 3, "PSUM")

    # ---------------- DMA helpers (needed for the head prefetch) --------
    def wpair(dram_pair_ap):
        """Load two [128,512] k-chunks in one DMA -> [128,2,512] tile."""
        wc = wpool.tile([128, 2, 512], MF, name="wcp", tag="wcp")
        nc.sync.dma_start(wc[:], dram_pair_ap.rearrange("c p n -> p c n"))
        return wc

    def load_wchunks(dram_4d, nk):
        """nk k-chunks -> list of per-chunk lhsT accessors f(m)->[128,128]."""
        fns = []
        for c0 in range(0, nk, 2):
            wc = wpair(dram_4d[c0:c0 + 2])
            for cc in range(2):
                fns.append(lambda m, wc=wc, cc=cc: wc[:, cc, ts(m, 128)])
        return fns

    def load_x(s):
        xcs = []
        for bt in range(2):
            for c0 in range(0, 8, 2):
                xc = xpool.tile([128, 2, 512], MF, name="xc")
                nc.sync.dma_start(
                    xc[:],
                    io[f"xT{s}"][ts(c0 // 2, 256), ts(bt, 512)].rearrange(
                        "(c p) b -> p c b", p=128))
                xcs.append(xc)
        return xcs

    # The Sync engine issues DMAs in emission order at ~0.7us apiece, so
    # the first compute units' inputs must be first in the queue.
    x_pf = [load_x(0)]
    whp_pf = [load_wchunks(io["w_hp"][0], 8)]
    lsg = []
    for s in range(3):
        t = lsp.tile([FS[s], 1024], MF, name=f"lsg{s}")
        nc.sync.dma_start(t[:], io[f"lT{s}"])
        lsg.append(t)

    # ---------------- constants / resident weights ----------------
    ident = const.tile([128, 128], F32)
    make_identity(nc, ident)
    ones_col = const.tile([128, 1], MF)
    nc.sync.dma_start(ones_col[:], io["ones_col"])
    ones_row = const.tile([1, 128], MF)
    nc.sync.dma_start(ones_row[:], io["ones_row"])

    def fconst(value, name):
        t = const.tile([1, 1], F32, name=name)
        nc.gpsimd.memset(t[:], value)
        return t
    eps1 = fconst(EPS, "eps1")
    inv_d = fconst(1.0 / D, "inv_d")
    inv_d2 = fconst(2.0 / D, "inv_d2")

    def load(name, shape, rearr=None, dtype=F32):
        t = const.tile(shape, dtype, name=name)
        src = io[name]
        if rearr:
            src = src.rearrange(rearr)
        nc.sync.dma_start(t[:], src)
        return t

    b_hp = load("b_hp", [128, 3, 4], "s p c -> p s c")
    b_r = load("b_r", [128, 4])
    b_m = load("b_m", [128, 3, 4], "s p c -> p s c")
    b_lp = load("b_lp", [128, 3, 2], "s p c -> p s c")
    b_f1 = load("b_f1", [128, 4])
    b_f2 = load("b_f2", [128, 4])
    g_hp = load("g_hp", [128, 3, 4], "s p c -> p s c")
    be_hp = load("be_hp", [128, 3, 4], "s p c -> p s c")
    g_n1 = load("g_n1", [128, 3, 4], "s p c -> p s c")
    be_n1 = load("be_n1", [128, 3, 4], "s p c -> p s c")
    b_gh = load("b_g_half", [128, 3, 4], "s p c -> p s c")
    g_lp = load("g_lp", [128, 3, 2], "s p c -> p s c")
    be_lp = load("be_lp", [128, 3, 2], "s p c -> p s c")
    g_f1 = load("g_f1", [128, 4])
    be_f1 = load("be_f1", [128, 4])
    g_f2 = load("g_f2", [128, 4])
    be_f2 = load("be_f2", [128, 4])
    negc_t = load("negc_f1", [1, 3, 512], dtype=MF)
    w_lp = [load(f"w_lp{s}", [FS[s], 256], dtype=MF) for s in range(3)]
    # w_r is shared by all three r units: load once into the const pool.
    w_r_t = []
    for c0 in (0, 2):
        t = const.tile([128, 2, 512], MF, name=f"w_r{c0}")
        nc.sync.dma_start(t[:], io["w_r"][c0:c0 + 2].rearrange("c p n -> p c n"))
        w_r_t.append(t)
    wr_fns = [(lambda m, t=w_r_t[c // 2], cc=c % 2: t[:, cc, ts(m, 128)])
              for c in range(4)]

    # ---------------- helpers ----------------
    def mm_groups(srcs, n_m, evict_fn):
        """srcs: list of (lhsT_fn(m), rhs_fn(bt)). Emits matmuls in two
        m-halves; after each half's accumulation completes, evict_fn(bt, m,
        ps) is called. bt is innermost so consecutive matmuls share the
        stationary operand."""
        last = len(srcs) - 1
        for mh in range(0, n_m, 2):
            mis = range(mh, min(mh + 2, n_m))
            ps = {(m, bt): mm_ps.tile([128, 512], F32, name="mm", tag="mm")
                  for m in mis for bt in range(2)}
            for ci, (lf, rf) in enumerate(srcs):
                for m in mis:
                    for bt in range(2):
                        nc.tensor.matmul(ps[(m, bt)][:], lf(m), rf(bt),
                                         start=(ci == 0), stop=(ci == last))
            for bt in range(2):
                for m in mis:
                    evict_fn(bt, m, ps[(m, bt)])

    def transpose_rows(rows):
        """PE-transpose k [1,512] sbuf rows into one [128,4k] SBUF tile
        (via PSUM) so the per-sample scalar math runs on fat tiles."""
        k = len(rows)
        vT = st_ps.tile([128, 4 * k], F32, name="vT", tag="stat_ps")
        for c in range(4):
            for i, v in enumerate(rows):
                nc.tensor.transpose(vT[:, c * k + i:c * k + i + 1],
                                    v[0:1, ts(c, 128)], ident[0:1, 0:1])
        vs = nrp.tile([128, 4 * k], F32, name="nr_v")
        nc.vector.tensor_copy(vs[:], vT[:])
        return vs

    def nr_rsqrt_T(vs, k):
        """Newton-Raphson rsqrt of a transposed [128,4k] tile (GPSIMD)."""
        y = nrp.tile([128, 4 * k], F32, name="nr_y")
        t = nrp.tile([128, 4 * k], F32, name="nr_t")
        nc.vector.tensor_scalar(y[:].bitcast(I32), vs[:].bitcast(I32),
                                1, None, ALU.logical_shift_right)
        nc.vector.tensor_scalar(y[:].bitcast(I32), y[:].bitcast(I32),
                                -1, MAGIC, ALU.mult, ALU.add)
        for _ in range(NR_ITERS):
            nc.vector.tensor_mul(t[:], y[:], y[:])
            nc.vector.tensor_mul(t[:], t[:], vs[:])
            nc.vector.tensor_scalar(t[:], t[:], -0.5, 1.5, ALU.mult, ALU.add)
            nc.vector.tensor_mul(y[:], y[:], t[:])
        return y

    def row_back(y, k, i):
        """Transpose column set i of [128,4k] back to a [1,512] MF row."""
        rT = st_ps.tile([1, 512], F32, name="rT", tag="stat_ps")
        for c in range(4):
            nc.tensor.transpose(rT[0:1, ts(c, 128)],
                                y[:, c * k + i:c * k + i + 1], ident)
        row = stb.tile([1, 512], MF, name="r16")
        nc.vector.tensor_copy(row[:], rT[:])
        return row

    def bcast(row):
        """[1,512] row -> [128,512] MF tile via PE outer product."""
        bps = mm_ps.tile([128, 512], F32, name="mm", tag="mm")
        nc.tensor.matmul(bps[:], ones_row[:], row[0:1, :],
                         start=True, stop=True)
        bc = bcp.tile([128, 512], MF, name="bc")
        nc.scalar.activation(bc[:], bps[:], ACT.Identity)
        return bc

    # ---------------- unit emitters ----------------
    # Each unit emits its matmuls+evictions inline and returns
    # (stats_fn, fin_fn) closures to be sequenced by the main schedule.

    def ln_unit(srcs, n_m, bias_cols, gam_cols, bet_cols, func, out_pool,
                dim, mm_emitter=None):
        """Generic matmul->LN->activation unit over both batch tiles.
        PSUM is evicted (bias added) straight into the unit's output tile;
        the final activation overwrites the same slice in place.
        Returns (stats_fn, fin_a_fn, fin_b_fn, outs)."""
        outs = [None, None]

        def evict(bt, m, ps):
            if outs[bt] is None:
                outs[bt] = out_pool.tile([128, n_m, 512], MF, name="out")
            nc.vector.tensor_scalar_add(outs[bt][:, m, :], ps[:],
                                        bias_cols[m])

        (mm_emitter or mm_groups)(srcs, n_m, evict)
        st = [None, None]

        def stats():
            for bt in range(2):
                st[bt] = st_ps.tile([1, 512], F32, name="st", tag="stat_ps")
                for m in range(n_m):
                    sq = sqp.tile([128, 512], MF, name="sq")
                    nc.gpsimd.tensor_mul(sq[:], outs[bt][:, m, :],
                                         outs[bt][:, m, :])
                    nc.tensor.matmul(st[bt][:], ones_col[:], sq[:],
                                     start=(m == 0), stop=(m == n_m - 1))

        hold = {}

        def fin_a():
            vr = []
            for bt in range(2):
                v = stf.tile([1, 512], F32, name="r32")
                nc.scalar.activation(v[0:1, :], st[bt][:], ACT.Identity,
                                     bias=eps1[:],
                                     scale=(inv_d if dim == D else inv_d2)[:])
                vr.append(v)
            hold["y"] = nr_rsqrt_T(transpose_rows(vr), 2)

        def fin_b(bts=(0, 1)):
            for bt in bts:
                istd = row_back(hold["y"], 2, bt)
                bc = bcast(istd)
                for m in range(n_m):
                    z = zp.tile([128, 512], MF, name="z")
                    nc.vector.tensor_mul(z[:], outs[bt][:, m, :], bc[:])
                    nc.scalar.activation(outs[bt][:, m, :], z[:], func,
                                         bias=bet_cols[m], scale=gam_cols[m])

        return stats, fin_a, fin_b, outs

    def m_unit(s, e_tiles, m_streams, wfns):
        """Pair-average matmul; eviction only (adds bias)."""
        sa, sb = m_streams[s]
        srcs = []
        for c in range(8):
            if c < 4:
                rf = (lambda c: (lambda bt: e_tiles[sa][bt][:, c, :]))(c)
            else:
                rf = (lambda c: (lambda bt: e_tiles[sb][bt][:, c - 4, :]))(c)
            srcs.append((wfns[c], rf))
        m_sb = [mp.tile([128, 4, 512], MF, name="m_sb") for _ in range(2)]

        def evict(bt, m, ps):
            nc.vector.tensor_scalar_add(m_sb[bt][:, m, :], ps[:],
                                        b_m[:, s, m:m + 1])

        mm_groups(srcs, 4, evict)
        return m_sb

    def gate_unit(s, e_tiles, m_sb, wfns):
        """Gate matmul -> tanh-sigmoid -> t = e + gate*m -> n2 stats.
        zt = t*istd_bc - (mu*istd)_bc is produced in fin."""
        srcs = []
        for c in range(8):
            if c < 4:
                rf = (lambda c: (lambda bt: e_tiles[s][bt][:, c, :]))(c)
            else:
                rf = (lambda c: (lambda bt: m_sb[bt][:, c - 4, :]))(c)
            srcs.append((wfns[c], rf))
        t_sb = [tp.tile([128, 4, 512], MF, name="t_sb") for _ in range(2)]

        def evict(bt, m, ps):
            # t = e + sigmoid(pre)*m_avg = e + (th+1)*m_tilde
            th = thp.tile([128, 512], MF, name="th")
            nc.scalar.activation(th[:], ps[:], ACT.Tanh,
                                 bias=b_gh[:, s, m:m + 1], scale=0.5)
            u = up.tile([128, 512], MF, name="u")
            nc.vector.scalar_tensor_tensor(u[:], th[:], 1.0,
                                           m_sb[bt][:, m, :],
                                           ALU.add, ALU.mult)
            nc.gpsimd.tensor_add(t_sb[bt][:, m, :], u[:],
                                 e_tiles[s][bt][:, m, :])

        mm_groups(srcs, 4, evict)
        st_sum = [None, None]
        st_sq = [None, None]

        def stats():
            for bt in range(2):
                st_sum[bt] = st_ps.tile([1, 512], F32, name="st_sum",
                                        tag="stat_ps")
                for m in range(4):
                    nc.tensor.matmul(st_sum[bt][:], ones_col[:],
                                     t_sb[bt][:, m, :],
                                     start=(m == 0), stop=(m == 3))
            for bt in range(2):
                st_sq[bt] = st_ps.tile([1, 512], F32, name="st_sq",
                                       tag="stat_ps")
                for m in range(4):
                    sq = sqp.tile([128, 512], MF, name="sq")
                    nc.gpsimd.tensor_mul(sq[:], t_sb[bt][:, m, :],
                                         t_sb[bt][:, m, :])
                    nc.tensor.matmul(st_sq[bt][:], ones_col[:], sq[:],
                                     start=(m == 0), stop=(m == 3))

        hold = {}

        def fin_a():
            mu_rows, v_rows = [], []
            for bt in range(2):
                m_ = stf.tile([1, 512], F32, name="r32")
                nc.scalar.activation(m_[0:1, :], st_sum[bt][:], ACT.Identity,
                                     scale=inv_d[:])
                v = stf.tile([1, 512], F32, name="r32")
                nc.scalar.activation(v[0:1, :], st_sq[bt][:], ACT.Identity,
                                     bias=eps1[:], scale=inv_d[:])
                mu_rows.append(m_)
                v_rows.append(v)
            muS = transpose_rows(mu_rows)
            vS = transpose_rows(v_rows)
            musq = nrp.tile([128, 8], F32, name="nr_t")
            nc.vector.tensor_mul(musq[:], muS[:], muS[:])
            nc.vector.tensor_sub(vS[:], vS[:], musq[:])
            y = nr_rsqrt_T(vS, 2)
            wT = nrp.tile([128, 8], F32, name="nr_w")
            nc.vector.tensor_mul(wT[:], muS[:], y[:])
            hold["y"], hold["w"] = y, wT

        wrows = {}

        def fin_b(bts=(0, 1)):
            # zt = t*istd_bc in place; the -mu*istd mean correction is a
            # rank-1 negc matmul inside f1 (wrows are its rhs rows).
            for bt in bts:
                istd = row_back(hold["y"], 2, bt)
                wrows[bt] = row_back(hold["w"], 2, bt)
                bci = bcast(istd)
                for m in range(4):
                    nc.vector.tensor_mul(t_sb[bt][:, m, :],
                                         t_sb[bt][:, m, :], bci[:])

        return stats, fin_a, fin_b, t_sb, wrows

    # ---------------- unit constructors ----------------
    def make_lp(s):
        srcs = [(lambda m, s=s: w_lp[s][:, ts(m, 128)],
                 lambda bt, s=s: lsg[s][:, ts(bt, 512)])]
        return ln_unit(srcs, 2,
                       [b_lp[:, s, c:c + 1] for c in range(2)],
                       [g_lp[:, s, c:c + 1] for c in range(2)],
                       [be_lp[:, s, c:c + 1] for c in range(2)],
                       ACT.Gelu, lpo, D // 2)

    def make_hp(s, xcs, wfns):
        srcs = [(wfns[c],
                 (lambda c: (lambda bt: xcs[bt * 4 + c // 2][:, c % 2, :]))(c))
                for c in range(8)]
        return ln_unit(srcs, 4,
                       [b_hp[:, s, c:c + 1] for c in range(4)],
                       [g_hp[:, s, c:c + 1] for c in range(4)],
                       [be_hp[:, s, c:c + 1] for c in range(4)],
                       ACT.Gelu, yhp, D)

    def make_r(s, yh):
        srcs = [(wr_fns[c], (lambda c: (lambda bt: yh[bt][:, c, :]))(c))
                for c in range(4)]
        return ln_unit(srcs, 4,
                       [b_r[:, c:c + 1] for c in range(4)],
                       [g_n1[:, s, c:c + 1] for c in range(4)],
                       [be_n1[:, s, c:c + 1] for c in range(4)],
                       ACT.Identity, ep, D)

    def prefetch_f1():
        fns = []
        for s in range(3):
            fns.append(load_wchunks(io["w_f1l"][s], 2))
        for s in (2, 1, 0):
            fns.append(load_wchunks(io["w_f1"][s], 4))
        return fns

    def make_f1(l_tiles, zt_tiles, gate_fbs, wf, wrows_by_s):
        """f1 with the three gate fin_b's interleaved between chunk stages:
        l chunks first, then g2.fb, zt2 chunks, g1.fb, zt1, g0.fb, zt0,
        and the rank-1 mean-correction (negc x mu*istd rows) last."""
        srcs = []
        for s in range(3):
            for c in range(2):
                srcs.append((wf[s][c],
                             (lambda s, c: (lambda bt: l_tiles[s][bt][:, c, :]))(s, c)))
        for i, s in enumerate((2, 1, 0)):
            for c in range(4):
                srcs.append((wf[3 + i][c],
                             (lambda s, c: (lambda bt: zt_tiles[s][bt][:, c, :]))(s, c)))
        for s in (2, 1, 0):
            srcs.append(((lambda m, s=s: negc_t[0:1, s, ts(m, 128)]),
                         (lambda s=s: (lambda bt: wrows_by_s[s][bt][0:1, :]))()))
        fb_at = {6: gate_fbs[2], 10: gate_fbs[1], 14: gate_fbs[0]}

        def emitter(srcs, n_m, evict_fn):
            last = len(srcs) - 1
            for mh in range(0, n_m, 2):
                mis = range(mh, mh + 2)
                ps = {(m, bt): mm_ps.tile([128, 512], F32, name="mm",
                                          tag="mm")
                      for m in mis for bt in range(2)}
                for ci, (lf, rf) in enumerate(srcs):
                    if mh == 0 and ci in fb_at:
                        fb_at[ci]()
                    for m in mis:
                        for bt in range(2):
                            nc.tensor.matmul(ps[(m, bt)][:], lf(m), rf(bt),
                                             start=(ci == 0),
                                             stop=(ci == last))
                for bt in range(2):
                    for m in mis:
                        evict_fn(bt, m, ps[(m, bt)])

        return ln_unit(srcs, 4,
                       [b_f1[:, c:c + 1] for c in range(4)],
                       [g_f1[:, c:c + 1] for c in range(4)],
                       [be_f1[:, c:c + 1] for c in range(4)],
                       ACT.Gelu, hp_, D, mm_emitter=emitter)

    # ---------------- main schedule ----------------
    # Emission order == per-engine execution order (all engines run their
    # queues in order). Each unit's fin is split: fin_a (stat eviction +
    # transposes + NR chain) is emitted early so its latency runs under
    # later matmul blocks; fin_b (back-transposes + broadcast + normalize)
    # is emitted just before the consumer needs the result.
    m_streams = [(1, 2), (0, 2), (0, 1)]

    x_pf.append(load_x(1))
    whp_pf.append(load_wchunks(io["w_hp"][1], 8))
    hp0 = make_hp(0, x_pf[0], whp_pf[0])
    lp_u = [make_lp(s) for s in range(3)]
    x_pf.append(load_x(2))
    whp_pf.append(load_wchunks(io["w_hp"][2], 8))
    hp1 = make_hp(1, x_pf[1], whp_pf[1])
    hp0[0]()                   # hp0 stats
    for s in range(3):
        lp_u[s][0]()           # lp stats
    hp0[1]()                   # hp0 fin_a
    for s in range(3):
        lp_u[s][1]()           # lp fin_a
    hp2 = make_hp(2, x_pf[2], whp_pf[2])
    wf_m2 = load_wchunks(io["w_m"][2], 8)
    hp1[0]()
    hp0[2]()                   # hp0 fin_b -> yh0
    for s in range(3):
        lp_u[s][2]()           # lp fin_b -> l
    r0 = make_r(0, hp0[3])
    hp1[1]()
    hp2[0]()
    hp1[2]()                   # -> yh1
    r1 = make_r(1, hp1[3])
    wf_m1 = load_wchunks(io["w_m"][1], 8)
    r0[0]()
    hp2[1]()
    hp2[2]()                   # -> yh2
    r0[1]()
    r2 = make_r(2, hp2[3])
    wf_g2 = load_wchunks(io["w_g"][2], 8)
    r1[0]()
    r0[2]()                    # -> e0
    r1[1]()
    r1[2]()                    # -> e1
    e_tiles = [r0[3], r1[3], r2[3]]
    m2 = m_unit(2, e_tiles, m_streams, wf_m2)   # e0 (c0-3), e1 (c4-7)
    wf_m0 = load_wchunks(io["w_m"][0], 8)
    r2[0]()
    r2[1]()
    r2[2]()                    # -> e2
    m1 = m_unit(1, e_tiles, m_streams, wf_m1)   # e0 (c0-3), e2 (c4-7)
    wf_g1 = load_wchunks(io["w_g"][1], 8)
    g2 = gate_unit(2, e_tiles, m2, wf_g2)
    m0 = m_unit(0, e_tiles, m_streams, wf_m0)   # e1, e2
    wf_g0 = load_wchunks(io["w_g"][0], 8)
    g2[0]()                    # g2 stats
    g1 = gate_unit(1, e_tiles, m1, wf_g1)
    g2[1]()                    # g2 fin_a
    wf_f1 = prefetch_f1()
    g0 = gate_unit(0, e_tiles, m0, wf_g0)
    wf_f2 = load_wchunks(io["w_f2"], 4)
    g1[0]()
    g0[0]()
    g1[1]()                    # g1 fin_a
    g0[1]()                    # g0 fin_a
    l_tiles = [u[3] for u in lp_u]
    zt_tiles = [g0[3], g1[3], g2[3]]
    f1 = make_f1(l_tiles, zt_tiles, [g0[2], g1[2], g2[2]], wf_f1,
                 {0: g0[4], 1: g1[4], 2: g2[4]})
    f1[0]()
    f1[1]()

    # ---- f2 (final LN), pipelined per batch tile with f1's fin_b ----
    h_tiles = f1[3]
    o_tiles = [None, None]
    f2st = [None, None]
    f2hold = {}

    def f2_mm(bt):
        o_tiles[bt] = op_.tile([128, 4, 512], MF, name="o_sb")
        ps = [mm_ps.tile([128, 512], F32, name="mm", tag="mm")
              for _ in range(4)]
        for ci in range(4):
            for m in range(4):
                nc.tensor.matmul(ps[m][:], wf_f2[ci](m),
                                 h_tiles[bt][:, ci, :],
                                 start=(ci == 0), stop=(ci == 3))
        for m in range(4):
            nc.vector.tensor_scalar_add(o_tiles[bt][:, m, :], ps[m][:],
                                        b_f2[:, m:m + 1])

    def f2_stats(bt):
        f2st[bt] = st_ps.tile([1, 512], F32, name="st", tag="stat_ps")
        for m in range(4):
            sq = sqp.tile([128, 512], MF, name="sq")
            nc.gpsimd.tensor_mul(sq[:], o_tiles[bt][:, m, :],
                                 o_tiles[bt][:, m, :])
            nc.tensor.matmul(f2st[bt][:], ones_col[:], sq[:],
                             start=(m == 0), stop=(m == 3))

    def f2_fa():
        vr = []
        for bt in range(2):
            v = stf.tile([1, 512], F32, name="r32")
            nc.scalar.activation(v[0:1, :], f2st[bt][:], ACT.Identity,
                                 bias=eps1[:], scale=inv_d[:])
            vr.append(v)
        f2hold["y"] = nr_rsqrt_T(transpose_rows(vr), 2)

    def f2_fb(bt):
        istd = row_back(f2hold["y"], 2, bt)
        bc = bcast(istd)
        for m in range(4):
            z = zp.tile([128, 512], MF, name="z")
            nc.vector.tensor_mul(z[:], o_tiles[bt][:, m, :], bc[:])
            nc.scalar.activation(o_tiles[bt][:, m, :], z[:], ACT.Identity,
                                 bias=be_f2[:, m:m + 1],
                                 scale=g_f2[:, m:m + 1])
        nc.sync.dma_start(
            io["outT"].rearrange("(c p) b -> p c b", p=128)[:, :, ts(bt, 512)],
            o_tiles[bt][:])

    f1[2]((0,))                # -> h[bt0]
    f2_mm(0)
    f1[2]((1,))                # -> h[bt1]
    f2_mm(1)
    f2_stats(0)
    f2_stats(1)
    f2_fa()
    f2_fb(0)
    f2_fb(1)

    ctx.close()


def build_program():
    nc = bacc.Bacc("TRN2", target_bir_lowering=False, debug=False,
                   num_devices=NCORES)
    io = {}

    def din(name, shape, dtype=F32):
        io[name] = nc.dram_tensor(name, list(shape), dtype,
                                  kind="ExternalInput").ap()

    for s in range(3):
        din(f"xT{s}", (HID, BL), dtype=MM_DT)
        din(f"lT{s}", (FS[s], BL), dtype=MM_DT)
    din("w_hp", (3, 8, 128, 512), dtype=MM_DT)
    din("b_hp", (3, 128, 4))
    din("w_r", (4, 128, 512), dtype=MM_DT)
    din("b_r", (128, 4))
    din("w_m", (3, 8, 128, 512), dtype=MM_DT)
    din("b_m", (3, 128, 4))
    din("w_g", (3, 8, 128, 512), dtype=MM_DT)
    for s in range(3):
        din(f"w_lp{s}", (FS[s], 256), dtype=MM_DT)
    din("b_lp", (3, 128, 2))
    din("w_f1", (3, 4, 128, 512), dtype=MM_DT)
    din("negc_f1", (1, 3, 512), dtype=MM_DT)
    din("w_f1l", (3, 2, 128, 512), dtype=MM_DT)
    din("b_f1", (128, 4))
    din("w_f2", (4, 128, 512), dtype=MM_DT)
    din("b_f2", (128, 4))
    for name in ("g_hp", "be_hp", "g_n1", "be_n1", "b_g_half"):
        din(name, (3, 128, 4))
    for name in ("g_lp", "be_lp"):
        din(name, (3, 128, 2))
    for name in ("g_f1", "be_f1", "g_f2", "be_f2"):
        din(name, (128, 4))
    din("ones_col", (128, 1), dtype=MM_DT)
    din("ones_row", (1, 128), dtype=MM_DT)
    io["outT"] = nc.dram_tensor("outT", [D, BL], MM_DT,
                                kind="ExternalOutput").ap()

    with tile.TileContext(nc) as tc:
        emit_program(tc, io)
    nc.compile()
    return nc


def make_in_maps(inputs):
    fw = fold_weights(inputs)
    dev = device_arrays(fw)
    hidden = [np.asarray(inputs["verb_hidden"], np.float32),
              np.asarray(inputs["inst_hidden"], np.float32),
              np.asarray(inputs["target_hidden"], np.float32)]
    logits = [np.asarray(inputs["verb_logits"], np.float32),
              np.asarray(inputs["inst_logits"], np.float32),
              np.asarray(inputs["target_logits"], np.float32)]
    sig = [1.0 / (1.0 + np.exp(-np.asarray(l, F64))) for l in logits]
    in_maps = []
    for core in range(NCORES):
        rows = slice(core * BL, (core + 1) * BL)
        m = dict(dev)
        for s in range(3):
            m[f"xT{s}"] = _mf_np(hidden[s][rows].T)
            m[f"lT{s}"] = _mf_np(sig[s][rows].T)
        in_maps.append(m)
    return in_maps


_NC_CACHE = None


def _run(inputs, **spmd_kwargs):
    global _NC_CACHE
    if _NC_CACHE is None:
        _NC_CACHE = build_program()
    nc = _NC_CACHE
    in_maps = make_in_maps(inputs)
    res = run_bass_kernel_spmd(nc, in_maps, list(range(NCORES)),
                               **spmd_kwargs)
    out = np.empty((B, D), dtype=np.float32)
    for core in range(NCORES):
        out[core * BL:(core + 1) * BL] = np.asarray(
            res.results[core]["outT"], dtype=np.float32).T
    return out, res


def kernel(**inputs) -> np.ndarray:
    return _run(inputs)[0]


def kernel_profiled(inputs, tmpdir=None):
    """Returns (out, BassKernelResults) with an NTFF-based profile."""
    return _run(inputs, trace=True, tmpdir=tmpdir)


# revision 91
# speedup vs baseline: 1.1801x; 1.1801x over previous
"""Trainium2 Bass kernel for nn_AttentionModule_7146825580577.

Strategy (see spec sharding_hint): pure data parallel over the batch dim
(8192 rows -> 1024 rows per core, 8 cores), weights replicated.

v2 schedule: the two 512-column batch tiles of each core are interleaved
inside every layer unit so the Tensor engine always has independent
matmul work queued (keeps the PE out of its low-clock pstates), weights
are streamed from HBM once (each chunk feeds both batch tiles), all
matmul operands are bf16, the gate sigmoid is computed via tanh (so the
scalar engine never has to swap activation tables), the logit sigmoid is
folded into the host-side preprocessing, and the rsqrt Newton iteration
runs on the otherwise-idle GPSIMD engine over PE-transposed stat tiles.

Device math (per core), in feature-transposed layout (features on SBUF
partitions, batch on the free dim):

  - All LayerNorms whose input is an affine function of a previous
    activation use host-side column-centered weights, so mean(y) == 0 by
    construction and only sum(y^2) is needed on device (computed by a
    ones-vector matmul on the PE, reduced over partitions).
  - seq_len==1 MHA reduces to out_proj(v_proj(kv)); both projections are
    fused on the host into a single 512x512 effective matrix. The self-
    attention residual (x + sa(x)) is folded into a single matmul with
    weights I + Wv@Wo.
  - The cross-attention pair average (a+b)/2 is a single concat-matmul.
  - The n2 LayerNorm (after gating) is folded into the fus_W1 matmul:
    gamma scales fold into the weights, betas fold into the bias; the
    per-sample mean correction is applied as zt = t*istd_bc - wrow_bc
    with both rows partition-broadcast on GPSIMD.
  - 1/sqrt(var+eps) uses the int32 bit trick + Newton-Raphson on GPSIMD,
    on PE-transposed [128, k] stat tiles so each op is tiny.
"""
import os
import sys

sys.path.insert(0, "/opt/trn_rl_repo")

import numpy as np

import concourse.bass as bass
import concourse.tile as tile
from concourse import bacc, mybir
from concourse.bass import ts
from concourse.bass_utils import run_bass_kernel_spmd
from concourse.masks import make_identity

D = 512
HID = 1024
B = 8192
NCORES = 8
BL = B // NCORES          # rows per core
EPS = 1e-5
MAGIC = 0x5F3759DF
F32 = mybir.dt.float32
I32 = mybir.dt.int32
FS = [10, 6, 15]          # logit dims per stream
NR_ITERS = int(os.environ.get("KERNEL_NR_ITERS", "2"))
MM_DT = {
    "f32r": mybir.dt.float32r,
    "f32": mybir.dt.float32,
    "bf16": mybir.dt.bfloat16,
}[os.environ.get("KERNEL_MM_DTYPE", "bf16")]
MM_IS_BF16 = MM_DT == mybir.dt.bfloat16
MF = MM_DT

F64 = np.float64


# --------------------------------------------------------------------------
# Host-side weight folding
# --------------------------------------------------------------------------

def _center_cols(W, b):
    W = np.asarray(W, F64)
    b = np.asarray(b, F64)
    return W - W.mean(axis=1, keepdims=True), b - b.mean()


def fold_weights(inp):
    g = lambda k: np.asarray(inp[k], dtype=F64)
    out = {}

    w_hp, b_hp = [], []
    for s in range(3):
        W, b = _center_cols(g("hp_W")[s], g("hp_b")[s])
        w_hp.append(W)
        b_hp.append(b)
    out["w_hp"] = np.stack(w_hp)
    out["b_hp"] = np.stack(b_hp)
    out["g_hp"], out["be_hp"] = g("hp_g"), g("hp_be")

    mhaW, mhab = g("mha_in_W"), g("mha_in_b")
    moW, mob = g("mha_out_W"), g("mha_out_b")
    Wv0, bv0 = mhaW[0][:, 2 * D:], mhab[0][2 * D:]
    Wr, br = _center_cols(np.eye(D) + Wv0 @ moW[0], bv0 @ moW[0] + mob[0])
    out["w_r"], out["b_r"] = Wr, br
    out["g_n1"], out["be_n1"] = g("n1_g"), g("n1_be")

    Wj, bj = [None] * 4, [None] * 4
    for j in (1, 2, 3):
        Wv, bv = mhaW[j][:, 2 * D:], mhab[j][2 * D:]
        Wj[j] = Wv @ moW[j]
        bj[j] = bv @ moW[j] + mob[j]
    # m_verb uses (inst_e @ W1, target_e @ W2); m_inst (verb @ W1, target @ W3);
    # m_target (verb @ W2, inst @ W3)
    # m is stored pre-halved (0.25 = average 0.5 x sigmoid-via-tanh 0.5):
    # t = e + sigmoid(pre)*m_avg = (e + m_tilde) + tanh(pre/2)*m_tilde
    # with m_tilde = 0.5*m_avg.
    mods = [(1, 2), (1, 3), (2, 3)]
    w_m, b_m = [], []
    for s in range(3):
        ja, jb = mods[s]
        w_m.append(np.concatenate([0.25 * Wj[ja], 0.25 * Wj[jb]], axis=0))
        b_m.append(0.25 * (bj[ja] + bj[jb]))
    out["w_m"] = np.stack(w_m)
    out["b_m"] = np.stack(b_m)

    out["w_g"] = g("gate_W")
    # tanh trick: sigmoid(x + b) = 0.5*tanh(0.5*x + 0.5*b) + 0.5
    out["b_g_half"] = 0.5 * g("gate_b")

    w_lp, b_lp = [], []
    for s, key in enumerate(["verb", "inst", "target"]):
        W, b = _center_cols(g(f"lp_W_{key}"), g(f"lp_b_{key}"))
        w_lp.append(W)
        b_lp.append(b)
    out["w_lp"] = w_lp
    out["b_lp"] = np.stack(b_lp)
    out["g_lp"], out["be_lp"] = g("lp_g"), g("lp_be")

    W1 = g("fus_W1")
    g2, be2 = g("n2_g"), g("n2_be")
    A1, negc = [], []
    bias_total = g("fus_b1").copy()
    for s in range(3):
        blk = W1[s * D:(s + 1) * D]
        A = g2[s][:, None] * blk
        c = blk.T @ g2[s]
        A1.append(A - A.mean(axis=1, keepdims=True))
        negc.append(-(c - c.mean()))
        bias_total += be2[s] @ blk
    L1 = []
    for s in range(3):
        off = 3 * D + s * (D // 2)
        blk = W1[off: off + D // 2]
        L1.append(blk - blk.mean(axis=1, keepdims=True))
    out["w_f1"] = np.stack(A1)
    out["negc_f1"] = np.stack(negc)
    out["w_f1l"] = np.stack(L1)
    out["b_f1"] = bias_total - bias_total.mean()
    out["g_f1"], out["be_f1"] = g("fus_g1"), g("fus_ge1")

    W2c, b2c = _center_cols(g("fus_W2"), g("fus_b2"))
    out["w_f2"], out["b_f2"] = W2c, b2c
    out["g_f2"], out["be_f2"] = g("fus_g2"), g("fus_ge2")
    return out


def _vec_pp(v, nk):
    """[.., nk*128] feature vector -> ACT per-partition layout [.., 128, nk]."""
    v = np.asarray(v, np.float32)
    return np.ascontiguousarray(v.reshape(v.shape[:-1] + (nk, 128)).swapaxes(-1, -2))


def _mf_np(v):
    """Host array in the matmul dtype (bf16 or fp32)."""
    if MM_IS_BF16:
        import ml_dtypes
        return np.ascontiguousarray(np.asarray(v, np.float32).astype(
            ml_dtypes.bfloat16))
    return np.ascontiguousarray(np.asarray(v, np.float32))


def device_arrays(fw):
    """Folded weights -> dict of arrays matching the DRAM tensor decls."""
    f32 = _mf_np
    dev = {}
    dev["w_hp"] = f32(fw["w_hp"].reshape(3, 8, 128, 512))
    dev["b_hp"] = _vec_pp(fw["b_hp"], 4)
    dev["w_r"] = f32(fw["w_r"].reshape(4, 128, 512))
    dev["b_r"] = _vec_pp(fw["b_r"], 4)
    dev["w_m"] = f32(fw["w_m"].reshape(3, 8, 128, 512))
    dev["b_m"] = _vec_pp(fw["b_m"], 4)
    dev["w_g"] = f32(fw["w_g"].reshape(3, 8, 128, 512))
    for s in range(3):
        dev[f"w_lp{s}"] = f32(fw["w_lp"][s])
    dev["b_lp"] = _vec_pp(fw["b_lp"], 2)
    dev["w_f1"] = f32(fw["w_f1"].reshape(3, 4, 128, 512))
    dev["negc_f1"] = f32(fw["negc_f1"][None])
    dev["w_f1l"] = f32(fw["w_f1l"].reshape(3, 2, 128, 512))
    dev["b_f1"] = _vec_pp(fw["b_f1"], 4)
    dev["w_f2"] = f32(fw["w_f2"].reshape(4, 128, 512))
    dev["b_f2"] = _vec_pp(fw["b_f2"], 4)
    for name in ("g_hp", "be_hp", "g_n1", "be_n1"):
        dev[name] = _vec_pp(fw[name], 4)
    dev["b_g_half"] = _vec_pp(fw["b_g_half"], 4)
    dev["g_lp"] = _vec_pp(fw["g_lp"], 2)
    dev["be_lp"] = _vec_pp(fw["be_lp"], 2)
    for name in ("g_f1", "be_f1", "g_f2", "be_f2"):
        dev[name] = _vec_pp(fw[name], 4)
    dev["ones_col"] = _mf_np(np.ones((128, 1), np.float32))
    dev["ones_row"] = _mf_np(np.ones((1, 128), np.float32))
    return dev


# --------------------------------------------------------------------------
# Device program
# --------------------------------------------------------------------------

ACT = mybir.ActivationFunctionType
ALU = mybir.AluOpType


def emit_program(tc, io):
    nc = tc.nc
    from contextlib import ExitStack
    ctx = ExitStack()

    # ---------------- pools ----------------
    P = lambda name, bufs, space="SBUF": ctx.enter_context(
        tc.tile_pool(name=name, bufs=bufs, space=space))
    const = P("const", 1)
    wpool = P("wchunk", 11)     # [128,2,512] MF weight pair chunks
    #                             (f1 holds 9 pairs + f2 prefetch)
    xpool = P("xchunk", 8)      # [128,2,512] MF input pair chunks
    #                             (all 8 live across both m-halves of a unit)
    sqp = P("sq", 7)            # [128,512] MF squares (live until deferred
    #                             stats matmuls run, a unit later)
    thp = P("th", 2)            # [128,512] MF gate tanh tiles
    zp = P("z", 2)              # [128,512] MF z = y*istd tiles
    up = P("u", 2)              # [128,512] MF gate (th+1)*m tiles
    yhp = P("yh", 6)            # [128,4,512] MF hp outputs (alive hp->r)
    ep = P("e", 6)              # [128,4,512] MF n1 outputs (alive r->gate)
    mp = P("m", 4)              # [128,4,512] MF pair-average (alive m->gate)
    tp = P("t", 6)              # [128,4,512] MF gate t; normalized in place,
    #                             alive until f1 consumes it
    lpo = P("l", 6)             # [128,2,512] MF lp outputs (alive ->f1)
    hp_ = P("h", 2)             # [128,4,512] MF f1 outputs
    op_ = P("o", 2)             # [128,4,512] MF f2 outputs
    lsp = P("ls", 1)            # [Fs,1024] MF host-sigmoided logits
    stf = P("stats_f32", 6)     # [1,512] F32 stat rows
    stb = P("stats_mf", 8)      # [1,512] MF istd/wrow rows (gate wrow rows
    #                             stay live until f1's rank-1 matmuls)
    nrp = P("nr", 4)            # [128,8] F32 NR tiles
    bcp = P("bc_sb", 2)         # [128,512] MF broadcast rows
    mm_ps = P("mm_ps", 4, "PSUM")
    st_ps = P("st_ps", 4, "PSUM")

    # ---------------- DMA helpers (needed for the head prefetch) --------
    def wpair(dram_pair_ap):
        """Load two [128,512] k-chunks in one DMA -> [128,2,512] tile."""
        wc = wpool.tile([128, 2, 512], MF, name="wcp", tag="wcp")
        nc.sync.dma_start(wc[:], dram_pair_ap.rearrange("c p n -> p c n"))
        return wc

    def load_wchunks(dram_4d, nk):
        """nk k-chunks -> list of per-chunk lhsT accessors f(m)->[128,128]."""
        fns = []
        for c0 in range(0, nk, 2):
            wc = wpair(dram_4d[c0:c0 + 2])
            for cc in range(2):
                fns.append(lambda m, wc=wc, cc=cc: wc[:, cc, ts(m, 128)])
        return fns

    def load_x(s):
        xcs = []
        for bt in range(2):
            for c0 in range(0, 8, 2):
                xc = xpool.tile([128, 2, 512], MF, name="xc")
                nc.sync.dma_start(
                    xc[:],
                    io[f"xT{s}"][ts(c0 // 2, 256), ts(bt, 512)].rearrange(
                        "(c p) b -> p c b", p=128))
                xcs.append(xc)
        return xcs

    # The Sync engine issues DMAs in emission order at ~0.7us apiece, so
    # the first compute units' inputs must be first in the queue.
    x_pf = [load_x(0)]
    whp_pf = [load_wchunks(io["w_hp"][0], 8)]
    lsg = []
    for s in range(3):
        t = lsp.tile([FS[s], 1024], MF, name=f"lsg{s}")
        nc.sync.dma_start(t[:], io[f"lT{s}"])
        lsg.append(t)

    # ---------------- constants / resident weights ----------------
    ident = const.tile([128, 128], F32)
    make_identity(nc, ident)
    ones_col = const.tile([128, 1], MF)
    nc.sync.dma_start(ones_col[:], io["ones_col"])
    ones_row = const.tile([1, 128], MF)
    nc.sync.dma_start(ones_row[:], io["ones_row"])

    def fconst(value, name):
        t = const.tile([1, 1], F32, name=name)
        nc.gpsimd.memset(t[:], value)
        return t
    eps1 = fconst(EPS, "eps1")
    inv_d = fconst(1.0 / D, "inv_d")
    inv_d2 = fconst(2.0 / D, "inv_d2")

    def load(name, shape, rearr=None, dtype=F32):
        t = const.tile(shape, dtype, name=name)
        src = io[name]
        if rearr:
            src = src.rearrange(rearr)
        nc.sync.dma_start(t[:], src)
        return t

    b_hp = load("b_hp", [128, 3, 4], "s p c -> p s c")
    b_r = load("b_r", [128, 4])
    b_m = load("b_m", [128, 3, 4], "s p c -> p s c")
    b_lp = load("b_lp", [128, 3, 2], "s p c -> p s c")
    b_f1 = load("b_f1", [128, 4])
    b_f2 = load("b_f2", [128, 4])
    g_hp = load("g_hp", [128, 3, 4], "s p c -> p s c")
    be_hp = load("be_hp", [128, 3, 4], "s p c -> p s c")
    g_n1 = load("g_n1", [128, 3, 4], "s p c -> p s c")
    be_n1 = load("be_n1", [128, 3, 4], "s p c -> p s c")
    b_gh = load("b_g_half", [128, 3, 4], "s p c -> p s c")
    g_lp = load("g_lp", [128, 3, 2], "s p c -> p s c")
    be_lp = load("be_lp", [128, 3, 2], "s p c -> p s c")
    g_f1 = load("g_f1", [128, 4])
    be_f1 = load("be_f1", [128, 4])
    g_f2 = load("g_f2", [128, 4])
    be_f2 = load("be_f2", [128, 4])
    negc_t = load("negc_f1", [1, 3, 512], dtype=MF)
    w_lp = [load(f"w_lp{s}", [FS[s], 256], dtype=MF) for s in range(3)]
    # w_r is shared by all three r units: load once into the const pool.
    w_r_t = []
    for c0 in (0, 2):
        t = const.tile([128, 2, 512], MF, name=f"w_r{c0}")
        nc.sync.dma_start(t[:], io["w_r"][c0:c0 + 2].rearrange("c p n -> p c n"))
        w_r_t.append(t)
    wr_fns = [(lambda m, t=w_r_t[c // 2], cc=c % 2: t[:, cc, ts(m, 128)])
              for c in range(4)]

    # ---------------- helpers ----------------
    def mm_groups(srcs, n_m, evict_fn):
        """srcs: list of (lhsT_fn(m), rhs_fn(bt)). Emits matmuls in two
        m-halves; after each half's accumulation completes, evict_fn(bt, m,
        ps) is called. bt is innermost so consecutive matmuls share the
        stationary operand."""
        last = len(srcs) - 1
        for mh in range(0, n_m, 2):
            mis = range(mh, min(mh + 2, n_m))
            ps = {(m, bt): mm_ps.tile([128, 512], F32, name="mm", tag="mm")
                  for m in mis for bt in range(2)}
            for ci, (lf, rf) in enumerate(srcs):
                for m in mis:
                    for bt in range(2):
                        nc.tensor.matmul(ps[(m, bt)][:], lf(m), rf(bt),
                                         start=(ci == 0), stop=(ci == last))
            for bt in range(2):
                for m in mis:
                    evict_fn(bt, m, ps[(m, bt)])

    def transpose_rows(rows):
        """PE-transpose k [1,512] sbuf rows into one [128,4k] SBUF tile
        (via PSUM) so the per-sample scalar math runs on fat tiles."""
        k = len(rows)
        vT = st_ps.tile([128, 4 * k], F32, name="vT", tag="stat_ps")
        for c in range(4):
            for i, v in enumerate(rows):
                nc.tensor.transpose(vT[:, c * k + i:c * k + i + 1],
                                    v[0:1, ts(c, 128)], ident[0:1, 0:1])
        vs = nrp.tile([128, 4 * k], F32, name="nr_v")
        nc.vector.tensor_copy(vs[:], vT[:])
        return vs

    def nr_rsqrt_T(vs, k):
        """Newton-Raphson rsqrt of a transposed [128,4k] tile (GPSIMD)."""
        y = nrp.tile([128, 4 * k], F32, name="nr_y")
        t = nrp.tile([128, 4 * k], F32, name="nr_t")
        nc.vector.tensor_scalar(y[:].bitcast(I32), vs[:].bitcast(I32),
                                1, None, ALU.logical_shift_right)
        nc.vector.tensor_scalar(y[:].bitcast(I32), y[:].bitcast(I32),
                                -1, MAGIC, ALU.mult, ALU.add)
        for _ in range(NR_ITERS):
            nc.gpsimd.tensor_mul(t[:], y[:], y[:])
            nc.gpsimd.tensor_mul(t[:], t[:], vs[:])
            nc.gpsimd.tensor_scalar(t[:], t[:], -0.5, 1.5, ALU.mult, ALU.add)
            nc.gpsimd.tensor_mul(y[:], y[:], t[:])
        return y

    def row_back(y, k, i):
        """Transpose column set i of [128,4k] back to a [1,512] MF row."""
        rT = st_ps.tile([1, 512], F32, name="rT", tag="stat_ps")
        for c in range(4):
            nc.tensor.transpose(rT[0:1, ts(c, 128)],
                                y[:, c * k + i:c * k + i + 1], ident)
        row = stb.tile([1, 512], MF, name="r16")
        nc.vector.tensor_copy(row[:], rT[:])
        return row

    def bcast(row):
        """[1,512] row -> [128,512] MF tile via PE outer product."""
        bps = st_ps.tile([128, 512], F32, name="bc_ps", tag="stat_ps")
        nc.tensor.matmul(bps[:], ones_row[:], row[0:1, :],
                         start=True, stop=True)
        bc = bcp.tile([128, 512], MF, name="bc")
        nc.scalar.activation(bc[:], bps[:], ACT.Identity)
        return bc

    # ---------------- unit emitters ----------------
    # Each unit emits its matmuls+evictions inline and returns
    # (stats_fn, fin_fn) closures to be sequenced by the main schedule.

    def ln_unit(srcs, n_m, bias_cols, gam_cols, bet_cols, func, out_pool,
                dim, mm_emitter=None):
        """Generic matmul->LN->activation unit over both batch tiles.
        PSUM is evicted (bias added) straight into the unit's output tile;
        the final activation overwrites the same slice in place.
        Returns (stats_fn, fin_a_fn, fin_b_fn, outs)."""
        outs = [None, None]

        def evict(bt, m, ps):
            if outs[bt] is None:
                outs[bt] = out_pool.tile([128, n_m, 512], MF, name="out")
            nc.vector.tensor_scalar_add(outs[bt][:, m, :], ps[:],
                                        bias_cols[m])

        (mm_emitter or mm_groups)(srcs, n_m, evict)
        st = [None, None]

        def stats():
            for bt in range(2):
                st[bt] = st_ps.tile([1, 512], F32, name="st", tag="stat_ps")
                for m in range(n_m):
                    sq = sqp.tile([128, 512], MF, name="sq")
                    nc.gpsimd.tensor_mul(sq[:], outs[bt][:, m, :],
                                         outs[bt][:, m, :])
                    nc.tensor.matmul(st[bt][:], ones_col[:], sq[:],
                                     start=(m == 0), stop=(m == n_m - 1))

        hold = {}

        def fin_a():
            vr = []
            for bt in range(2):
                v = stf.tile([1, 512], F32, name="r32")
                nc.scalar.activation(v[0:1, :], st[bt][:], ACT.Identity,
                                     bias=eps1[:],
                                     scale=(inv_d if dim == D else inv_d2)[:])
                vr.append(v)
            hold["y"] = nr_rsqrt_T(transpose_rows(vr), 2)

        def fin_b(bts=(0, 1)):
            for bt in bts:
                istd = row_back(hold["y"], 2, bt)
                bc = bcast(istd)
                for m in range(n_m):
                    z = zp.tile([128, 512], MF, name="z")
                    nc.vector.tensor_mul(z[:], outs[bt][:, m, :], bc[:])
                    nc.scalar.activation(outs[bt][:, m, :], z[:], func,
                                         bias=bet_cols[m], scale=gam_cols[m])

        return stats, fin_a, fin_b, outs

    def m_unit(s, e_tiles, m_streams, wfns):
        """Pair-average matmul; eviction only (adds bias)."""
        sa, sb = m_streams[s]
        srcs = []
        for c in range(8):
            if c < 4:
                rf = (lambda c: (lambda bt: e_tiles[sa][bt][:, c, :]))(c)
            else:
                rf = (lambda c: (lambda bt: e_tiles[sb][bt][:, c - 4, :]))(c)
            srcs.append((wfns[c], rf))
        m_sb = [mp.tile([128, 4, 512], MF, name="m_sb") for _ in range(2)]

        def evict(bt, m, ps):
            nc.vector.tensor_scalar_add(m_sb[bt][:, m, :], ps[:],
                                        b_m[:, s, m:m + 1])

        mm_groups(srcs, 4, evict)
        return m_sb

    def gate_unit(s, e_tiles, m_sb, wfns):
        """Gate matmul -> tanh-sigmoid -> t = e + gate*m -> n2 stats.
        zt = t*istd_bc - (mu*istd)_bc is produced in fin."""
        srcs = []
        for c in range(8):
            if c < 4:
                rf = (lambda c: (lambda bt: e_tiles[s][bt][:, c, :]))(c)
            else:
                rf = (lambda c: (lambda bt: m_sb[bt][:, c - 4, :]))(c)
            srcs.append((wfns[c], rf))
        t_sb = [tp.tile([128, 4, 512], MF, name="t_sb") for _ in range(2)]

        def evict(bt, m, ps):
            # t = e + sigmoid(pre)*m_avg = e + (th+1)*m_tilde
            th = thp.tile([128, 512], MF, name="th")
            nc.scalar.activation(th[:], ps[:], ACT.Tanh,
                                 bias=b_gh[:, s, m:m + 1], scale=0.5)
            u = up.tile([128, 512], MF, name="u")
            nc.vector.scalar_tensor_tensor(u[:], th[:], 1.0,
                                           m_sb[bt][:, m, :],
                                           ALU.add, ALU.mult)
            nc.gpsimd.tensor_add(t_sb[bt][:, m, :], u[:],
                                 e_tiles[s][bt][:, m, :])

        mm_groups(srcs, 4, evict)
        st_sum = [None, None]
        st_sq = [None, None]

        def stats():
            for bt in range(2):
                st_sum[bt] = st_ps.tile([1, 512], F32, name="st_sum",
                                        tag="stat_ps")
                for m in range(4):
                    nc.tensor.matmul(st_sum[bt][:], ones_col[:],
                                     t_sb[bt][:, m, :],
                                     start=(m == 0), stop=(m == 3))
            for bt in range(2):
                st_sq[bt] = st_ps.tile([1, 512], F32, name="st_sq",
                                       tag="stat_ps")
                for m in range(4):
                    sq = sqp.tile([128, 512], MF, name="sq")
                    nc.gpsimd.tensor_mul(sq[:], t_sb[bt][:, m, :],
                                         t_sb[bt][:, m, :])
                    nc.tensor.matmul(st_sq[bt][:], ones_col[:], sq[:],
                                     start=(m == 0), stop=(m == 3))

        hold = {}

        def fin_a():
            mu_rows, v_rows = [], []
            for bt in range(2):
                m_ = stf.tile([1, 512], F32, name="r32")
                nc.scalar.activation(m_[0:1, :], st_sum[bt][:], ACT.Identity,
                                     scale=inv_d[:])
                v = stf.tile([1, 512], F32, name="r32")
                nc.scalar.activation(v[0:1, :], st_sq[bt][:], ACT.Identity,
                                     bias=eps1[:], scale=inv_d[:])
                mu_rows.append(m_)
                v_rows.append(v)
            muS = transpose_rows(mu_rows)
            vS = transpose_rows(v_rows)
            musq = nrp.tile([128, 8], F32, name="nr_t")
            nc.gpsimd.tensor_mul(musq[:], muS[:], muS[:])
            nc.gpsimd.tensor_sub(vS[:], vS[:], musq[:])
            y = nr_rsqrt_T(vS, 2)
            wT = nrp.tile([128, 8], F32, name="nr_w")
            nc.gpsimd.tensor_mul(wT[:], muS[:], y[:])
            hold["y"], hold["w"] = y, wT

        wrows = {}

        def fin_b(bts=(0, 1)):
            # zt = t*istd_bc in place; the -mu*istd mean correction is a
            # rank-1 negc matmul inside f1 (wrows are its rhs rows).
            for bt in bts:
                istd = row_back(hold["y"], 2, bt)
                wrows[bt] = row_back(hold["w"], 2, bt)
                bci = bcast(istd)
                for m in range(4):
                    nc.vector.tensor_mul(t_sb[bt][:, m, :],
                                         t_sb[bt][:, m, :], bci[:])

        return stats, fin_a, fin_b, t_sb, wrows

    # ---------------- unit constructors ----------------
    def make_lp(s):
        srcs = [(lambda m, s=s: w_lp[s][:, ts(m, 128)],
                 lambda bt, s=s: lsg[s][:, ts(bt, 512)])]
        return ln_unit(srcs, 2,
                       [b_lp[:, s, c:c + 1] for c in range(2)],
                       [g_lp[:, s, c:c + 1] for c in range(2)],
                       [be_lp[:, s, c:c + 1] for c in range(2)],
                       ACT.Gelu, lpo, D // 2)

    def make_hp(s, xcs, wfns):
        srcs = [(wfns[c],
                 (lambda c: (lambda bt: xcs[bt * 4 + c // 2][:, c % 2, :]))(c))
                for c in range(8)]
        return ln_unit(srcs, 4,
                       [b_hp[:, s, c:c + 1] for c in range(4)],
                       [g_hp[:, s, c:c + 1] for c in range(4)],
                       [be_hp[:, s, c:c + 1] for c in range(4)],
                       ACT.Gelu, yhp, D)

    def make_r(s, yh):
        srcs = [(wr_fns[c], (lambda c: (lambda bt: yh[bt][:, c, :]))(c))
                for c in range(4)]
        return ln_unit(srcs, 4,
                       [b_r[:, c:c + 1] for c in range(4)],
                       [g_n1[:, s, c:c + 1] for c in range(4)],
                       [be_n1[:, s, c:c + 1] for c in range(4)],
                       ACT.Identity, ep, D)

    def prefetch_f1():
        fns = []
        for s in range(3):
            fns.append(load_wchunks(io["w_f1l"][s], 2))
        for s in (2, 1, 0):
            fns.append(load_wchunks(io["w_f1"][s], 4))
        return fns

    def make_f1(l_tiles, zt_tiles, gate_fbs, wf, wrows_by_s):
        """f1 with the three gate fin_b's interleaved between chunk stages:
        l chunks first, then g2.fb, zt2 chunks, g1.fb, zt1, g0.fb, zt0,
        and the rank-1 mean-correction (negc x mu*istd rows) last."""
        srcs = []
        for s in range(3):
            for c in range(2):
                srcs.append((wf[s][c],
                             (lambda s, c: (lambda bt: l_tiles[s][bt][:, c, :]))(s, c)))
        for i, s in enumerate((2, 1, 0)):
            for c in range(4):
                srcs.append((wf[3 + i][c],
                             (lambda s, c: (lambda bt: zt_tiles[s][bt][:, c, :]))(s, c)))
        for s in (2, 1, 0):
            srcs.append(((lambda m, s=s: negc_t[0:1, s, ts(m, 128)]),
                         (lambda s=s: (lambda bt: wrows_by_s[s][bt][0:1, :]))()))
        fb_at = {0: gate_fbs[2], 6: gate_fbs[1], 10: gate_fbs[0]}

        def emitter(srcs, n_m, evict_fn):
            last = len(srcs) - 1
            for mh in range(0, n_m, 2):
                mis = range(mh, mh + 2)
                ps = {(m, bt): mm_ps.tile([128, 512], F32, name="mm",
                                          tag="mm")
                      for m in mis for bt in range(2)}
                for ci, (lf, rf) in enumerate(srcs):
                    if mh == 0 and ci in fb_at:
                        fb_at[ci]()
                    for m in mis:
                        for bt in range(2):
                            nc.tensor.matmul(ps[(m, bt)][:], lf(m), rf(bt),
                                             start=(ci == 0),
                                             stop=(ci == last))
                for bt in range(2):
                    for m in mis:
                        evict_fn(bt, m, ps[(m, bt)])

        return ln_unit(srcs, 4,
                       [b_f1[:, c:c + 1] for c in range(4)],
                       [g_f1[:, c:c + 1] for c in range(4)],
                       [be_f1[:, c:c + 1] for c in range(4)],
                       ACT.Gelu, hp_, D, mm_emitter=emitter)

    # ---------------- main schedule ----------------
    # Emission order == per-engine execution order (all engines run their
    # queues in order). Each unit's fin is split: fin_a (stat eviction +
    # transposes + NR chain) is emitted early so its latency runs under
    # later matmul blocks; fin_b (back-transposes + broadcast + normalize)
    # is emitted just before the consumer needs the result.
    m_streams = [(1, 2), (0, 2), (0, 1)]

    x_pf.append(load_x(1))
    whp_pf.append(load_wchunks(io["w_hp"][1], 8))
    hp0 = make_hp(0, x_pf[0], whp_pf[0])
    lp_u = [make_lp(s) for s in range(3)]
    x_pf.append(load_x(2))
    whp_pf.append(load_wchunks(io["w_hp"][2], 8))
    hp1 = make_hp(1, x_pf[1], whp_pf[1])
    hp0[0]()                   # hp0 stats
    for s in range(3):
        lp_u[s][0]()           # lp stats
    hp0[1]()                   # hp0 fin_a
    for s in range(3):
        lp_u[s][1]()           # lp fin_a
    hp2 = make_hp(2, x_pf[2], whp_pf[2])
    wf_m2 = load_wchunks(io["w_m"][2], 8)
    hp1[0]()
    hp0[2]()                   # hp0 fin_b -> yh0
    for s in range(3):
        lp_u[s][2]()           # lp fin_b -> l
    r0 = make_r(0, hp0[3])
    hp1[1]()
    hp2[0]()
    hp1[2]()                   # -> yh1
    r1 = make_r(1, hp1[3])
    wf_m1 = load_wchunks(io["w_m"][1], 8)
    r0[0]()
    hp2[1]()
    hp2[2]()                   # -> yh2
    r2 = make_r(2, hp2[3])
    wf_g2 = load_wchunks(io["w_g"][2], 8)
    r0[1]()
    r1[0]()
    r0[2]()                    # -> e0
    r2[0]()
    r1[1]()
    r2[1]()
    r1[2]()                    # -> e1
    e_tiles = [r0[3], r1[3], r2[3]]
    m2 = m_unit(2, e_tiles, m_streams, wf_m2)   # e0 (c0-3), e1 (c4-7)
    wf_m0 = load_wchunks(io["w_m"][0], 8)
    r2[2]()                    # -> e2 (NR ran under m2's matmuls)
    m1 = m_unit(1, e_tiles, m_streams, wf_m1)   # e0 (c0-3), e2 (c4-7)
    wf_g1 = load_wchunks(io["w_g"][1], 8)
    g2 = gate_unit(2, e_tiles, m2, wf_g2)
    m0 = m_unit(0, e_tiles, m_streams, wf_m0)   # e1, e2
    wf_g0 = load_wchunks(io["w_g"][0], 8)
    g2[0]()                    # g2 stats
    g1 = gate_unit(1, e_tiles, m1, wf_g1)
    g2[1]()                    # g2 fin_a
    wf_f1 = prefetch_f1()
    g0 = gate_unit(0, e_tiles, m0, wf_g0)
    wf_f2 = load_wchunks(io["w_f2"], 4)
    g1[0]()
    g0[0]()
    g1[1]()                    # g1 fin_a
    g0[1]()                    # g0 fin_a
    l_tiles = [u[3] for u in lp_u]
    zt_tiles = [g0[3], g1[3], g2[3]]
    f1 = make_f1(l_tiles, zt_tiles, [g0[2], g1[2], g2[2]], wf_f1,
                 {0: g0[4], 1: g1[4], 2: g2[4]})
    f1[0]()
    f1[1]()

    # ---- f2 (final LN), pipelined per batch tile with f1's fin_b ----
    h_tiles = f1[3]
    o_tiles = [None, None]
    f2st = [None, None]
    f2hold = {}

    def f2_mm(bt):
        o_tiles[bt] = op_.tile([128, 4, 512], MF, name="o_sb")
        ps = [mm_ps.tile([128, 512], F32, name="mm", tag="mm")
              for _ in range(4)]
        for ci in range(4):
            for m in range(4):
                nc.tensor.matmul(ps[m][:], wf_f2[ci](m),
                                 h_tiles[bt][:, ci, :],
                                 start=(ci == 0), stop=(ci == 3))
        for m in range(4):
            nc.vector.tensor_scalar_add(o_tiles[bt][:, m, :], ps[m][:],
                                        b_f2[:, m:m + 1])

    def f2_stats(bt):
        f2st[bt] = st_ps.tile([1, 512], F32, name="st", tag="stat_ps")
        for m in range(4):
            sq = sqp.tile([128, 512], MF, name="sq")
            nc.gpsimd.tensor_mul(sq[:], o_tiles[bt][:, m, :],
                                 o_tiles[bt][:, m, :])
            nc.tensor.matmul(f2st[bt][:], ones_col[:], sq[:],
                             start=(m == 0), stop=(m == 3))

    def f2_fa():
        vr = []
        for bt in range(2):
            v = stf.tile([1, 512], F32, name="r32")
            nc.scalar.activation(v[0:1, :], f2st[bt][:], ACT.Identity,
                                 bias=eps1[:], scale=inv_d[:])
            vr.append(v)
        f2hold["y"] = nr_rsqrt_T(transpose_rows(vr), 2)

    def f2_fb(bt):
        istd = row_back(f2hold["y"], 2, bt)
        bc = bcast(istd)
        for m in range(4):
            z = zp.tile([128, 512], MF, name="z")
            nc.vector.tensor_mul(z[:], o_tiles[bt][:, m, :], bc[:])
            nc.scalar.activation(o_tiles[bt][:, m, :], z[:], ACT.Identity,
                                 bias=be_f2[:, m:m + 1],
                                 scale=g_f2[:, m:m + 1])
        nc.sync.dma_start(
            io["outT"].rearrange("(c p) b -> p c b", p=128)[:, :, ts(bt, 512)],
            o_tiles[bt][:])

    f1[2]((0,))                # -> h[bt0]
    f2_mm(0)
    f1[2]((1,))                # -> h[bt1]
    f2_mm(1)
    f2_stats(0)
    f2_stats(1)
    f2_fa()
    f2_fb(0)
    f2_fb(1)

    ctx.close()


def build_program():
    nc = bacc.Bacc("TRN2", target_bir_lowering=False, debug=False,
                   num_devices=NCORES)
    io = {}

    def din(name, shape, dtype=F32):
        io[name] = nc.dram_tensor(name, list(shape), dtype,
                                  kind="ExternalInput").ap()

    for s in range(3):
        din(f"xT{s}", (HID, BL), dtype=MM_DT)
        din(f"lT{s}", (FS[s], BL), dtype=MM_DT)
    din("w_hp", (3, 8, 128, 512), dtype=MM_DT)
    din("b_hp", (3, 128, 4))
    din("w_r", (4, 128, 512), dtype=MM_DT)
    din("b_r", (128, 4))
    din("w_m", (3, 8, 128, 512), dtype=MM_DT)
    din("b_m", (3, 128, 4))
    din("w_g", (3, 8, 128, 512), dtype=MM_DT)
    for s in range(3):
        din(f"w_lp{s}", (FS[s], 256), dtype=MM_DT)
    din("b_lp", (3, 128, 2))
    din("w_f1", (3, 4, 128, 512), dtype=MM_DT)
    din("negc_f1", (1, 3, 512), dtype=MM_DT)
    din("w_f1l", (3, 2, 128, 512), dtype=MM_DT)
    din("b_f1", (128, 4))
    din("w_f2", (4, 128, 512), dtype=MM_DT)
    din("b_f2", (128, 4))
    for name in ("g_hp", "be_hp", "g_n1", "be_n1", "b_g_half"):
        din(name, (3, 128, 4))
    for name in ("g_lp", "be_lp"):
        din(name, (3, 128, 2))
    for name in ("g_f1", "be_f1", "g_f2", "be_f2"):
        din(name, (128, 4))
    din("ones_col", (128, 1), dtype=MM_DT)
    din("ones_row", (1, 128), dtype=MM_DT)
    io["outT"] = nc.dram_tensor("outT", [D, BL], MM_DT,
                                kind="ExternalOutput").ap()

    with tile.TileContext(nc) as tc:
        emit_program(tc, io)
    nc.compile()
    return nc


def make_in_maps(inputs):
    fw = fold_weights(inputs)
    dev = device_arrays(fw)
    hidden = [np.asarray(inputs["verb_hidden"], np.float32),
              np.asarray(inputs["inst_hidden"], np.float32),
              np.asarray(inputs["target_hidden"], np.float32)]
    logits = [np.asarray(inputs["verb_logits"], np.float32),
              np.asarray(inputs["inst_logits"], np.float32),
              np.asarray(inputs["target_logits"], np.float32)]
    sig = [1.0 / (1.0 + np.exp(-np.asarray(l, F64))) for l in logits]
    in_maps = []
    for core in range(NCORES):
        rows = slice(core * BL, (core + 1) * BL)
        m = dict(dev)
        for s in range(3):
            m[f"xT{s}"] = _mf_np(hidden[s][rows].T)
            m[f"lT{s}"] = _mf_np(sig[s][rows].T)
        in_maps.append(m)
    return in_maps


_NC_CACHE = None


def _run(inputs, **spmd_kwargs):
    global _NC_CACHE
    if _NC_CACHE is None:
        _NC_CACHE = build_program()
    nc = _NC_CACHE
    in_maps = make_in_maps(inputs)
    res = run_bass_kernel_spmd(nc, in_maps, list(range(NCORES)),
                               **spmd_kwargs)
    out = np.empty((B, D), dtype=np.float32)
    for core in range(NCORES):
        out[core * BL:(core + 1) * BL] = np.asarray(
            res.results[core]["outT"], dtype=np.float32).T
    return out, res


def kernel(**inputs) -> np.ndarray:
    return _run(inputs)[0]


def kernel_profiled(inputs, tmpdir=None):
    """Returns (out, BassKernelResults) with an NTFF-based profile."""
    return _run(inputs, trace=True, tmpdir=tmpdir)


# revision 92
# speedup vs baseline: 1.2062x; 1.0221x over previous
"""Trainium2 Bass kernel for nn_AttentionModule_7146825580577.

Strategy (see spec sharding_hint): pure data parallel over the batch dim
(8192 rows -> 1024 rows per core, 8 cores), weights replicated.

v2 schedule: the two 512-column batch tiles of each core are interleaved
inside every layer unit so the Tensor engine always has independent
matmul work queued (keeps the PE out of its low-clock pstates), weights
are streamed from HBM once (each chunk feeds both batch tiles), all
matmul operands are bf16, the gate sigmoid is computed via tanh (so the
scalar engine never has to swap activation tables), the logit sigmoid is
folded into the host-side preprocessing, and the rsqrt Newton iteration
runs on the otherwise-idle GPSIMD engine over PE-transposed stat tiles.

Device math (per core), in feature-transposed layout (features on SBUF
partitions, batch on the free dim):

  - All LayerNorms whose input is an affine function of a previous
    activation use host-side column-centered weights, so mean(y) == 0 by
    construction and only sum(y^2) is needed on device (computed by a
    ones-vector matmul on the PE, reduced over partitions).
  - seq_len==1 MHA reduces to out_proj(v_proj(kv)); both projections are
    fused on the host into a single 512x512 effective matrix. The self-
    attention residual (x + sa(x)) is folded into a single matmul with
    weights I + Wv@Wo.
  - The cross-attention pair average (a+b)/2 is a single concat-matmul.
  - The n2 LayerNorm (after gating) is folded into the fus_W1 matmul:
    gamma scales fold into the weights, betas fold into the bias; the
    per-sample mean correction is applied as zt = t*istd_bc - wrow_bc
    with both rows partition-broadcast on GPSIMD.
  - 1/sqrt(var+eps) uses the int32 bit trick + Newton-Raphson on GPSIMD,
    on PE-transposed [128, k] stat tiles so each op is tiny.
"""
import os
import sys

sys.path.insert(0, "/opt/trn_rl_repo")

import numpy as np

import concourse.bass as bass
import concourse.tile as tile
from concourse import bacc, mybir
from concourse.bass import ts
from concourse.bass_utils import run_bass_kernel_spmd
from concourse.masks import make_identity

D = 512
HID = 1024
B = 8192
NCORES = 8
BL = B // NCORES          # rows per core
EPS = 1e-5
MAGIC = 0x5F3759DF
F32 = mybir.dt.float32
I32 = mybir.dt.int32
FS = [10, 6, 15]          # logit dims per stream
NR_ITERS = int(os.environ.get("KERNEL_NR_ITERS", "1"))
MM_DT = {
    "f32r": mybir.dt.float32r,
    "f32": mybir.dt.float32,
    "bf16": mybir.dt.bfloat16,
}[os.environ.get("KERNEL_MM_DTYPE", "bf16")]
MM_IS_BF16 = MM_DT == mybir.dt.bfloat16
MF = MM_DT

F64 = np.float64


# --------------------------------------------------------------------------
# Host-side weight folding
# --------------------------------------------------------------------------

def _center_cols(W, b):
    W = np.asarray(W, F64)
    b = np.asarray(b, F64)
    return W - W.mean(axis=1, keepdims=True), b - b.mean()


def fold_weights(inp):
    g = lambda k: np.asarray(inp[k], dtype=F64)
    out = {}

    w_hp, b_hp = [], []
    for s in range(3):
        W, b = _center_cols(g("hp_W")[s], g("hp_b")[s])
        w_hp.append(W)
        b_hp.append(b)
    out["w_hp"] = np.stack(w_hp)
    out["b_hp"] = np.stack(b_hp)
    out["g_hp"], out["be_hp"] = g("hp_g"), g("hp_be")

    mhaW, mhab = g("mha_in_W"), g("mha_in_b")
    moW, mob = g("mha_out_W"), g("mha_out_b")
    Wv0, bv0 = mhaW[0][:, 2 * D:], mhab[0][2 * D:]
    Wr, br = _center_cols(np.eye(D) + Wv0 @ moW[0], bv0 @ moW[0] + mob[0])
    out["w_r"], out["b_r"] = Wr, br
    out["g_n1"], out["be_n1"] = g("n1_g"), g("n1_be")

    Wj, bj = [None] * 4, [None] * 4
    for j in (1, 2, 3):
        Wv, bv = mhaW[j][:, 2 * D:], mhab[j][2 * D:]
        Wj[j] = Wv @ moW[j]
        bj[j] = bv @ moW[j] + mob[j]
    # m_verb uses (inst_e @ W1, target_e @ W2); m_inst (verb @ W1, target @ W3);
    # m_target (verb @ W2, inst @ W3)
    # m is stored pre-halved (0.25 = average 0.5 x sigmoid-via-tanh 0.5):
    # t = e + sigmoid(pre)*m_avg = (e + m_tilde) + tanh(pre/2)*m_tilde
    # with m_tilde = 0.5*m_avg.
    mods = [(1, 2), (1, 3), (2, 3)]
    w_m, b_m = [], []
    for s in range(3):
        ja, jb = mods[s]
        w_m.append(np.concatenate([0.25 * Wj[ja], 0.25 * Wj[jb]], axis=0))
        b_m.append(0.25 * (bj[ja] + bj[jb]))
    out["w_m"] = np.stack(w_m)
    out["b_m"] = np.stack(b_m)

    out["w_g"] = g("gate_W")
    # tanh trick: sigmoid(x + b) = 0.5*tanh(0.5*x + 0.5*b) + 0.5
    out["b_g_half"] = 0.5 * g("gate_b")

    w_lp, b_lp = [], []
    for s, key in enumerate(["verb", "inst", "target"]):
        W, b = _center_cols(g(f"lp_W_{key}"), g(f"lp_b_{key}"))
        w_lp.append(W)
        b_lp.append(b)
    out["w_lp"] = w_lp
    out["b_lp"] = np.stack(b_lp)
    out["g_lp"], out["be_lp"] = g("lp_g"), g("lp_be")

    W1 = g("fus_W1")
    g2, be2 = g("n2_g"), g("n2_be")
    A1, negc = [], []
    bias_total = g("fus_b1").copy()
    for s in range(3):
        blk = W1[s * D:(s + 1) * D]
        A = g2[s][:, None] * blk
        c = blk.T @ g2[s]
        A1.append(A - A.mean(axis=1, keepdims=True))
        negc.append(-(c - c.mean()))
        bias_total += be2[s] @ blk
    L1 = []
    for s in range(3):
        off = 3 * D + s * (D // 2)
        blk = W1[off: off + D // 2]
        L1.append(blk - blk.mean(axis=1, keepdims=True))
    out["w_f1"] = np.stack(A1)
    out["negc_f1"] = np.stack(negc)
    out["w_f1l"] = np.stack(L1)
    out["b_f1"] = bias_total - bias_total.mean()
    out["g_f1"], out["be_f1"] = g("fus_g1"), g("fus_ge1")

    W2c, b2c = _center_cols(g("fus_W2"), g("fus_b2"))
    out["w_f2"], out["b_f2"] = W2c, b2c
    out["g_f2"], out["be_f2"] = g("fus_g2"), g("fus_ge2")
    return out


def _vec_pp(v, nk):
    """[.., nk*128] feature vector -> ACT per-partition layout [.., 128, nk]."""
    v = np.asarray(v, np.float32)
    return np.ascontiguousarray(v.reshape(v.shape[:-1] + (nk, 128)).swapaxes(-1, -2))


def _mf_np(v):
    """Host array in the matmul dtype (bf16 or fp32)."""
    if MM_IS_BF16:
        import ml_dtypes
        return np.ascontiguousarray(np.asarray(v, np.float32).astype(
            ml_dtypes.bfloat16))
    return np.ascontiguousarray(np.asarray(v, np.float32))


def device_arrays(fw):
    """Folded weights -> dict of arrays matching the DRAM tensor decls."""
    f32 = _mf_np
    dev = {}
    dev["w_hp"] = f32(fw["w_hp"].reshape(3, 8, 128, 512))
    dev["b_hp"] = _vec_pp(fw["b_hp"], 4)
    dev["w_r"] = f32(fw["w_r"].reshape(4, 128, 512))
    dev["b_r"] = _vec_pp(fw["b_r"], 4)
    dev["w_m"] = f32(fw["w_m"].reshape(3, 8, 128, 512))
    dev["b_m"] = _vec_pp(fw["b_m"], 4)
    dev["w_g"] = f32(fw["w_g"].reshape(3, 8, 128, 512))
    for s in range(3):
        dev[f"w_lp{s}"] = f32(fw["w_lp"][s])
    dev["b_lp"] = _vec_pp(fw["b_lp"], 2)
    dev["w_f1"] = f32(fw["w_f1"].reshape(3, 4, 128, 512))
    dev["negc_f1"] = f32(fw["negc_f1"][None])
    dev["w_f1l"] = f32(fw["w_f1l"].reshape(3, 2, 128, 512))
    dev["b_f1"] = _vec_pp(fw["b_f1"], 4)
    dev["w_f2"] = f32(fw["w_f2"].reshape(4, 128, 512))
    dev["b_f2"] = _vec_pp(fw["b_f2"], 4)
    for name in ("g_hp", "be_hp", "g_n1", "be_n1"):
        dev[name] = _vec_pp(fw[name], 4)
    dev["b_g_half"] = _vec_pp(fw["b_g_half"], 4)
    dev["g_lp"] = _vec_pp(fw["g_lp"], 2)
    dev["be_lp"] = _vec_pp(fw["be_lp"], 2)
    for name in ("g_f1", "be_f1", "g_f2", "be_f2"):
        dev[name] = _vec_pp(fw[name], 4)
    dev["ones_col"] = _mf_np(np.ones((128, 1), np.float32))
    dev["ones_row"] = _mf_np(np.ones((1, 128), np.float32))
    return dev


# --------------------------------------------------------------------------
# Device program
# --------------------------------------------------------------------------

ACT = mybir.ActivationFunctionType
ALU = mybir.AluOpType


def emit_program(tc, io):
    nc = tc.nc
    from contextlib import ExitStack
    ctx = ExitStack()

    # ---------------- pools ----------------
    P = lambda name, bufs, space="SBUF": ctx.enter_context(
        tc.tile_pool(name=name, bufs=bufs, space=space))
    const = P("const", 1)
    wpool = P("wchunk", 11)     # [128,2,512] MF weight pair chunks
    #                             (f1 holds 9 pairs + f2 prefetch)
    xpool = P("xchunk", 8)      # [128,2,512] MF input pair chunks
    #                             (all 8 live across both m-halves of a unit)
    sqp = P("sq", 7)            # [128,512] MF squares (live until deferred
    #                             stats matmuls run, a unit later)
    thp = P("th", 2)            # [128,512] MF gate tanh tiles
    zp = P("z", 2)              # [128,512] MF z = y*istd tiles
    up = P("u", 2)              # [128,512] MF gate (th+1)*m tiles
    yhp = P("yh", 6)            # [128,4,512] MF hp outputs (alive hp->r)
    ep = P("e", 6)              # [128,4,512] MF n1 outputs (alive r->gate)
    mp = P("m", 4)              # [128,4,512] MF pair-average (alive m->gate)
    tp = P("t", 6)              # [128,4,512] MF gate t; normalized in place,
    #                             alive until f1 consumes it
    lpo = P("l", 6)             # [128,2,512] MF lp outputs (alive ->f1)
    hp_ = P("h", 2)             # [128,4,512] MF f1 outputs
    op_ = P("o", 2)             # [128,4,512] MF f2 outputs
    lsp = P("ls", 1)            # [Fs,1024] MF host-sigmoided logits
    stf = P("stats_f32", 6)     # [1,512] F32 stat rows
    stb = P("stats_mf", 8)      # [1,512] MF istd/wrow rows (gate wrow rows
    #                             stay live until f1's rank-1 matmuls)
    nrp = P("nr", 4)            # [128,8] F32 NR tiles
    bcp = P("bc_sb", 2)         # [128,512] MF broadcast rows
    mm_ps = P("mm_ps", 4, "PSUM")
    st_ps = P("st_ps", 4, "PSUM")

    # ---------------- DMA helpers (needed for the head prefetch) --------
    def wpair(dram_pair_ap):
        """Load two [128,512] k-chunks in one DMA -> [128,2,512] tile."""
        wc = wpool.tile([128, 2, 512], MF, name="wcp", tag="wcp")
        nc.sync.dma_start(wc[:], dram_pair_ap.rearrange("c p n -> p c n"))
        return wc

    def load_wchunks(dram_4d, nk):
        """nk k-chunks -> list of per-chunk lhsT accessors f(m)->[128,128]."""
        fns = []
        for c0 in range(0, nk, 2):
            wc = wpair(dram_4d[c0:c0 + 2])
            for cc in range(2):
                fns.append(lambda m, wc=wc, cc=cc: wc[:, cc, ts(m, 128)])
        return fns

    def load_x(s):
        xcs = []
        for bt in range(2):
            for c0 in range(0, 8, 2):
                xc = xpool.tile([128, 2, 512], MF, name="xc")
                nc.sync.dma_start(
                    xc[:],
                    io[f"xT{s}"][ts(c0 // 2, 256), ts(bt, 512)].rearrange(
                        "(c p) b -> p c b", p=128))
                xcs.append(xc)
        return xcs

    # The Sync engine issues DMAs in emission order at ~0.7us apiece, so
    # the first compute units' inputs must be first in the queue.
    x_pf = [load_x(0)]
    whp_pf = [load_wchunks(io["w_hp"][0], 8)]
    lsg = []
    for s in range(3):
        t = lsp.tile([FS[s], 1024], MF, name=f"lsg{s}")
        nc.sync.dma_start(t[:], io[f"lT{s}"])
        lsg.append(t)

    # ---------------- constants / resident weights ----------------
    ident = const.tile([128, 128], F32)
    make_identity(nc, ident)
    ones_col = const.tile([128, 1], MF)
    nc.sync.dma_start(ones_col[:], io["ones_col"])
    ones_row = const.tile([1, 128], MF)
    nc.sync.dma_start(ones_row[:], io["ones_row"])

    def fconst(value, name):
        t = const.tile([1, 1], F32, name=name)
        nc.gpsimd.memset(t[:], value)
        return t
    eps1 = fconst(EPS, "eps1")
    inv_d = fconst(1.0 / D, "inv_d")
    inv_d2 = fconst(2.0 / D, "inv_d2")

    def load(name, shape, rearr=None, dtype=F32):
        t = const.tile(shape, dtype, name=name)
        src = io[name]
        if rearr:
            src = src.rearrange(rearr)
        nc.sync.dma_start(t[:], src)
        return t

    b_hp = load("b_hp", [128, 3, 4], "s p c -> p s c")
    b_r = load("b_r", [128, 4])
    b_m = load("b_m", [128, 3, 4], "s p c -> p s c")
    b_lp = load("b_lp", [128, 3, 2], "s p c -> p s c")
    b_f1 = load("b_f1", [128, 4])
    b_f2 = load("b_f2", [128, 4])
    g_hp = load("g_hp", [128, 3, 4], "s p c -> p s c")
    be_hp = load("be_hp", [128, 3, 4], "s p c -> p s c")
    g_n1 = load("g_n1", [128, 3, 4], "s p c -> p s c")
    be_n1 = load("be_n1", [128, 3, 4], "s p c -> p s c")
    b_gh = load("b_g_half", [128, 3, 4], "s p c -> p s c")
    g_lp = load("g_lp", [128, 3, 2], "s p c -> p s c")
    be_lp = load("be_lp", [128, 3, 2], "s p c -> p s c")
    g_f1 = load("g_f1", [128, 4])
    be_f1 = load("be_f1", [128, 4])
    g_f2 = load("g_f2", [128, 4])
    be_f2 = load("be_f2", [128, 4])
    negc_t = load("negc_f1", [1, 3, 512], dtype=MF)
    w_lp = [load(f"w_lp{s}", [FS[s], 256], dtype=MF) for s in range(3)]
    # w_r is shared by all three r units: load once into the const pool.
    w_r_t = []
    for c0 in (0, 2):
        t = const.tile([128, 2, 512], MF, name=f"w_r{c0}")
        nc.sync.dma_start(t[:], io["w_r"][c0:c0 + 2].rearrange("c p n -> p c n"))
        w_r_t.append(t)
    wr_fns = [(lambda m, t=w_r_t[c // 2], cc=c % 2: t[:, cc, ts(m, 128)])
              for c in range(4)]

    # ---------------- helpers ----------------
    def mm_groups(srcs, n_m, evict_fn):
        """srcs: list of (lhsT_fn(m), rhs_fn(bt)). Emits matmuls in two
        m-halves; after each half's accumulation completes, evict_fn(bt, m,
        ps) is called. bt is innermost so consecutive matmuls share the
        stationary operand."""
        last = len(srcs) - 1
        for mh in range(0, n_m, 2):
            mis = range(mh, min(mh + 2, n_m))
            ps = {(m, bt): mm_ps.tile([128, 512], F32, name="mm", tag="mm")
                  for m in mis for bt in range(2)}
            for ci, (lf, rf) in enumerate(srcs):
                for m in mis:
                    for bt in range(2):
                        nc.tensor.matmul(ps[(m, bt)][:], lf(m), rf(bt),
                                         start=(ci == 0), stop=(ci == last))
            for bt in range(2):
                for m in mis:
                    evict_fn(bt, m, ps[(m, bt)])

    def transpose_rows(rows):
        """PE-transpose k [1,512] sbuf rows into one [128,4k] SBUF tile
        (via PSUM) so the per-sample scalar math runs on fat tiles."""
        k = len(rows)
        vT = st_ps.tile([128, 4 * k], F32, name="vT", tag="stat_ps")
        for c in range(4):
            for i, v in enumerate(rows):
                nc.tensor.transpose(vT[:, c * k + i:c * k + i + 1],
                                    v[0:1, ts(c, 128)], ident[0:1, 0:1])
        vs = nrp.tile([128, 4 * k], F32, name="nr_v")
        nc.vector.tensor_copy(vs[:], vT[:])
        return vs

    def nr_rsqrt_T(vs, k):
        """Newton-Raphson rsqrt of a transposed [128,4k] tile (GPSIMD)."""
        y = nrp.tile([128, 4 * k], F32, name="nr_y")
        t = nrp.tile([128, 4 * k], F32, name="nr_t")
        nc.vector.tensor_scalar(y[:].bitcast(I32), vs[:].bitcast(I32),
                                1, None, ALU.logical_shift_right)
        nc.vector.tensor_scalar(y[:].bitcast(I32), y[:].bitcast(I32),
                                -1, MAGIC, ALU.mult, ALU.add)
        for _ in range(NR_ITERS):
            nc.vector.tensor_mul(t[:], y[:], y[:])
            nc.vector.tensor_mul(t[:], t[:], vs[:])
            nc.vector.tensor_scalar(t[:], t[:], -0.5, 1.5, ALU.mult, ALU.add)
            nc.vector.tensor_mul(y[:], y[:], t[:])
        return y

    def row_back(y, k, i):
        """Transpose column set i of [128,4k] back to a [1,512] MF row."""
        rT = st_ps.tile([1, 512], F32, name="rT", tag="stat_ps")
        for c in range(4):
            nc.tensor.transpose(rT[0:1, ts(c, 128)],
                                y[:, c * k + i:c * k + i + 1], ident)
        row = stb.tile([1, 512], MF, name="r16")
        nc.vector.tensor_copy(row[:], rT[:])
        return row

    def bcast(row):
        """[1,512] row -> [128,512] MF tile via PE outer product."""
        bps = st_ps.tile([128, 512], F32, name="bc_ps", tag="stat_ps")
        nc.tensor.matmul(bps[:], ones_row[:], row[0:1, :],
                         start=True, stop=True)
        bc = bcp.tile([128, 512], MF, name="bc")
        nc.scalar.activation(bc[:], bps[:], ACT.Identity)
        return bc

    # ---------------- unit emitters ----------------
    # Each unit emits its matmuls+evictions inline and returns
    # (stats_fn, fin_fn) closures to be sequenced by the main schedule.

    def ln_unit(srcs, n_m, bias_cols, gam_cols, bet_cols, func, out_pool,
                dim, mm_emitter=None):
        """Generic matmul->LN->activation unit over both batch tiles.
        PSUM is evicted (bias added) straight into the unit's output tile;
        the final activation overwrites the same slice in place.
        Returns (stats_fn, fin_a_fn, fin_b_fn, outs)."""
        outs = [None, None]

        def evict(bt, m, ps):
            if outs[bt] is None:
                outs[bt] = out_pool.tile([128, n_m, 512], MF, name="out")
            nc.vector.tensor_scalar_add(outs[bt][:, m, :], ps[:],
                                        bias_cols[m])

        (mm_emitter or mm_groups)(srcs, n_m, evict)
        st = [None, None]

        def stats():
            for bt in range(2):
                st[bt] = st_ps.tile([1, 512], F32, name="st", tag="stat_ps")
                for m in range(n_m):
                    sq = sqp.tile([128, 512], MF, name="sq")
                    nc.gpsimd.tensor_mul(sq[:], outs[bt][:, m, :],
                                         outs[bt][:, m, :])
                    nc.tensor.matmul(st[bt][:], ones_col[:], sq[:],
                                     start=(m == 0), stop=(m == n_m - 1))

        hold = {}

        def fin_a():
            vr = []
            for bt in range(2):
                v = stf.tile([1, 512], F32, name="r32")
                nc.scalar.activation(v[0:1, :], st[bt][:], ACT.Identity,
                                     bias=eps1[:],
                                     scale=(inv_d if dim == D else inv_d2)[:])
                vr.append(v)
            hold["y"] = nr_rsqrt_T(transpose_rows(vr), 2)

        def fin_b(bts=(0, 1)):
            for bt in bts:
                istd = row_back(hold["y"], 2, bt)
                bc = bcast(istd)
                for m in range(n_m):
                    z = zp.tile([128, 512], MF, name="z")
                    nc.vector.tensor_mul(z[:], outs[bt][:, m, :], bc[:])
                    nc.scalar.activation(outs[bt][:, m, :], z[:], func,
                                         bias=bet_cols[m], scale=gam_cols[m])

        return stats, fin_a, fin_b, outs

    def m_unit(s, e_tiles, m_streams, wfns):
        """Pair-average matmul; eviction only (adds bias)."""
        sa, sb = m_streams[s]
        srcs = []
        for c in range(8):
            if c < 4:
                rf = (lambda c: (lambda bt: e_tiles[sa][bt][:, c, :]))(c)
            else:
                rf = (lambda c: (lambda bt: e_tiles[sb][bt][:, c - 4, :]))(c)
            srcs.append((wfns[c], rf))
        m_sb = [mp.tile([128, 4, 512], MF, name="m_sb") for _ in range(2)]

        def evict(bt, m, ps):
            nc.vector.tensor_scalar_add(m_sb[bt][:, m, :], ps[:],
                                        b_m[:, s, m:m + 1])

        mm_groups(srcs, 4, evict)
        return m_sb

    def gate_unit(s, e_tiles, m_sb, wfns):
        """Gate matmul -> tanh-sigmoid -> t = e + gate*m -> n2 stats.
        zt = t*istd_bc - (mu*istd)_bc is produced in fin."""
        srcs = []
        for c in range(8):
            if c < 4:
                rf = (lambda c: (lambda bt: e_tiles[s][bt][:, c, :]))(c)
            else:
                rf = (lambda c: (lambda bt: m_sb[bt][:, c - 4, :]))(c)
            srcs.append((wfns[c], rf))
        t_sb = [tp.tile([128, 4, 512], MF, name="t_sb") for _ in range(2)]

        def evict(bt, m, ps):
            # t = e + sigmoid(pre)*m_avg = e + (th+1)*m_tilde
            th = thp.tile([128, 512], MF, name="th")
            nc.scalar.activation(th[:], ps[:], ACT.Tanh,
                                 bias=b_gh[:, s, m:m + 1], scale=0.5)
            u = up.tile([128, 512], MF, name="u")
            nc.vector.scalar_tensor_tensor(u[:], th[:], 1.0,
                                           m_sb[bt][:, m, :],
                                           ALU.add, ALU.mult)
            nc.gpsimd.tensor_add(t_sb[bt][:, m, :], u[:],
                                 e_tiles[s][bt][:, m, :])

        mm_groups(srcs, 4, evict)
        st_sum = [None, None]
        st_sq = [None, None]

        def stats():
            for bt in range(2):
                st_sum[bt] = st_ps.tile([1, 512], F32, name="st_sum",
                                        tag="stat_ps")
                for m in range(4):
                    nc.tensor.matmul(st_sum[bt][:], ones_col[:],
                                     t_sb[bt][:, m, :],
                                     start=(m == 0), stop=(m == 3))
            for bt in range(2):
                st_sq[bt] = st_ps.tile([1, 512], F32, name="st_sq",
                                       tag="stat_ps")
                for m in range(4):
                    sq = sqp.tile([128, 512], MF, name="sq")
                    nc.gpsimd.tensor_mul(sq[:], t_sb[bt][:, m, :],
                                         t_sb[bt][:, m, :])
                    nc.tensor.matmul(st_sq[bt][:], ones_col[:], sq[:],
                                     start=(m == 0), stop=(m == 3))

        hold = {}

        def fin_a():
            mu_rows, v_rows = [], []
            for bt in range(2):
                m_ = stf.tile([1, 512], F32, name="r32")
                nc.scalar.activation(m_[0:1, :], st_sum[bt][:], ACT.Identity,
                                     scale=inv_d[:])
                v = stf.tile([1, 512], F32, name="r32")
                nc.scalar.activation(v[0:1, :], st_sq[bt][:], ACT.Identity,
                                     bias=eps1[:], scale=inv_d[:])
                mu_rows.append(m_)
                v_rows.append(v)
            muS = transpose_rows(mu_rows)
            vS = transpose_rows(v_rows)
            musq = nrp.tile([128, 8], F32, name="nr_t")
            nc.vector.tensor_mul(musq[:], muS[:], muS[:])
            nc.vector.tensor_sub(vS[:], vS[:], musq[:])
            y = nr_rsqrt_T(vS, 2)
            wT = nrp.tile([128, 8], F32, name="nr_w")
            nc.vector.tensor_mul(wT[:], muS[:], y[:])
            hold["y"], hold["w"] = y, wT

        wrows = {}

        def fin_b(bts=(0, 1)):
            # zt = t*istd_bc in place; the -mu*istd mean correction is a
            # rank-1 negc matmul inside f1 (wrows are its rhs rows).
            for bt in bts:
                istd = row_back(hold["y"], 2, bt)
                wrows[bt] = row_back(hold["w"], 2, bt)
                bci = bcast(istd)
                for m in range(4):
                    nc.vector.tensor_mul(t_sb[bt][:, m, :],
                                         t_sb[bt][:, m, :], bci[:])

        return stats, fin_a, fin_b, t_sb, wrows

    # ---------------- unit constructors ----------------
    def make_lp(s):
        srcs = [(lambda m, s=s: w_lp[s][:, ts(m, 128)],
                 lambda bt, s=s: lsg[s][:, ts(bt, 512)])]
        return ln_unit(srcs, 2,
                       [b_lp[:, s, c:c + 1] for c in range(2)],
                       [g_lp[:, s, c:c + 1] for c in range(2)],
                       [be_lp[:, s, c:c + 1] for c in range(2)],
                       ACT.Gelu, lpo, D // 2)

    def make_hp(s, xcs, wfns):
        srcs = [(wfns[c],
                 (lambda c: (lambda bt: xcs[bt * 4 + c // 2][:, c % 2, :]))(c))
                for c in range(8)]
        return ln_unit(srcs, 4,
                       [b_hp[:, s, c:c + 1] for c in range(4)],
                       [g_hp[:, s, c:c + 1] for c in range(4)],
                       [be_hp[:, s, c:c + 1] for c in range(4)],
                       ACT.Gelu, yhp, D)

    def make_r(s, yh):
        srcs = [(wr_fns[c], (lambda c: (lambda bt: yh[bt][:, c, :]))(c))
                for c in range(4)]
        return ln_unit(srcs, 4,
                       [b_r[:, c:c + 1] for c in range(4)],
                       [g_n1[:, s, c:c + 1] for c in range(4)],
                       [be_n1[:, s, c:c + 1] for c in range(4)],
                       ACT.Identity, ep, D)

    def prefetch_f1():
        fns = []
        for s in range(3):
            fns.append(load_wchunks(io["w_f1l"][s], 2))
        for s in (2, 1, 0):
            fns.append(load_wchunks(io["w_f1"][s], 4))
        return fns

    def make_f1(l_tiles, zt_tiles, gate_fbs, wf, wrows_by_s):
        """f1 with the three gate fin_b's interleaved between chunk stages:
        l chunks first, then g2.fb, zt2 chunks, g1.fb, zt1, g0.fb, zt0,
        and the rank-1 mean-correction (negc x mu*istd rows) last."""
        srcs = []
        for s in range(3):
            for c in range(2):
                srcs.append((wf[s][c],
                             (lambda s, c: (lambda bt: l_tiles[s][bt][:, c, :]))(s, c)))
        for i, s in enumerate((2, 1, 0)):
            for c in range(4):
                srcs.append((wf[3 + i][c],
                             (lambda s, c: (lambda bt: zt_tiles[s][bt][:, c, :]))(s, c)))
        for s in (2, 1, 0):
            srcs.append(((lambda m, s=s: negc_t[0:1, s, ts(m, 128)]),
                         (lambda s=s: (lambda bt: wrows_by_s[s][bt][0:1, :]))()))
        fb_at = {0: gate_fbs[2], 6: gate_fbs[1], 10: gate_fbs[0]}

        def emitter(srcs, n_m, evict_fn):
            last = len(srcs) - 1
            for mh in range(0, n_m, 2):
                mis = range(mh, mh + 2)
                ps = {(m, bt): mm_ps.tile([128, 512], F32, name="mm",
                                          tag="mm")
                      for m in mis for bt in range(2)}
                for ci, (lf, rf) in enumerate(srcs):
                    if mh == 0 and ci in fb_at:
                        fb_at[ci]()
                    for m in mis:
                        for bt in range(2):
                            nc.tensor.matmul(ps[(m, bt)][:], lf(m), rf(bt),
                                             start=(ci == 0),
                                             stop=(ci == last))
                for bt in range(2):
                    for m in mis:
                        evict_fn(bt, m, ps[(m, bt)])

        return ln_unit(srcs, 4,
                       [b_f1[:, c:c + 1] for c in range(4)],
                       [g_f1[:, c:c + 1] for c in range(4)],
                       [be_f1[:, c:c + 1] for c in range(4)],
                       ACT.Gelu, hp_, D, mm_emitter=emitter)

    # ---------------- main schedule ----------------
    # Emission order == per-engine execution order (all engines run their
    # queues in order). Each unit's fin is split: fin_a (stat eviction +
    # transposes + NR chain) is emitted early so its latency runs under
    # later matmul blocks; fin_b (back-transposes + broadcast + normalize)
    # is emitted just before the consumer needs the result.
    m_streams = [(1, 2), (0, 2), (0, 1)]

    x_pf.append(load_x(1))
    whp_pf.append(load_wchunks(io["w_hp"][1], 8))
    hp0 = make_hp(0, x_pf[0], whp_pf[0])
    lp_u = [make_lp(s) for s in range(3)]
    x_pf.append(load_x(2))
    whp_pf.append(load_wchunks(io["w_hp"][2], 8))
    hp1 = make_hp(1, x_pf[1], whp_pf[1])
    hp0[0]()                   # hp0 stats
    for s in range(3):
        lp_u[s][0]()           # lp stats
    hp0[1]()                   # hp0 fin_a
    for s in range(3):
        lp_u[s][1]()           # lp fin_a
    hp2 = make_hp(2, x_pf[2], whp_pf[2])
    wf_m2 = load_wchunks(io["w_m"][2], 8)
    hp1[0]()
    hp0[2]()                   # hp0 fin_b -> yh0
    for s in range(3):
        lp_u[s][2]()           # lp fin_b -> l
    r0 = make_r(0, hp0[3])
    hp1[1]()
    hp2[0]()
    hp1[2]()                   # -> yh1
    r1 = make_r(1, hp1[3])
    wf_m1 = load_wchunks(io["w_m"][1], 8)
    r0[0]()
    hp2[1]()
    hp2[2]()                   # -> yh2
    r2 = make_r(2, hp2[3])
    wf_g2 = load_wchunks(io["w_g"][2], 8)
    r0[1]()
    r1[0]()
    r0[2]()                    # -> e0
    r2[0]()
    r1[1]()
    r2[1]()
    r1[2]()                    # -> e1
    e_tiles = [r0[3], r1[3], r2[3]]
    m2 = m_unit(2, e_tiles, m_streams, wf_m2)   # e0 (c0-3), e1 (c4-7)
    wf_m0 = load_wchunks(io["w_m"][0], 8)
    r2[2]()                    # -> e2 (NR ran under m2's matmuls)
    m1 = m_unit(1, e_tiles, m_streams, wf_m1)   # e0 (c0-3), e2 (c4-7)
    wf_g1 = load_wchunks(io["w_g"][1], 8)
    g2 = gate_unit(2, e_tiles, m2, wf_g2)
    m0 = m_unit(0, e_tiles, m_streams, wf_m0)   # e1, e2
    wf_g0 = load_wchunks(io["w_g"][0], 8)
    g2[0]()                    # g2 stats
    g1 = gate_unit(1, e_tiles, m1, wf_g1)
    g2[1]()                    # g2 fin_a
    wf_f1 = prefetch_f1()
    g0 = gate_unit(0, e_tiles, m0, wf_g0)
    wf_f2 = load_wchunks(io["w_f2"], 4)
    g1[0]()
    g0[0]()
    g1[1]()                    # g1 fin_a
    g0[1]()                    # g0 fin_a
    l_tiles = [u[3] for u in lp_u]
    zt_tiles = [g0[3], g1[3], g2[3]]
    f1 = make_f1(l_tiles, zt_tiles, [g0[2], g1[2], g2[2]], wf_f1,
                 {0: g0[4], 1: g1[4], 2: g2[4]})
    f1[0]()
    f1[1]()

    # ---- f2 (final LN), pipelined per batch tile with f1's fin_b ----
    h_tiles = f1[3]
    o_tiles = [None, None]
    f2st = [None, None]
    f2hold = {}

    def f2_mm(bt):
        o_tiles[bt] = op_.tile([128, 4, 512], MF, name="o_sb")
        ps = [mm_ps.tile([128, 512], F32, name="mm", tag="mm")
              for _ in range(4)]
        for ci in range(4):
            for m in range(4):
                nc.tensor.matmul(ps[m][:], wf_f2[ci](m),
                                 h_tiles[bt][:, ci, :],
                                 start=(ci == 0), stop=(ci == 3))
        for m in range(4):
            nc.vector.tensor_scalar_add(o_tiles[bt][:, m, :], ps[m][:],
                                        b_f2[:, m:m + 1])

    def f2_stats(bt):
        f2st[bt] = st_ps.tile([1, 512], F32, name="st", tag="stat_ps")
        for m in range(4):
            sq = sqp.tile([128, 512], MF, name="sq")
            nc.gpsimd.tensor_mul(sq[:], o_tiles[bt][:, m, :],
                                 o_tiles[bt][:, m, :])
            nc.tensor.matmul(f2st[bt][:], ones_col[:], sq[:],
                             start=(m == 0), stop=(m == 3))

    def f2_fa():
        vr = []
        for bt in range(2):
            v = stf.tile([1, 512], F32, name="r32")
            nc.scalar.activation(v[0:1, :], f2st[bt][:], ACT.Identity,
                                 bias=eps1[:], scale=inv_d[:])
            vr.append(v)
        f2hold["y"] = nr_rsqrt_T(transpose_rows(vr), 2)

    def f2_fb(bt):
        istd = row_back(f2hold["y"], 2, bt)
        bc = bcast(istd)
        for m in range(4):
            z = zp.tile([128, 512], MF, name="z")
            nc.vector.tensor_mul(z[:], o_tiles[bt][:, m, :], bc[:])
            nc.scalar.activation(o_tiles[bt][:, m, :], z[:], ACT.Identity,
                                 bias=be_f2[:, m:m + 1],
                                 scale=g_f2[:, m:m + 1])
        nc.sync.dma_start(
            io["outT"].rearrange("(c p) b -> p c b", p=128)[:, :, ts(bt, 512)],
            o_tiles[bt][:])

    f1[2]((0,))                # -> h[bt0]
    f2_mm(0)
    f1[2]((1,))                # -> h[bt1]
    f2_mm(1)
    f2_stats(0)
    f2_stats(1)
    f2_fa()
    f2_fb(0)
    f2_fb(1)

    ctx.close()


def build_program():
    nc = bacc.Bacc("TRN2", target_bir_lowering=False, debug=False,
                   num_devices=NCORES)
    io = {}

    def din(name, shape, dtype=F32):
        io[name] = nc.dram_tensor(name, list(shape), dtype,
                                  kind="ExternalInput").ap()

    for s in range(3):
        din(f"xT{s}", (HID, BL), dtype=MM_DT)
        din(f"lT{s}", (FS[s], BL), dtype=MM_DT)
    din("w_hp", (3, 8, 128, 512), dtype=MM_DT)
    din("b_hp", (3, 128, 4))
    din("w_r", (4, 128, 512), dtype=MM_DT)
    din("b_r", (128, 4))
    din("w_m", (3, 8, 128, 512), dtype=MM_DT)
    din("b_m", (3, 128, 4))
    din("w_g", (3, 8, 128, 512), dtype=MM_DT)
    for s in range(3):
        din(f"w_lp{s}", (FS[s], 256), dtype=MM_DT)
    din("b_lp", (3, 128, 2))
    din("w_f1", (3, 4, 128, 512), dtype=MM_DT)
    din("negc_f1", (1, 3, 512), dtype=MM_DT)
    din("w_f1l", (3, 2, 128, 512), dtype=MM_DT)
    din("b_f1", (128, 4))
    din("w_f2", (4, 128, 512), dtype=MM_DT)
    din("b_f2", (128, 4))
    for name in ("g_hp", "be_hp", "g_n1", "be_n1", "b_g_half"):
        din(name, (3, 128, 4))
    for name in ("g_lp", "be_lp"):
        din(name, (3, 128, 2))
    for name in ("g_f1", "be_f1", "g_f2", "be_f2"):
        din(name, (128, 4))
    din("ones_col", (128, 1), dtype=MM_DT)
    din("ones_row", (1, 128), dtype=MM_DT)
    io["outT"] = nc.dram_tensor("outT", [D, BL], MM_DT,
                                kind="ExternalOutput").ap()

    with tile.TileContext(nc) as tc:
        emit_program(tc, io)
    nc.compile()
    return nc


def make_in_maps(inputs):
    fw = fold_weights(inputs)
    dev = device_arrays(fw)
    hidden = [np.asarray(inputs["verb_hidden"], np.float32),
              np.asarray(inputs["inst_hidden"], np.float32),
              np.asarray(inputs["target_hidden"], np.float32)]
    logits = [np.asarray(inputs["verb_logits"], np.float32),
              np.asarray(inputs["inst_logits"], np.float32),
              np.asarray(inputs["target_logits"], np.float32)]
    sig = [1.0 / (1.0 + np.exp(-np.asarray(l, F64))) for l in logits]
    in_maps = []
    for core in range(NCORES):
        rows = slice(core * BL, (core + 1) * BL)
        m = dict(dev)
        for s in range(3):
            m[f"xT{s}"] = _mf_np(hidden[s][rows].T)
            m[f"lT{s}"] = _mf_np(sig[s][rows].T)
        in_maps.append(m)
    return in_maps


_NC_CACHE = None


def _run(inputs, **spmd_kwargs):
    global _NC_CACHE
    if _NC_CACHE is None:
        _NC_CACHE = build_program()
    nc = _NC_CACHE
    in_maps = make_in_maps(inputs)
    res = run_bass_kernel_spmd(nc, in_maps, list(range(NCORES)),
                               **spmd_kwargs)
    out = np.empty((B, D), dtype=np.float32)
    for core in range(NCORES):
        out[core * BL:(core + 1) * BL] = np.asarray(
            res.results[core]["outT"], dtype=np.float32).T
    return out, res


def kernel(**inputs) -> np.ndarray:
    return _run(inputs)[0]


def kernel_profiled(inputs, tmpdir=None):
    """Returns (out, BassKernelResults) with an NTFF-based profile."""
    return _run(inputs, trace=True, tmpdir=tmpdir)


# revision 93
# speedup vs baseline: 1.2097x; 1.0029x over previous
"""Trainium2 Bass kernel for nn_AttentionModule_7146825580577.

Strategy (see spec sharding_hint): pure data parallel over the batch dim
(8192 rows -> 1024 rows per core, 8 cores), weights replicated.

Schedule: the two 512-column batch tiles of each core are interleaved
inside every layer unit so the Tensor engine always has independent
matmul work queued; weights are streamed from HBM once (each chunk feeds
both batch tiles, with DMAs prefetched a unit ahead since the Sync
engine issues them serially); all matmul operands are bf16. Every
engine executes its queue in emission order, so each LayerNorm "fin" is
split into fin_a (stat eviction + PE transposes + Newton-Raphson rsqrt)
emitted early, and fin_b (back-transposes + PE outer-product broadcast
+ normalize/activation) emitted just before the consuming matmuls.

Device math (per core), in feature-transposed layout (features on SBUF
partitions, batch on the free dim):

  - All LayerNorms whose input is an affine function of a previous
    activation use host-side column-centered weights, so mean(y) == 0 by
    construction and only sum(y^2) is needed on device (squares on
    GPSIMD, reduced over partitions by a ones-vector matmul on the PE).
  - seq_len==1 MHA reduces to out_proj(v_proj(kv)); both projections are
    fused on the host into a single 512x512 effective matrix. The self-
    attention residual (x + sa(x)) is folded into a single matmul with
    weights I + Wv@Wo.
  - The cross-attention pair average (a+b)/2 is a single concat-matmul,
    prescaled by 0.25 on the host so the gate combine is
    t = e + (tanh(pre/2 + b/2) + 1) * m_tilde (sigmoid via tanh keeps
    the scalar engine on a single activation table).
  - The n2 LayerNorm (after gating) is folded into the fus_W1 matmul:
    gamma scales fold into the weights, betas fold into the bias; the
    per-sample mean correction is a rank-1 (negc x mu*istd-row) matmul
    appended to the f1 accumulation group.
  - 1/sqrt(var+eps) uses the int32 bit trick + one Newton-Raphson step
    on PE-transposed [128, k] stat tiles (tiny DVE ops), transposed back
    and broadcast across partitions via a PE outer product.
"""
import os
import sys

sys.path.insert(0, "/opt/trn_rl_repo")

import numpy as np

import concourse.bass as bass
import concourse.tile as tile
from concourse import bacc, mybir
from concourse.bass import ts
from concourse.bass_utils import run_bass_kernel_spmd
from concourse.masks import make_identity

D = 512
HID = 1024
B = 8192
NCORES = 8
BL = B // NCORES          # rows per core
EPS = 1e-5
MAGIC = 0x5F3759DF
F32 = mybir.dt.float32
I32 = mybir.dt.int32
FS = [10, 6, 15]          # logit dims per stream
NR_ITERS = int(os.environ.get("KERNEL_NR_ITERS", "1"))
MM_DT = {
    "f32r": mybir.dt.float32r,
    "f32": mybir.dt.float32,
    "bf16": mybir.dt.bfloat16,
}[os.environ.get("KERNEL_MM_DTYPE", "bf16")]
MM_IS_BF16 = MM_DT == mybir.dt.bfloat16
MF = MM_DT

F64 = np.float64


# --------------------------------------------------------------------------
# Host-side weight folding
# --------------------------------------------------------------------------

def _center_cols(W, b):
    W = np.asarray(W, F64)
    b = np.asarray(b, F64)
    return W - W.mean(axis=1, keepdims=True), b - b.mean()


def fold_weights(inp):
    g = lambda k: np.asarray(inp[k], dtype=F64)
    out = {}

    w_hp, b_hp = [], []
    for s in range(3):
        W, b = _center_cols(g("hp_W")[s], g("hp_b")[s])
        w_hp.append(W)
        b_hp.append(b)
    out["w_hp"] = np.stack(w_hp)
    out["b_hp"] = np.stack(b_hp)
    out["g_hp"], out["be_hp"] = g("hp_g"), g("hp_be")

    mhaW, mhab = g("mha_in_W"), g("mha_in_b")
    moW, mob = g("mha_out_W"), g("mha_out_b")
    Wv0, bv0 = mhaW[0][:, 2 * D:], mhab[0][2 * D:]
    Wr, br = _center_cols(np.eye(D) + Wv0 @ moW[0], bv0 @ moW[0] + mob[0])
    out["w_r"], out["b_r"] = Wr, br
    out["g_n1"], out["be_n1"] = g("n1_g"), g("n1_be")

    Wj, bj = [None] * 4, [None] * 4
    for j in (1, 2, 3):
        Wv, bv = mhaW[j][:, 2 * D:], mhab[j][2 * D:]
        Wj[j] = Wv @ moW[j]
        bj[j] = bv @ moW[j] + mob[j]
    # m_verb uses (inst_e @ W1, target_e @ W2); m_inst (verb @ W1, target @ W3);
    # m_target (verb @ W2, inst @ W3)
    # m is stored pre-halved (0.25 = average 0.5 x sigmoid-via-tanh 0.5):
    # t = e + sigmoid(pre)*m_avg = (e + m_tilde) + tanh(pre/2)*m_tilde
    # with m_tilde = 0.5*m_avg.
    mods = [(1, 2), (1, 3), (2, 3)]
    w_m, b_m = [], []
    for s in range(3):
        ja, jb = mods[s]
        w_m.append(np.concatenate([0.25 * Wj[ja], 0.25 * Wj[jb]], axis=0))
        b_m.append(0.25 * (bj[ja] + bj[jb]))
    out["w_m"] = np.stack(w_m)
    out["b_m"] = np.stack(b_m)

    out["w_g"] = g("gate_W")
    # tanh trick: sigmoid(x + b) = 0.5*tanh(0.5*x + 0.5*b) + 0.5
    out["b_g_half"] = 0.5 * g("gate_b")

    w_lp, b_lp = [], []
    for s, key in enumerate(["verb", "inst", "target"]):
        W, b = _center_cols(g(f"lp_W_{key}"), g(f"lp_b_{key}"))
        w_lp.append(W)
        b_lp.append(b)
    out["w_lp"] = w_lp
    out["b_lp"] = np.stack(b_lp)
    out["g_lp"], out["be_lp"] = g("lp_g"), g("lp_be")

    W1 = g("fus_W1")
    g2, be2 = g("n2_g"), g("n2_be")
    A1, negc = [], []
    bias_total = g("fus_b1").copy()
    for s in range(3):
        blk = W1[s * D:(s + 1) * D]
        A = g2[s][:, None] * blk
        c = blk.T @ g2[s]
        A1.append(A - A.mean(axis=1, keepdims=True))
        negc.append(-(c - c.mean()))
        bias_total += be2[s] @ blk
    L1 = []
    for s in range(3):
        off = 3 * D + s * (D // 2)
        blk = W1[off: off + D // 2]
        L1.append(blk - blk.mean(axis=1, keepdims=True))
    out["w_f1"] = np.stack(A1)
    out["negc_f1"] = np.stack(negc)
    out["w_f1l"] = np.stack(L1)
    out["b_f1"] = bias_total - bias_total.mean()
    out["g_f1"], out["be_f1"] = g("fus_g1"), g("fus_ge1")

    W2c, b2c = _center_cols(g("fus_W2"), g("fus_b2"))
    out["w_f2"], out["b_f2"] = W2c, b2c
    out["g_f2"], out["be_f2"] = g("fus_g2"), g("fus_ge2")
    return out


def _vec_pp(v, nk):
    """[.., nk*128] feature vector -> ACT per-partition layout [.., 128, nk]."""
    v = np.asarray(v, np.float32)
    return np.ascontiguousarray(v.reshape(v.shape[:-1] + (nk, 128)).swapaxes(-1, -2))


def _mf_np(v):
    """Host array in the matmul dtype (bf16 or fp32)."""
    if MM_IS_BF16:
        import ml_dtypes
        return np.ascontiguousarray(np.asarray(v, np.float32).astype(
            ml_dtypes.bfloat16))
    return np.ascontiguousarray(np.asarray(v, np.float32))


def device_arrays(fw):
    """Folded weights -> dict of arrays matching the DRAM tensor decls."""
    f32 = _mf_np
    dev = {}
    dev["w_hp"] = f32(fw["w_hp"].reshape(3, 8, 128, 512))
    dev["b_hp"] = _vec_pp(fw["b_hp"], 4)
    dev["w_r"] = f32(fw["w_r"].reshape(4, 128, 512))
    dev["b_r"] = _vec_pp(fw["b_r"], 4)
    dev["w_m"] = f32(fw["w_m"].reshape(3, 8, 128, 512))
    dev["b_m"] = _vec_pp(fw["b_m"], 4)
    dev["w_g"] = f32(fw["w_g"].reshape(3, 8, 128, 512))
    for s in range(3):
        dev[f"w_lp{s}"] = f32(fw["w_lp"][s])
    dev["b_lp"] = _vec_pp(fw["b_lp"], 2)
    dev["w_f1"] = f32(fw["w_f1"].reshape(3, 4, 128, 512))
    dev["negc_f1"] = f32(fw["negc_f1"][None])
    dev["w_f1l"] = f32(fw["w_f1l"].reshape(3, 2, 128, 512))
    dev["b_f1"] = _vec_pp(fw["b_f1"], 4)
    dev["w_f2"] = f32(fw["w_f2"].reshape(4, 128, 512))
    dev["b_f2"] = _vec_pp(fw["b_f2"], 4)
    for name in ("g_hp", "be_hp", "g_n1", "be_n1"):
        dev[name] = _vec_pp(fw[name], 4)
    dev["b_g_half"] = _vec_pp(fw["b_g_half"], 4)
    dev["g_lp"] = _vec_pp(fw["g_lp"], 2)
    dev["be_lp"] = _vec_pp(fw["be_lp"], 2)
    for name in ("g_f1", "be_f1", "g_f2", "be_f2"):
        dev[name] = _vec_pp(fw[name], 4)
    dev["ones_col"] = _mf_np(np.ones((128, 1), np.float32))
    dev["ones_row"] = _mf_np(np.ones((1, 128), np.float32))
    return dev


# --------------------------------------------------------------------------
# Device program
# --------------------------------------------------------------------------

ACT = mybir.ActivationFunctionType
ALU = mybir.AluOpType


def emit_program(tc, io):
    nc = tc.nc
    from contextlib import ExitStack
    ctx = ExitStack()

    # ---------------- pools ----------------
    P = lambda name, bufs, space="SBUF": ctx.enter_context(
        tc.tile_pool(name=name, bufs=bufs, space=space))
    const = P("const", 1)
    wpool = P("wchunk", 11)     # [128,2,512] MF weight pair chunks
    #                             (f1 holds 9 pairs + f2 prefetch)
    xpool = P("xchunk", 8)      # [128,2,512] MF input pair chunks
    #                             (all 8 live across both m-halves of a unit)
    sqp = P("sq", 7)            # [128,512] MF squares (live until deferred
    #                             stats matmuls run, a unit later)
    thp = P("th", 2)            # [128,512] MF gate tanh tiles
    zp = P("z", 2)              # [128,512] MF z = y*istd tiles
    up = P("u", 2)              # [128,512] MF gate (th+1)*m tiles
    yhp = P("yh", 6)            # [128,4,512] MF hp outputs (alive hp->r)
    ep = P("e", 6)              # [128,4,512] MF n1 outputs (alive r->gate)
    mp = P("m", 4)              # [128,4,512] MF pair-average (alive m->gate)
    tp = P("t", 6)              # [128,4,512] MF gate t; normalized in place,
    #                             alive until f1 consumes it
    lpo = P("l", 6)             # [128,2,512] MF lp outputs (alive ->f1)
    hp_ = P("h", 2)             # [128,4,512] MF f1 outputs
    op_ = P("o", 2)             # [128,4,512] MF f2 outputs
    lsp = P("ls", 1)            # [Fs,1024] MF host-sigmoided logits
    stf = P("stats_f32", 6)     # [1,512] F32 stat rows
    stb = P("stats_mf", 8)      # [1,512] MF istd/wrow rows (gate wrow rows
    #                             stay live until f1's rank-1 matmuls)
    nrp = P("nr", 4)            # [128,8] F32 NR tiles
    bcp = P("bc_sb", 2)         # [128,512] MF broadcast rows
    mm_ps = P("mm_ps", 4, "PSUM")
    st_ps = P("st_ps", 4, "PSUM")

    # ---------------- DMA helpers (needed for the head prefetch) --------
    def wpair(dram_pair_ap):
        """Load two [128,512] k-chunks in one DMA -> [128,2,512] tile."""
        wc = wpool.tile([128, 2, 512], MF, name="wcp", tag="wcp")
        nc.sync.dma_start(wc[:], dram_pair_ap.rearrange("c p n -> p c n"))
        return wc

    def load_wchunks(dram_4d, nk):
        """nk k-chunks -> list of per-chunk lhsT accessors f(m)->[128,128]."""
        fns = []
        for c0 in range(0, nk, 2):
            wc = wpair(dram_4d[c0:c0 + 2])
            for cc in range(2):
                fns.append(lambda m, wc=wc, cc=cc: wc[:, cc, ts(m, 128)])
        return fns

    def load_x(s):
        xcs = []
        for bt in range(2):
            for c0 in range(0, 8, 2):
                xc = xpool.tile([128, 2, 512], MF, name="xc")
                nc.sync.dma_start(
                    xc[:],
                    io[f"xT{s}"][ts(c0 // 2, 256), ts(bt, 512)].rearrange(
                        "(c p) b -> p c b", p=128))
                xcs.append(xc)
        return xcs

    # The Sync engine issues DMAs in emission order at ~0.7us apiece, so
    # the first compute units' inputs must be first in the queue.
    x_pf = [load_x(0)]
    whp_pf = [load_wchunks(io["w_hp"][0], 8)]
    lsg = []
    for s in range(3):
        t = lsp.tile([FS[s], 1024], MF, name=f"lsg{s}")
        nc.sync.dma_start(t[:], io[f"lT{s}"])
        lsg.append(t)

    # ---------------- constants / resident weights ----------------
    ident = const.tile([128, 128], F32)
    make_identity(nc, ident)
    ones_col = const.tile([128, 1], MF)
    nc.sync.dma_start(ones_col[:], io["ones_col"])
    ones_row = const.tile([1, 128], MF)
    nc.sync.dma_start(ones_row[:], io["ones_row"])

    def fconst(value, name):
        t = const.tile([1, 1], F32, name=name)
        nc.gpsimd.memset(t[:], value)
        return t
    eps1 = fconst(EPS, "eps1")
    inv_d = fconst(1.0 / D, "inv_d")
    inv_d2 = fconst(2.0 / D, "inv_d2")

    def load(name, shape, rearr=None, dtype=F32):
        t = const.tile(shape, dtype, name=name)
        src = io[name]
        if rearr:
            src = src.rearrange(rearr)
        nc.sync.dma_start(t[:], src)
        return t

    b_hp = load("b_hp", [128, 3, 4], "s p c -> p s c")
    b_r = load("b_r", [128, 4])
    b_m = load("b_m", [128, 3, 4], "s p c -> p s c")
    b_lp = load("b_lp", [128, 3, 2], "s p c -> p s c")
    b_f1 = load("b_f1", [128, 4])
    b_f2 = load("b_f2", [128, 4])
    g_hp = load("g_hp", [128, 3, 4], "s p c -> p s c")
    be_hp = load("be_hp", [128, 3, 4], "s p c -> p s c")
    g_n1 = load("g_n1", [128, 3, 4], "s p c -> p s c")
    be_n1 = load("be_n1", [128, 3, 4], "s p c -> p s c")
    b_gh = load("b_g_half", [128, 3, 4], "s p c -> p s c")
    g_lp = load("g_lp", [128, 3, 2], "s p c -> p s c")
    be_lp = load("be_lp", [128, 3, 2], "s p c -> p s c")
    g_f1 = load("g_f1", [128, 4])
    be_f1 = load("be_f1", [128, 4])
    g_f2 = load("g_f2", [128, 4])
    be_f2 = load("be_f2", [128, 4])
    negc_t = load("negc_f1", [1, 3, 512], dtype=MF)
    w_lp = [load(f"w_lp{s}", [FS[s], 256], dtype=MF) for s in range(3)]
    # w_r is shared by all three r units: load once into the const pool.
    w_r_t = []
    for c0 in (0, 2):
        t = const.tile([128, 2, 512], MF, name=f"w_r{c0}")
        nc.sync.dma_start(t[:], io["w_r"][c0:c0 + 2].rearrange("c p n -> p c n"))
        w_r_t.append(t)
    wr_fns = [(lambda m, t=w_r_t[c // 2], cc=c % 2: t[:, cc, ts(m, 128)])
              for c in range(4)]

    # ---------------- helpers ----------------
    def mm_groups(srcs, n_m, evict_fn):
        """srcs: list of (lhsT_fn(m), rhs_fn(bt)). Emits matmuls in two
        m-halves; after each half's accumulation completes, evict_fn(bt, m,
        ps) is called. bt is innermost so consecutive matmuls share the
        stationary operand."""
        last = len(srcs) - 1
        for mh in range(0, n_m, 2):
            mis = range(mh, min(mh + 2, n_m))
            ps = {(m, bt): mm_ps.tile([128, 512], F32, name="mm", tag="mm")
                  for m in mis for bt in range(2)}
            for ci, (lf, rf) in enumerate(srcs):
                for m in mis:
                    for bt in range(2):
                        nc.tensor.matmul(ps[(m, bt)][:], lf(m), rf(bt),
                                         start=(ci == 0), stop=(ci == last))
            for bt in range(2):
                for m in mis:
                    evict_fn(bt, m, ps[(m, bt)])

    def transpose_rows(rows):
        """PE-transpose k [1,512] sbuf rows into one [128,4k] SBUF tile
        (via PSUM) so the per-sample scalar math runs on fat tiles."""
        k = len(rows)
        vT = st_ps.tile([128, 4 * k], F32, name="vT", tag="stat_ps")
        for c in range(4):
            for i, v in enumerate(rows):
                nc.tensor.transpose(vT[:, c * k + i:c * k + i + 1],
                                    v[0:1, ts(c, 128)], ident[0:1, 0:1])
        vs = nrp.tile([128, 4 * k], F32, name="nr_v")
        nc.vector.tensor_copy(vs[:], vT[:])
        return vs

    def nr_rsqrt_T(vs, k):
        """Newton-Raphson rsqrt of a transposed [128,4k] tile (GPSIMD)."""
        y = nrp.tile([128, 4 * k], F32, name="nr_y")
        t = nrp.tile([128, 4 * k], F32, name="nr_t")
        nc.vector.tensor_scalar(y[:].bitcast(I32), vs[:].bitcast(I32),
                                1, None, ALU.logical_shift_right)
        nc.vector.tensor_scalar(y[:].bitcast(I32), y[:].bitcast(I32),
                                -1, MAGIC, ALU.mult, ALU.add)
        for _ in range(NR_ITERS):
            nc.vector.tensor_mul(t[:], y[:], y[:])
            nc.vector.tensor_mul(t[:], t[:], vs[:])
            nc.vector.tensor_scalar(t[:], t[:], -0.5, 1.5, ALU.mult, ALU.add)
            nc.vector.tensor_mul(y[:], y[:], t[:])
        return y

    def row_back(y, k, i):
        """Transpose column set i of [128,4k] back to a [1,512] MF row."""
        rT = st_ps.tile([1, 512], F32, name="rT", tag="stat_ps")
        for c in range(4):
            nc.tensor.transpose(rT[0:1, ts(c, 128)],
                                y[:, c * k + i:c * k + i + 1], ident)
        row = stb.tile([1, 512], MF, name="r16")
        nc.vector.tensor_copy(row[:], rT[:])
        return row

    def bcast(row):
        """[1,512] row -> [128,512] MF tile via PE outer product."""
        bps = st_ps.tile([128, 512], F32, name="bc_ps", tag="stat_ps")
        nc.tensor.matmul(bps[:], ones_row[:], row[0:1, :],
                         start=True, stop=True)
        bc = bcp.tile([128, 512], MF, name="bc")
        nc.scalar.activation(bc[:], bps[:], ACT.Identity)
        return bc

    # ---------------- unit emitters ----------------
    # Each unit emits its matmuls+evictions inline and returns
    # (stats_fn, fin_fn) closures to be sequenced by the main schedule.

    def ln_unit(srcs, n_m, bias_cols, gam_cols, bet_cols, func, out_pool,
                dim, mm_emitter=None):
        """Generic matmul->LN->activation unit over both batch tiles.
        PSUM is evicted (bias added) straight into the unit's output tile;
        the final activation overwrites the same slice in place.
        Returns (stats_fn, fin_a_fn, fin_b_fn, outs)."""
        outs = [None, None]

        def evict(bt, m, ps):
            if outs[bt] is None:
                outs[bt] = out_pool.tile([128, n_m, 512], MF, name="out")
            nc.vector.tensor_scalar_add(outs[bt][:, m, :], ps[:],
                                        bias_cols[m])

        (mm_emitter or mm_groups)(srcs, n_m, evict)
        st = [None, None]

        def stats():
            for bt in range(2):
                st[bt] = st_ps.tile([1, 512], F32, name="st", tag="stat_ps")
                for m in range(n_m):
                    sq = sqp.tile([128, 512], MF, name="sq")
                    nc.gpsimd.tensor_mul(sq[:], outs[bt][:, m, :],
                                         outs[bt][:, m, :])
                    nc.tensor.matmul(st[bt][:], ones_col[:], sq[:],
                                     start=(m == 0), stop=(m == n_m - 1))

        hold = {}

        def fin_a():
            vr = []
            for bt in range(2):
                v = stf.tile([1, 512], F32, name="r32")
                nc.scalar.activation(v[0:1, :], st[bt][:], ACT.Identity,
                                     bias=eps1[:],
                                     scale=(inv_d if dim == D else inv_d2)[:])
                vr.append(v)
            hold["y"] = nr_rsqrt_T(transpose_rows(vr), 2)

        def fin_b(bts=(0, 1)):
            for bt in bts:
                istd = row_back(hold["y"], 2, bt)
                bc = bcast(istd)
                for m in range(n_m):
                    z = zp.tile([128, 512], MF, name="z")
                    nc.vector.tensor_mul(z[:], outs[bt][:, m, :], bc[:])
                    nc.scalar.activation(outs[bt][:, m, :], z[:], func,
                                         bias=bet_cols[m], scale=gam_cols[m])

        return stats, fin_a, fin_b, outs

    def m_unit(s, e_tiles, m_streams, wfns):
        """Pair-average matmul; eviction only (adds bias)."""
        sa, sb = m_streams[s]
        srcs = []
        for c in range(8):
            if c < 4:
                rf = (lambda c: (lambda bt: e_tiles[sa][bt][:, c, :]))(c)
            else:
                rf = (lambda c: (lambda bt: e_tiles[sb][bt][:, c - 4, :]))(c)
            srcs.append((wfns[c], rf))
        m_sb = [mp.tile([128, 4, 512], MF, name="m_sb") for _ in range(2)]

        def evict(bt, m, ps):
            nc.vector.tensor_scalar_add(m_sb[bt][:, m, :], ps[:],
                                        b_m[:, s, m:m + 1])

        mm_groups(srcs, 4, evict)
        return m_sb

    def gate_unit(s, e_tiles, m_sb, wfns):
        """Gate matmul -> tanh-sigmoid -> t = e + gate*m -> n2 stats.
        zt = t*istd_bc - (mu*istd)_bc is produced in fin."""
        srcs = []
        for c in range(8):
            if c < 4:
                rf = (lambda c: (lambda bt: e_tiles[s][bt][:, c, :]))(c)
            else:
                rf = (lambda c: (lambda bt: m_sb[bt][:, c - 4, :]))(c)
            srcs.append((wfns[c], rf))
        t_sb = [tp.tile([128, 4, 512], MF, name="t_sb") for _ in range(2)]

        def evict(bt, m, ps):
            # t = e + sigmoid(pre)*m_avg = e + (th+1)*m_tilde
            th = thp.tile([128, 512], MF, name="th")
            nc.scalar.activation(th[:], ps[:], ACT.Tanh,
                                 bias=b_gh[:, s, m:m + 1], scale=0.5)
            u = up.tile([128, 512], MF, name="u")
            nc.vector.scalar_tensor_tensor(u[:], th[:], 1.0,
                                           m_sb[bt][:, m, :],
                                           ALU.add, ALU.mult)
            nc.gpsimd.tensor_add(t_sb[bt][:, m, :], u[:],
                                 e_tiles[s][bt][:, m, :])

        mm_groups(srcs, 4, evict)
        st_sum = [None, None]
        st_sq = [None, None]

        def stats():
            for bt in range(2):
                st_sum[bt] = st_ps.tile([1, 512], F32, name="st_sum",
                                        tag="stat_ps")
                for m in range(4):
                    nc.tensor.matmul(st_sum[bt][:], ones_col[:],
                                     t_sb[bt][:, m, :],
                                     start=(m == 0), stop=(m == 3))
            for bt in range(2):
                st_sq[bt] = st_ps.tile([1, 512], F32, name="st_sq",
                                       tag="stat_ps")
                for m in range(4):
                    sq = sqp.tile([128, 512], MF, name="sq")
                    nc.gpsimd.tensor_mul(sq[:], t_sb[bt][:, m, :],
                                         t_sb[bt][:, m, :])
                    nc.tensor.matmul(st_sq[bt][:], ones_col[:], sq[:],
                                     start=(m == 0), stop=(m == 3))

        hold = {}

        def fin_a():
            mu_rows, v_rows = [], []
            for bt in range(2):
                m_ = stf.tile([1, 512], F32, name="r32")
                nc.scalar.activation(m_[0:1, :], st_sum[bt][:], ACT.Identity,
                                     scale=inv_d[:])
                v = stf.tile([1, 512], F32, name="r32")
                nc.scalar.activation(v[0:1, :], st_sq[bt][:], ACT.Identity,
                                     bias=eps1[:], scale=inv_d[:])
                mu_rows.append(m_)
                v_rows.append(v)
            muS = transpose_rows(mu_rows)
            vS = transpose_rows(v_rows)
            musq = nrp.tile([128, 8], F32, name="nr_t")
            nc.vector.tensor_mul(musq[:], muS[:], muS[:])
            nc.vector.tensor_sub(vS[:], vS[:], musq[:])
            y = nr_rsqrt_T(vS, 2)
            wT = nrp.tile([128, 8], F32, name="nr_w")
            nc.vector.tensor_mul(wT[:], muS[:], y[:])
            hold["y"], hold["w"] = y, wT

        wrows = {}

        def fin_b(bts=(0, 1)):
            # zt = t*istd_bc in place; the -mu*istd mean correction is a
            # rank-1 negc matmul inside f1 (wrows are its rhs rows).
            for bt in bts:
                istd = row_back(hold["y"], 2, bt)
                wrows[bt] = row_back(hold["w"], 2, bt)
                bci = bcast(istd)
                for m in range(4):
                    nc.vector.tensor_mul(t_sb[bt][:, m, :],
                                         t_sb[bt][:, m, :], bci[:])

        return stats, fin_a, fin_b, t_sb, wrows

    # ---------------- unit constructors ----------------
    def make_lp(s):
        srcs = [(lambda m, s=s: w_lp[s][:, ts(m, 128)],
                 lambda bt, s=s: lsg[s][:, ts(bt, 512)])]
        return ln_unit(srcs, 2,
                       [b_lp[:, s, c:c + 1] for c in range(2)],
                       [g_lp[:, s, c:c + 1] for c in range(2)],
                       [be_lp[:, s, c:c + 1] for c in range(2)],
                       ACT.Gelu, lpo, D // 2)

    def make_hp(s, xcs, wfns):
        srcs = [(wfns[c],
                 (lambda c: (lambda bt: xcs[bt * 4 + c // 2][:, c % 2, :]))(c))
                for c in range(8)]
        return ln_unit(srcs, 4,
                       [b_hp[:, s, c:c + 1] for c in range(4)],
                       [g_hp[:, s, c:c + 1] for c in range(4)],
                       [be_hp[:, s, c:c + 1] for c in range(4)],
                       ACT.Gelu, yhp, D)

    def make_r(s, yh):
        srcs = [(wr_fns[c], (lambda c: (lambda bt: yh[bt][:, c, :]))(c))
                for c in range(4)]
        return ln_unit(srcs, 4,
                       [b_r[:, c:c + 1] for c in range(4)],
                       [g_n1[:, s, c:c + 1] for c in range(4)],
                       [be_n1[:, s, c:c + 1] for c in range(4)],
                       ACT.Identity, ep, D)

    def prefetch_f1():
        fns = []
        for s in range(3):
            fns.append(load_wchunks(io["w_f1l"][s], 2))
        for s in (2, 1, 0):
            fns.append(load_wchunks(io["w_f1"][s], 4))
        return fns

    def make_f1(l_tiles, zt_tiles, gate_fbs, wf, wrows_by_s):
        """f1 with the three gate fin_b's interleaved between chunk stages:
        l chunks first, then g2.fb, zt2 chunks, g1.fb, zt1, g0.fb, zt0,
        and the rank-1 mean-correction (negc x mu*istd rows) last."""
        srcs = []
        for s in range(3):
            for c in range(2):
                srcs.append((wf[s][c],
                             (lambda s, c: (lambda bt: l_tiles[s][bt][:, c, :]))(s, c)))
        for i, s in enumerate((2, 1, 0)):
            for c in range(4):
                srcs.append((wf[3 + i][c],
                             (lambda s, c: (lambda bt: zt_tiles[s][bt][:, c, :]))(s, c)))
        for s in (2, 1, 0):
            srcs.append(((lambda m, s=s: negc_t[0:1, s, ts(m, 128)]),
                         (lambda s=s: (lambda bt: wrows_by_s[s][bt][0:1, :]))()))
        fb_at = {0: gate_fbs[2], 6: gate_fbs[1], 10: gate_fbs[0]}

        def emitter(srcs, n_m, evict_fn):
            last = len(srcs) - 1
            for mh in range(0, n_m, 2):
                mis = range(mh, mh + 2)
                ps = {(m, bt): mm_ps.tile([128, 512], F32, name="mm",
                                          tag="mm")
                      for m in mis for bt in range(2)}
                for ci, (lf, rf) in enumerate(srcs):
                    if mh == 0 and ci in fb_at:
                        fb_at[ci]()
                    for m in mis:
                        for bt in range(2):
                            nc.tensor.matmul(ps[(m, bt)][:], lf(m), rf(bt),
                                             start=(ci == 0),
                                             stop=(ci == last))
                for bt in range(2):
                    for m in mis:
                        evict_fn(bt, m, ps[(m, bt)])

        return ln_unit(srcs, 4,
                       [b_f1[:, c:c + 1] for c in range(4)],
                       [g_f1[:, c:c + 1] for c in range(4)],
                       [be_f1[:, c:c + 1] for c in range(4)],
                       ACT.Gelu, hp_, D, mm_emitter=emitter)

    # ---------------- main schedule ----------------
    # Emission order == per-engine execution order (all engines run their
    # queues in order). Each unit's fin is split: fin_a (stat eviction +
    # transposes + NR chain) is emitted early so its latency runs under
    # later matmul blocks; fin_b (back-transposes + broadcast + normalize)
    # is emitted just before the consumer needs the result.
    m_streams = [(1, 2), (0, 2), (0, 1)]

    x_pf.append(load_x(1))
    whp_pf.append(load_wchunks(io["w_hp"][1], 8))
    hp0 = make_hp(0, x_pf[0], whp_pf[0])
    lp_u = [make_lp(s) for s in range(3)]
    x_pf.append(load_x(2))
    whp_pf.append(load_wchunks(io["w_hp"][2], 8))
    hp1 = make_hp(1, x_pf[1], whp_pf[1])
    hp0[0]()                   # hp0 stats
    for s in range(3):
        lp_u[s][0]()           # lp stats
    hp0[1]()                   # hp0 fin_a
    for s in range(3):
        lp_u[s][1]()           # lp fin_a
    hp2 = make_hp(2, x_pf[2], whp_pf[2])
    wf_m2 = load_wchunks(io["w_m"][2], 8)
    hp1[0]()
    hp0[2]()                   # hp0 fin_b -> yh0
    for s in range(3):
        lp_u[s][2]()           # lp fin_b -> l
    r0 = make_r(0, hp0[3])
    hp1[1]()
    hp2[0]()
    hp1[2]()                   # -> yh1
    r1 = make_r(1, hp1[3])
    wf_m1 = load_wchunks(io["w_m"][1], 8)
    r0[0]()
    hp2[1]()
    hp2[2]()                   # -> yh2
    r2 = make_r(2, hp2[3])
    wf_g2 = load_wchunks(io["w_g"][2], 8)
    r0[1]()
    r1[0]()
    r0[2]()                    # -> e0
    r2[0]()
    r1[1]()
    r2[1]()
    r1[2]()                    # -> e1
    e_tiles = [r0[3], r1[3], r2[3]]
    m2 = m_unit(2, e_tiles, m_streams, wf_m2)   # e0 (c0-3), e1 (c4-7)
    wf_m0 = load_wchunks(io["w_m"][0], 8)
    r2[2]()                    # -> e2 (NR ran under m2's matmuls)
    m1 = m_unit(1, e_tiles, m_streams, wf_m1)   # e0 (c0-3), e2 (c4-7)
    wf_g1 = load_wchunks(io["w_g"][1], 8)
    g2 = gate_unit(2, e_tiles, m2, wf_g2)
    m0 = m_unit(0, e_tiles, m_streams, wf_m0)   # e1, e2
    wf_g0 = load_wchunks(io["w_g"][0], 8)
    g2[0]()                    # g2 stats
    g1 = gate_unit(1, e_tiles, m1, wf_g1)
    g2[1]()                    # g2 fin_a
    wf_f1 = prefetch_f1()
    g0 = gate_unit(0, e_tiles, m0, wf_g0)
    wf_f2 = load_wchunks(io["w_f2"], 4)
    g1[0]()
    g0[0]()
    g1[1]()                    # g1 fin_a
    g0[1]()                    # g0 fin_a
    l_tiles = [u[3] for u in lp_u]
    zt_tiles = [g0[3], g1[3], g2[3]]
    f1 = make_f1(l_tiles, zt_tiles, [g0[2], g1[2], g2[2]], wf_f1,
                 {0: g0[4], 1: g1[4], 2: g2[4]})
    f1[0]()
    f1[1]()

    # ---- f2 (final LN), pipelined per batch tile with f1's fin_b ----
    h_tiles = f1[3]
    o_tiles = [None, None]
    f2st = [None, None]
    f2hold = {}

    def f2_mm(bt):
        o_tiles[bt] = op_.tile([128, 4, 512], MF, name="o_sb")
        ps = [mm_ps.tile([128, 512], F32, name="mm", tag="mm")
              for _ in range(4)]
        for ci in range(4):
            for m in range(4):
                nc.tensor.matmul(ps[m][:], wf_f2[ci](m),
                                 h_tiles[bt][:, ci, :],
                                 start=(ci == 0), stop=(ci == 3))
        for m in range(4):
            nc.vector.tensor_scalar_add(o_tiles[bt][:, m, :], ps[m][:],
                                        b_f2[:, m:m + 1])

    def f2_stats(bt):
        f2st[bt] = st_ps.tile([1, 512], F32, name="st", tag="stat_ps")
        for m in range(4):
            sq = sqp.tile([128, 512], MF, name="sq")
            nc.gpsimd.tensor_mul(sq[:], o_tiles[bt][:, m, :],
                                 o_tiles[bt][:, m, :])
            nc.tensor.matmul(f2st[bt][:], ones_col[:], sq[:],
                             start=(m == 0), stop=(m == 3))

    def f2_fa():
        vr = []
        for bt in range(2):
            v = stf.tile([1, 512], F32, name="r32")
            nc.scalar.activation(v[0:1, :], f2st[bt][:], ACT.Identity,
                                 bias=eps1[:], scale=inv_d[:])
            vr.append(v)
        f2hold["y"] = nr_rsqrt_T(transpose_rows(vr), 2)

    def f2_fb(bt):
        istd = row_back(f2hold["y"], 2, bt)
        bc = bcast(istd)
        for m in range(4):
            z = zp.tile([128, 512], MF, name="z")
            nc.vector.tensor_mul(z[:], o_tiles[bt][:, m, :], bc[:])
            nc.scalar.activation(o_tiles[bt][:, m, :], z[:], ACT.Identity,
                                 bias=be_f2[:, m:m + 1],
                                 scale=g_f2[:, m:m + 1])
        nc.sync.dma_start(
            io["outT"].rearrange("(c p) b -> p c b", p=128)[:, :, ts(bt, 512)],
            o_tiles[bt][:])

    f1[2]((0,))                # -> h[bt0]
    f2_mm(0)
    f1[2]((1,))                # -> h[bt1]
    f2_mm(1)
    f2_stats(0)
    f2_stats(1)
    f2_fa()
    f2_fb(0)
    f2_fb(1)

    ctx.close()


def build_program():
    nc = bacc.Bacc("TRN2", target_bir_lowering=False, debug=False,
                   num_devices=NCORES)
    io = {}

    def din(name, shape, dtype=F32):
        io[name] = nc.dram_tensor(name, list(shape), dtype,
                                  kind="ExternalInput").ap()

    for s in range(3):
        din(f"xT{s}", (HID, BL), dtype=MM_DT)
        din(f"lT{s}", (FS[s], BL), dtype=MM_DT)
    din("w_hp", (3, 8, 128, 512), dtype=MM_DT)
    din("b_hp", (3, 128, 4))
    din("w_r", (4, 128, 512), dtype=MM_DT)
    din("b_r", (128, 4))
    din("w_m", (3, 8, 128, 512), dtype=MM_DT)
    din("b_m", (3, 128, 4))
    din("w_g", (3, 8, 128, 512), dtype=MM_DT)
    for s in range(3):
        din(f"w_lp{s}", (FS[s], 256), dtype=MM_DT)
    din("b_lp", (3, 128, 2))
    din("w_f1", (3, 4, 128, 512), dtype=MM_DT)
    din("negc_f1", (1, 3, 512), dtype=MM_DT)
    din("w_f1l", (3, 2, 128, 512), dtype=MM_DT)
    din("b_f1", (128, 4))
    din("w_f2", (4, 128, 512), dtype=MM_DT)
    din("b_f2", (128, 4))
    for name in ("g_hp", "be_hp", "g_n1", "be_n1", "b_g_half"):
        din(name, (3, 128, 4))
    for name in ("g_lp", "be_lp"):
        din(name, (3, 128, 2))
    for name in ("g_f1", "be_f1", "g_f2", "be_f2"):
        din(name, (128, 4))
    din("ones_col", (128, 1), dtype=MM_DT)
    din("ones_row", (1, 128), dtype=MM_DT)
    io["outT"] = nc.dram_tensor("outT", [D, BL], MM_DT,
                                kind="ExternalOutput").ap()

    with tile.TileContext(nc) as tc:
        emit_program(tc, io)
    nc.compile()
    return nc


def make_in_maps(inputs):
    fw = fold_weights(inputs)
    dev = device_arrays(fw)
    hidden = [np.asarray(inputs["verb_hidden"], np.float32),
              np.asarray(inputs["inst_hidden"], np.float32),
              np.asarray(inputs["target_hidden"], np.float32)]
    logits = [np.asarray(inputs["verb_logits"], np.float32),
              np.asarray(inputs["inst_logits"], np.float32),
              np.asarray(inputs["target_logits"], np.float32)]
    sig = [1.0 / (1.0 + np.exp(-np.asarray(l, F64))) for l in logits]
    in_maps = []
    for core in range(NCORES):
        rows = slice(core * BL, (core + 1) * BL)
        m = dict(dev)
        for s in range(3):
            m[f"xT{s}"] = _mf_np(hidden[s][rows].T)
            m[f"lT{s}"] = _mf_np(sig[s][rows].T)
        in_maps.append(m)
    return in_maps


_NC_CACHE = None


def _run(inputs, **spmd_kwargs):
    global _NC_CACHE
    if _NC_CACHE is None:
        _NC_CACHE = build_program()
    nc = _NC_CACHE
    in_maps = make_in_maps(inputs)
    res = run_bass_kernel_spmd(nc, in_maps, list(range(NCORES)),
                               **spmd_kwargs)
    out = np.empty((B, D), dtype=np.float32)
    for core in range(NCORES):
        out[core * BL:(core + 1) * BL] = np.asarray(
            res.results[core]["outT"], dtype=np.float32).T
    return out, res


def kernel(**inputs) -> np.ndarray:
    return _run(inputs)[0]


def kernel_profiled(inputs, tmpdir=None):
    """Returns (out, BassKernelResults) with an NTFF-based profile."""
    return _run(inputs, trace=True, tmpdir=tmpdir)


# revision 94
# speedup vs baseline: 1.2106x; 1.0008x over previous
"""Trainium2 Bass kernel for nn_AttentionModule_7146825580577.

Strategy (see spec sharding_hint): pure data parallel over the batch dim
(8192 rows -> 1024 rows per core, 8 cores), weights replicated.

Schedule: the two 512-column batch tiles of each core are interleaved
inside every layer unit so the Tensor engine always has independent
matmul work queued; weights are streamed from HBM once (each chunk feeds
both batch tiles, with DMAs prefetched a unit ahead since the Sync
engine issues them serially); all matmul operands are bf16. Every
engine executes its queue in emission order, so each LayerNorm "fin" is
split into fin_a (stat eviction + PE transposes + Newton-Raphson rsqrt)
emitted early, and fin_b (back-transposes + PE outer-product broadcast
+ normalize/activation) emitted just before the consuming matmuls.

Device math (per core), in feature-transposed layout (features on SBUF
partitions, batch on the free dim):

  - All LayerNorms whose input is an affine function of a previous
    activation use host-side column-centered weights, so mean(y) == 0 by
    construction and only sum(y^2) is needed on device (squares on
    GPSIMD, reduced over partitions by a ones-vector matmul on the PE).
  - seq_len==1 MHA reduces to out_proj(v_proj(kv)); both projections are
    fused on the host into a single 512x512 effective matrix. The self-
    attention residual (x + sa(x)) is folded into a single matmul with
    weights I + Wv@Wo.
  - The cross-attention pair average (a+b)/2 is a single concat-matmul,
    prescaled by 0.25 on the host so the gate combine is
    t = e + (tanh(pre/2 + b/2) + 1) * m_tilde (sigmoid via tanh keeps
    the scalar engine on a single activation table).
  - The n2 LayerNorm (after gating) is folded into the fus_W1 matmul:
    gamma scales fold into the weights, betas fold into the bias; the
    per-sample mean correction is a rank-1 (negc x mu*istd-row) matmul
    appended to the f1 accumulation group.
  - 1/sqrt(var+eps) uses the int32 bit trick + one Newton-Raphson step
    on PE-transposed [128, k] stat tiles (tiny DVE ops), transposed back
    and broadcast across partitions via a PE outer product.
"""
import os
import sys

sys.path.insert(0, "/opt/trn_rl_repo")

import numpy as np

import concourse.bass as bass
import concourse.tile as tile
from concourse import bacc, mybir
from concourse.bass import ts
from concourse.bass_utils import run_bass_kernel_spmd
from concourse.masks import make_identity

D = 512
HID = 1024
B = 8192
NCORES = 8
BL = B // NCORES          # rows per core
EPS = 1e-5
MAGIC = 0x5F3759DF
F32 = mybir.dt.float32
I32 = mybir.dt.int32
FS = [10, 6, 15]          # logit dims per stream
NR_ITERS = int(os.environ.get("KERNEL_NR_ITERS", "1"))
MM_DT = {
    "f32r": mybir.dt.float32r,
    "f32": mybir.dt.float32,
    "bf16": mybir.dt.bfloat16,
}[os.environ.get("KERNEL_MM_DTYPE", "bf16")]
MM_IS_BF16 = MM_DT == mybir.dt.bfloat16
MF = MM_DT

F64 = np.float64


# --------------------------------------------------------------------------
# Host-side weight folding
# --------------------------------------------------------------------------

def _center_cols(W, b):
    W = np.asarray(W, F64)
    b = np.asarray(b, F64)
    return W - W.mean(axis=1, keepdims=True), b - b.mean()


def fold_weights(inp):
    g = lambda k: np.asarray(inp[k], dtype=F64)
    out = {}

    w_hp, b_hp = [], []
    for s in range(3):
        W, b = _center_cols(g("hp_W")[s], g("hp_b")[s])
        w_hp.append(W)
        b_hp.append(b)
    out["w_hp"] = np.stack(w_hp)
    out["b_hp"] = np.stack(b_hp)
    out["g_hp"], out["be_hp"] = g("hp_g"), g("hp_be")

    mhaW, mhab = g("mha_in_W"), g("mha_in_b")
    moW, mob = g("mha_out_W"), g("mha_out_b")
    Wv0, bv0 = mhaW[0][:, 2 * D:], mhab[0][2 * D:]
    Wr, br = _center_cols(np.eye(D) + Wv0 @ moW[0], bv0 @ moW[0] + mob[0])
    out["w_r"], out["b_r"] = Wr, br
    out["g_n1"], out["be_n1"] = g("n1_g"), g("n1_be")

    Wj, bj = [None] * 4, [None] * 4
    for j in (1, 2, 3):
        Wv, bv = mhaW[j][:, 2 * D:], mhab[j][2 * D:]
        Wj[j] = Wv @ moW[j]
        bj[j] = bv @ moW[j] + mob[j]
    # m_verb uses (inst_e @ W1, target_e @ W2); m_inst (verb @ W1, target @ W3);
    # m_target (verb @ W2, inst @ W3)
    # m is stored pre-halved (0.25 = average 0.5 x sigmoid-via-tanh 0.5):
    # t = e + sigmoid(pre)*m_avg = (e + m_tilde) + tanh(pre/2)*m_tilde
    # with m_tilde = 0.5*m_avg.
    mods = [(1, 2), (1, 3), (2, 3)]
    w_m, b_m = [], []
    for s in range(3):
        ja, jb = mods[s]
        w_m.append(np.concatenate([0.25 * Wj[ja], 0.25 * Wj[jb]], axis=0))
        b_m.append(0.25 * (bj[ja] + bj[jb]))
    out["w_m"] = np.stack(w_m)
    out["b_m"] = np.stack(b_m)

    out["w_g"] = g("gate_W")
    # tanh trick: sigmoid(x + b) = 0.5*tanh(0.5*x + 0.5*b) + 0.5
    out["b_g_half"] = 0.5 * g("gate_b")

    w_lp, b_lp = [], []
    for s, key in enumerate(["verb", "inst", "target"]):
        W, b = _center_cols(g(f"lp_W_{key}"), g(f"lp_b_{key}"))
        w_lp.append(W)
        b_lp.append(b)
    out["w_lp"] = w_lp
    out["b_lp"] = np.stack(b_lp)
    out["g_lp"], out["be_lp"] = g("lp_g"), g("lp_be")

    W1 = g("fus_W1")
    g2, be2 = g("n2_g"), g("n2_be")
    A1, negc = [], []
    bias_total = g("fus_b1").copy()
    for s in range(3):
        blk = W1[s * D:(s + 1) * D]
        A = g2[s][:, None] * blk
        c = blk.T @ g2[s]
        A1.append(A - A.mean(axis=1, keepdims=True))
        negc.append(-(c - c.mean()))
        bias_total += be2[s] @ blk
    L1 = []
    for s in range(3):
        off = 3 * D + s * (D // 2)
        blk = W1[off: off + D // 2]
        L1.append(blk - blk.mean(axis=1, keepdims=True))
    out["w_f1"] = np.stack(A1)
    out["negc_f1"] = np.stack(negc)
    out["w_f1l"] = np.stack(L1)
    out["b_f1"] = bias_total - bias_total.mean()
    out["g_f1"], out["be_f1"] = g("fus_g1"), g("fus_ge1")

    W2c, b2c = _center_cols(g("fus_W2"), g("fus_b2"))
    out["w_f2"], out["b_f2"] = W2c, b2c
    out["g_f2"], out["be_f2"] = g("fus_g2"), g("fus_ge2")
    return out


def _vec_pp(v, nk):
    """[.., nk*128] feature vector -> ACT per-partition layout [.., 128, nk]."""
    v = np.asarray(v, np.float32)
    return np.ascontiguousarray(v.reshape(v.shape[:-1] + (nk, 128)).swapaxes(-1, -2))


def _mf_np(v):
    """Host array in the matmul dtype (bf16 or fp32)."""
    if MM_IS_BF16:
        import ml_dtypes
        return np.ascontiguousarray(np.asarray(v, np.float32).astype(
            ml_dtypes.bfloat16))
    return np.ascontiguousarray(np.asarray(v, np.float32))


def device_arrays(fw):
    """Folded weights -> dict of arrays matching the DRAM tensor decls."""
    f32 = _mf_np
    dev = {}
    dev["w_hp"] = f32(fw["w_hp"].reshape(3, 8, 128, 512))
    dev["b_hp"] = _vec_pp(fw["b_hp"], 4)
    dev["w_r"] = f32(fw["w_r"].reshape(4, 128, 512))
    dev["b_r"] = _vec_pp(fw["b_r"], 4)
    dev["w_m"] = f32(fw["w_m"].reshape(3, 8, 128, 512))
    dev["b_m"] = _vec_pp(fw["b_m"], 4)
    dev["w_g"] = f32(fw["w_g"].reshape(3, 8, 128, 512))
    for s in range(3):
        dev[f"w_lp{s}"] = f32(fw["w_lp"][s])
    dev["b_lp"] = _vec_pp(fw["b_lp"], 2)
    dev["w_f1"] = f32(fw["w_f1"].reshape(3, 4, 128, 512))
    dev["negc_f1"] = f32(fw["negc_f1"][None])
    dev["w_f1l"] = f32(fw["w_f1l"].reshape(3, 2, 128, 512))
    dev["b_f1"] = _vec_pp(fw["b_f1"], 4)
    dev["w_f2"] = f32(fw["w_f2"].reshape(4, 128, 512))
    dev["b_f2"] = _vec_pp(fw["b_f2"], 4)
    for name in ("g_hp", "be_hp", "g_n1", "be_n1"):
        dev[name] = _vec_pp(fw[name], 4)
    dev["b_g_half"] = _vec_pp(fw["b_g_half"], 4)
    dev["g_lp"] = _vec_pp(fw["g_lp"], 2)
    dev["be_lp"] = _vec_pp(fw["be_lp"], 2)
    for name in ("g_f1", "be_f1", "g_f2", "be_f2"):
        dev[name] = _vec_pp(fw[name], 4)
    dev["ones_col"] = _mf_np(np.ones((128, 1), np.float32))
    dev["ones_row"] = _mf_np(np.ones((1, 128), np.float32))
    return dev


# --------------------------------------------------------------------------
# Device program
# --------------------------------------------------------------------------

ACT = mybir.ActivationFunctionType
ALU = mybir.AluOpType


def emit_program(tc, io):
    nc = tc.nc
    from contextlib import ExitStack
    ctx = ExitStack()

    # ---------------- pools ----------------
    P = lambda name, bufs, space="SBUF": ctx.enter_context(
        tc.tile_pool(name=name, bufs=bufs, space=space))
    const = P("const", 1)
    wpool = P("wchunk", 11)     # [128,2,512] MF weight pair chunks
    #                             (f1 holds 9 pairs + f2 prefetch)
    xpool = P("xchunk", 8)      # [128,2,512] MF input pair chunks
    #                             (all 8 live across both m-halves of a unit)
    sqp = P("sq", 7)            # [128,512] MF squares (live until deferred
    #                             stats matmuls run, a unit later)
    thp = P("th", 2)            # [128,512] MF gate tanh tiles
    zp = P("z", 2)              # [128,512] MF z = y*istd tiles
    up = P("u", 2)              # [128,512] MF gate (th+1)*m tiles
    yhp = P("yh", 6)            # [128,4,512] MF hp outputs (alive hp->r)
    ep = P("e", 6)              # [128,4,512] MF n1 outputs (alive r->gate)
    mp = P("m", 4)              # [128,4,512] MF pair-average (alive m->gate)
    tp = P("t", 6)              # [128,4,512] MF gate t; normalized in place,
    #                             alive until f1 consumes it
    lpo = P("l", 6)             # [128,2,512] MF lp outputs (alive ->f1)
    hp_ = P("h", 2)             # [128,4,512] MF f1 outputs
    op_ = P("o", 2)             # [128,4,512] MF f2 outputs
    lsp = P("ls", 1)            # [Fs,1024] MF host-sigmoided logits
    stf = P("stats_f32", 6)     # [1,512] F32 stat rows
    stb = P("stats_mf", 8)      # [1,512] MF istd/wrow rows (gate wrow rows
    #                             stay live until f1's rank-1 matmuls)
    nrp = P("nr", 4)            # [128,8] F32 NR tiles
    bcp = P("bc_sb", 2)         # [128,512] MF broadcast rows
    mm_ps = P("mm_ps", 4, "PSUM")
    st_ps = P("st_ps", 4, "PSUM")

    # ---------------- DMA helpers (needed for the head prefetch) --------
    def wpair(dram_pair_ap):
        """Load two [128,512] k-chunks in one DMA -> [128,2,512] tile."""
        wc = wpool.tile([128, 2, 512], MF, name="wcp", tag="wcp")
        nc.sync.dma_start(wc[:], dram_pair_ap.rearrange("c p n -> p c n"))
        return wc

    def load_wchunks(dram_4d, nk):
        """nk k-chunks -> list of per-chunk lhsT accessors f(m)->[128,128]."""
        fns = []
        for c0 in range(0, nk, 2):
            wc = wpair(dram_4d[c0:c0 + 2])
            for cc in range(2):
                fns.append(lambda m, wc=wc, cc=cc: wc[:, cc, ts(m, 128)])
        return fns

    def load_x(s):
        xcs = []
        for bt in range(2):
            for cp in range(4):
                xc = xpool.tile([128, 2, 512], MF, name="xc")
                nc.sync.dma_start(xc[:], io[f"xT{s}"][bt, cp])
                xcs.append(xc)
        return xcs

    # The Sync engine issues DMAs in emission order at ~0.7us apiece, so
    # the first compute units' inputs must be first in the queue.
    x_pf = [load_x(0)]
    whp_pf = [load_wchunks(io["w_hp"][0], 8)]
    lsg = []
    for s in range(3):
        t = lsp.tile([FS[s], 1024], MF, name=f"lsg{s}")
        nc.sync.dma_start(t[:], io[f"lT{s}"])
        lsg.append(t)

    # ---------------- constants / resident weights ----------------
    ident = const.tile([128, 128], F32)
    make_identity(nc, ident)
    ones_col = const.tile([128, 1], MF)
    nc.sync.dma_start(ones_col[:], io["ones_col"])
    ones_row = const.tile([1, 128], MF)
    nc.sync.dma_start(ones_row[:], io["ones_row"])

    def fconst(value, name):
        t = const.tile([1, 1], F32, name=name)
        nc.gpsimd.memset(t[:], value)
        return t
    eps1 = fconst(EPS, "eps1")
    inv_d = fconst(1.0 / D, "inv_d")
    inv_d2 = fconst(2.0 / D, "inv_d2")

    def load(name, shape, rearr=None, dtype=F32):
        t = const.tile(shape, dtype, name=name)
        src = io[name]
        if rearr:
            src = src.rearrange(rearr)
        nc.sync.dma_start(t[:], src)
        return t

    b_hp = load("b_hp", [128, 3, 4], "s p c -> p s c")
    b_r = load("b_r", [128, 4])
    b_m = load("b_m", [128, 3, 4], "s p c -> p s c")
    b_lp = load("b_lp", [128, 3, 2], "s p c -> p s c")
    b_f1 = load("b_f1", [128, 4])
    b_f2 = load("b_f2", [128, 4])
    g_hp = load("g_hp", [128, 3, 4], "s p c -> p s c")
    be_hp = load("be_hp", [128, 3, 4], "s p c -> p s c")
    g_n1 = load("g_n1", [128, 3, 4], "s p c -> p s c")
    be_n1 = load("be_n1", [128, 3, 4], "s p c -> p s c")
    b_gh = load("b_g_half", [128, 3, 4], "s p c -> p s c")
    g_lp = load("g_lp", [128, 3, 2], "s p c -> p s c")
    be_lp = load("be_lp", [128, 3, 2], "s p c -> p s c")
    g_f1 = load("g_f1", [128, 4])
    be_f1 = load("be_f1", [128, 4])
    g_f2 = load("g_f2", [128, 4])
    be_f2 = load("be_f2", [128, 4])
    negc_t = load("negc_f1", [1, 3, 512], dtype=MF)
    w_lp = [load(f"w_lp{s}", [FS[s], 256], dtype=MF) for s in range(3)]
    # w_r is shared by all three r units: load once into the const pool.
    w_r_t = []
    for c0 in (0, 2):
        t = const.tile([128, 2, 512], MF, name=f"w_r{c0}")
        nc.sync.dma_start(t[:], io["w_r"][c0:c0 + 2].rearrange("c p n -> p c n"))
        w_r_t.append(t)
    wr_fns = [(lambda m, t=w_r_t[c // 2], cc=c % 2: t[:, cc, ts(m, 128)])
              for c in range(4)]

    # ---------------- helpers ----------------
    def mm_groups(srcs, n_m, evict_fn):
        """srcs: list of (lhsT_fn(m), rhs_fn(bt)). Emits matmuls in two
        m-halves; after each half's accumulation completes, evict_fn(bt, m,
        ps) is called. bt is innermost so consecutive matmuls share the
        stationary operand."""
        last = len(srcs) - 1
        for mh in range(0, n_m, 2):
            mis = range(mh, min(mh + 2, n_m))
            ps = {(m, bt): mm_ps.tile([128, 512], F32, name="mm", tag="mm")
                  for m in mis for bt in range(2)}
            for ci, (lf, rf) in enumerate(srcs):
                for m in mis:
                    for bt in range(2):
                        nc.tensor.matmul(ps[(m, bt)][:], lf(m), rf(bt),
                                         start=(ci == 0), stop=(ci == last))
            for bt in range(2):
                for m in mis:
                    evict_fn(bt, m, ps[(m, bt)])

    def transpose_rows(rows):
        """PE-transpose k [1,512] sbuf rows into one [128,4k] SBUF tile
        (via PSUM) so the per-sample scalar math runs on fat tiles."""
        k = len(rows)
        vT = st_ps.tile([128, 4 * k], F32, name="vT", tag="stat_ps")
        for c in range(4):
            for i, v in enumerate(rows):
                nc.tensor.transpose(vT[:, c * k + i:c * k + i + 1],
                                    v[0:1, ts(c, 128)], ident[0:1, 0:1])
        vs = nrp.tile([128, 4 * k], F32, name="nr_v")
        nc.vector.tensor_copy(vs[:], vT[:])
        return vs

    def nr_rsqrt_T(vs, k):
        """Newton-Raphson rsqrt of a transposed [128,4k] tile (GPSIMD)."""
        y = nrp.tile([128, 4 * k], F32, name="nr_y")
        t = nrp.tile([128, 4 * k], F32, name="nr_t")
        nc.vector.tensor_scalar(y[:].bitcast(I32), vs[:].bitcast(I32),
                                1, None, ALU.logical_shift_right)
        nc.vector.tensor_scalar(y[:].bitcast(I32), y[:].bitcast(I32),
                                -1, MAGIC, ALU.mult, ALU.add)
        for _ in range(NR_ITERS):
            nc.vector.tensor_mul(t[:], y[:], y[:])
            nc.vector.tensor_mul(t[:], t[:], vs[:])
            nc.vector.tensor_scalar(t[:], t[:], -0.5, 1.5, ALU.mult, ALU.add)
            nc.vector.tensor_mul(y[:], y[:], t[:])
        return y

    def row_back(y, k, i):
        """Transpose column set i of [128,4k] back to a [1,512] MF row."""
        rT = st_ps.tile([1, 512], F32, name="rT", tag="stat_ps")
        for c in range(4):
            nc.tensor.transpose(rT[0:1, ts(c, 128)],
                                y[:, c * k + i:c * k + i + 1], ident)
        row = stb.tile([1, 512], MF, name="r16")
        nc.vector.tensor_copy(row[:], rT[:])
        return row

    def bcast(row):
        """[1,512] row -> [128,512] MF tile via PE outer product."""
        bps = st_ps.tile([128, 512], F32, name="bc_ps", tag="stat_ps")
        nc.tensor.matmul(bps[:], ones_row[:], row[0:1, :],
                         start=True, stop=True)
        bc = bcp.tile([128, 512], MF, name="bc")
        nc.scalar.activation(bc[:], bps[:], ACT.Identity)
        return bc

    # ---------------- unit emitters ----------------
    # Each unit emits its matmuls+evictions inline and returns
    # (stats_fn, fin_fn) closures to be sequenced by the main schedule.

    def ln_unit(srcs, n_m, bias_cols, gam_cols, bet_cols, func, out_pool,
                dim, mm_emitter=None):
        """Generic matmul->LN->activation unit over both batch tiles.
        PSUM is evicted (bias added) straight into the unit's output tile;
        the final activation overwrites the same slice in place.
        Returns (stats_fn, fin_a_fn, fin_b_fn, outs)."""
        outs = [None, None]

        def evict(bt, m, ps):
            if outs[bt] is None:
                outs[bt] = out_pool.tile([128, n_m, 512], MF, name="out")
            nc.vector.tensor_scalar_add(outs[bt][:, m, :], ps[:],
                                        bias_cols[m])

        (mm_emitter or mm_groups)(srcs, n_m, evict)
        st = [None, None]

        def stats():
            for bt in range(2):
                st[bt] = st_ps.tile([1, 512], F32, name="st", tag="stat_ps")
                for m in range(n_m):
                    sq = sqp.tile([128, 512], MF, name="sq")
                    nc.gpsimd.tensor_mul(sq[:], outs[bt][:, m, :],
                                         outs[bt][:, m, :])
                    nc.tensor.matmul(st[bt][:], ones_col[:], sq[:],
                                     start=(m == 0), stop=(m == n_m - 1))

        hold = {}

        def fin_a():
            vr = []
            for bt in range(2):
                v = stf.tile([1, 512], F32, name="r32")
                nc.scalar.activation(v[0:1, :], st[bt][:], ACT.Identity,
                                     bias=eps1[:],
                                     scale=(inv_d if dim == D else inv_d2)[:])
                vr.append(v)
            hold["y"] = nr_rsqrt_T(transpose_rows(vr), 2)

        def fin_b(bts=(0, 1)):
            for bt in bts:
                istd = row_back(hold["y"], 2, bt)
                bc = bcast(istd)
                for m in range(n_m):
                    z = zp.tile([128, 512], MF, name="z")
                    nc.vector.tensor_mul(z[:], outs[bt][:, m, :], bc[:])
                    nc.scalar.activation(outs[bt][:, m, :], z[:], func,
                                         bias=bet_cols[m], scale=gam_cols[m])

        return stats, fin_a, fin_b, outs

    def m_unit(s, e_tiles, m_streams, wfns):
        """Pair-average matmul; eviction only (adds bias)."""
        sa, sb = m_streams[s]
        srcs = []
        for c in range(8):
            if c < 4:
                rf = (lambda c: (lambda bt: e_tiles[sa][bt][:, c, :]))(c)
            else:
                rf = (lambda c: (lambda bt: e_tiles[sb][bt][:, c - 4, :]))(c)
            srcs.append((wfns[c], rf))
        m_sb = [mp.tile([128, 4, 512], MF, name="m_sb") for _ in range(2)]

        def evict(bt, m, ps):
            nc.vector.tensor_scalar_add(m_sb[bt][:, m, :], ps[:],
                                        b_m[:, s, m:m + 1])

        mm_groups(srcs, 4, evict)
        return m_sb

    def gate_unit(s, e_tiles, m_sb, wfns):
        """Gate matmul -> tanh-sigmoid -> t = e + gate*m -> n2 stats.
        zt = t*istd_bc - (mu*istd)_bc is produced in fin."""
        srcs = []
        for c in range(8):
            if c < 4:
                rf = (lambda c: (lambda bt: e_tiles[s][bt][:, c, :]))(c)
            else:
                rf = (lambda c: (lambda bt: m_sb[bt][:, c - 4, :]))(c)
            srcs.append((wfns[c], rf))
        t_sb = [tp.tile([128, 4, 512], MF, name="t_sb") for _ in range(2)]

        def evict(bt, m, ps):
            # t = e + sigmoid(pre)*m_avg = e + (th+1)*m_tilde
            th = thp.tile([128, 512], MF, name="th")
            nc.scalar.activation(th[:], ps[:], ACT.Tanh,
                                 bias=b_gh[:, s, m:m + 1], scale=0.5)
            u = up.tile([128, 512], MF, name="u")
            nc.vector.scalar_tensor_tensor(u[:], th[:], 1.0,
                                           m_sb[bt][:, m, :],
                                           ALU.add, ALU.mult)
            nc.gpsimd.tensor_add(t_sb[bt][:, m, :], u[:],
                                 e_tiles[s][bt][:, m, :])

        mm_groups(srcs, 4, evict)
        st_sum = [None, None]
        st_sq = [None, None]

        def stats():
            for bt in range(2):
                st_sum[bt] = st_ps.tile([1, 512], F32, name="st_sum",
                                        tag="stat_ps")
                for m in range(4):
                    nc.tensor.matmul(st_sum[bt][:], ones_col[:],
                                     t_sb[bt][:, m, :],
                                     start=(m == 0), stop=(m == 3))
            for bt in range(2):
                st_sq[bt] = st_ps.tile([1, 512], F32, name="st_sq",
                                       tag="stat_ps")
                for m in range(4):
                    sq = sqp.tile([128, 512], MF, name="sq")
                    nc.gpsimd.tensor_mul(sq[:], t_sb[bt][:, m, :],
                                         t_sb[bt][:, m, :])
                    nc.tensor.matmul(st_sq[bt][:], ones_col[:], sq[:],
                                     start=(m == 0), stop=(m == 3))

        hold = {}

        def fin_a():
            mu_rows, v_rows = [], []
            for bt in range(2):
                m_ = stf.tile([1, 512], F32, name="r32")
                nc.scalar.activation(m_[0:1, :], st_sum[bt][:], ACT.Identity,
                                     scale=inv_d[:])
                v = stf.tile([1, 512], F32, name="r32")
                nc.scalar.activation(v[0:1, :], st_sq[bt][:], ACT.Identity,
                                     bias=eps1[:], scale=inv_d[:])
                mu_rows.append(m_)
                v_rows.append(v)
            muS = transpose_rows(mu_rows)
            vS = transpose_rows(v_rows)
            musq = nrp.tile([128, 8], F32, name="nr_t")
            nc.vector.tensor_mul(musq[:], muS[:], muS[:])
            nc.vector.tensor_sub(vS[:], vS[:], musq[:])
            y = nr_rsqrt_T(vS, 2)
            wT = nrp.tile([128, 8], F32, name="nr_w")
            nc.vector.tensor_mul(wT[:], muS[:], y[:])
            hold["y"], hold["w"] = y, wT

        wrows = {}

        def fin_b(bts=(0, 1)):
            # zt = t*istd_bc in place; the -mu*istd mean correction is a
            # rank-1 negc matmul inside f1 (wrows are its rhs rows).
            for bt in bts:
                istd = row_back(hold["y"], 2, bt)
                wrows[bt] = row_back(hold["w"], 2, bt)
                bci = bcast(istd)
                for m in range(4):
                    nc.vector.tensor_mul(t_sb[bt][:, m, :],
                                         t_sb[bt][:, m, :], bci[:])

        return stats, fin_a, fin_b, t_sb, wrows

    # ---------------- unit constructors ----------------
    def make_lp(s):
        srcs = [(lambda m, s=s: w_lp[s][:, ts(m, 128)],
                 lambda bt, s=s: lsg[s][:, ts(bt, 512)])]
        return ln_unit(srcs, 2,
                       [b_lp[:, s, c:c + 1] for c in range(2)],
                       [g_lp[:, s, c:c + 1] for c in range(2)],
                       [be_lp[:, s, c:c + 1] for c in range(2)],
                       ACT.Gelu, lpo, D // 2)

    def make_hp(s, xcs, wfns):
        srcs = [(wfns[c],
                 (lambda c: (lambda bt: xcs[bt * 4 + c // 2][:, c % 2, :]))(c))
                for c in range(8)]
        return ln_unit(srcs, 4,
                       [b_hp[:, s, c:c + 1] for c in range(4)],
                       [g_hp[:, s, c:c + 1] for c in range(4)],
                       [be_hp[:, s, c:c + 1] for c in range(4)],
                       ACT.Gelu, yhp, D)

    def make_r(s, yh):
        srcs = [(wr_fns[c], (lambda c: (lambda bt: yh[bt][:, c, :]))(c))
                for c in range(4)]
        return ln_unit(srcs, 4,
                       [b_r[:, c:c + 1] for c in range(4)],
                       [g_n1[:, s, c:c + 1] for c in range(4)],
                       [be_n1[:, s, c:c + 1] for c in range(4)],
                       ACT.Identity, ep, D)

    def prefetch_f1():
        fns = []
        for s in range(3):
            fns.append(load_wchunks(io["w_f1l"][s], 2))
        for s in (2, 1, 0):
            fns.append(load_wchunks(io["w_f1"][s], 4))
        return fns

    def make_f1(l_tiles, zt_tiles, gate_fbs, wf, wrows_by_s):
        """f1 with the three gate fin_b's interleaved between chunk stages:
        l chunks first, then g2.fb, zt2 chunks, g1.fb, zt1, g0.fb, zt0,
        and the rank-1 mean-correction (negc x mu*istd rows) last."""
        srcs = []
        for s in range(3):
            for c in range(2):
                srcs.append((wf[s][c],
                             (lambda s, c: (lambda bt: l_tiles[s][bt][:, c, :]))(s, c)))
        for i, s in enumerate((2, 1, 0)):
            for c in range(4):
                srcs.append((wf[3 + i][c],
                             (lambda s, c: (lambda bt: zt_tiles[s][bt][:, c, :]))(s, c)))
        for s in (2, 1, 0):
            srcs.append(((lambda m, s=s: negc_t[0:1, s, ts(m, 128)]),
                         (lambda s=s: (lambda bt: wrows_by_s[s][bt][0:1, :]))()))
        fb_at = {0: gate_fbs[2], 6: gate_fbs[1], 10: gate_fbs[0]}

        def emitter(srcs, n_m, evict_fn):
            last = len(srcs) - 1
            for mh in range(0, n_m, 2):
                mis = range(mh, mh + 2)
                ps = {(m, bt): mm_ps.tile([128, 512], F32, name="mm",
                                          tag="mm")
                      for m in mis for bt in range(2)}
                for ci, (lf, rf) in enumerate(srcs):
                    if mh == 0 and ci in fb_at:
                        fb_at[ci]()
                    for m in mis:
                        for bt in range(2):
                            nc.tensor.matmul(ps[(m, bt)][:], lf(m), rf(bt),
                                             start=(ci == 0),
                                             stop=(ci == last))
                for bt in range(2):
                    for m in mis:
                        evict_fn(bt, m, ps[(m, bt)])

        return ln_unit(srcs, 4,
                       [b_f1[:, c:c + 1] for c in range(4)],
                       [g_f1[:, c:c + 1] for c in range(4)],
                       [be_f1[:, c:c + 1] for c in range(4)],
                       ACT.Gelu, hp_, D, mm_emitter=emitter)

    # ---------------- main schedule ----------------
    # Emission order == per-engine execution order (all engines run their
    # queues in order). Each unit's fin is split: fin_a (stat eviction +
    # transposes + NR chain) is emitted early so its latency runs under
    # later matmul blocks; fin_b (back-transposes + broadcast + normalize)
    # is emitted just before the consumer needs the result.
    m_streams = [(1, 2), (0, 2), (0, 1)]

    x_pf.append(load_x(1))
    whp_pf.append(load_wchunks(io["w_hp"][1], 8))
    hp0 = make_hp(0, x_pf[0], whp_pf[0])
    lp_u = [make_lp(s) for s in range(3)]
    x_pf.append(load_x(2))
    whp_pf.append(load_wchunks(io["w_hp"][2], 8))
    hp1 = make_hp(1, x_pf[1], whp_pf[1])
    hp0[0]()                   # hp0 stats
    for s in range(3):
        lp_u[s][0]()           # lp stats
    hp0[1]()                   # hp0 fin_a
    for s in range(3):
        lp_u[s][1]()           # lp fin_a
    hp2 = make_hp(2, x_pf[2], whp_pf[2])
    wf_m2 = load_wchunks(io["w_m"][2], 8)
    hp1[0]()
    hp0[2]()                   # hp0 fin_b -> yh0
    for s in range(3):
        lp_u[s][2]()           # lp fin_b -> l
    r0 = make_r(0, hp0[3])
    hp1[1]()
    hp2[0]()
    hp1[2]()                   # -> yh1
    r1 = make_r(1, hp1[3])
    wf_m1 = load_wchunks(io["w_m"][1], 8)
    r0[0]()
    hp2[1]()
    hp2[2]()                   # -> yh2
    r2 = make_r(2, hp2[3])
    wf_g2 = load_wchunks(io["w_g"][2], 8)
    r0[1]()
    r1[0]()
    r0[2]()                    # -> e0
    r2[0]()
    r1[1]()
    r2[1]()
    r1[2]()                    # -> e1
    e_tiles = [r0[3], r1[3], r2[3]]
    m2 = m_unit(2, e_tiles, m_streams, wf_m2)   # e0 (c0-3), e1 (c4-7)
    wf_m0 = load_wchunks(io["w_m"][0], 8)
    r2[2]()                    # -> e2 (NR ran under m2's matmuls)
    m1 = m_unit(1, e_tiles, m_streams, wf_m1)   # e0 (c0-3), e2 (c4-7)
    wf_g1 = load_wchunks(io["w_g"][1], 8)
    g2 = gate_unit(2, e_tiles, m2, wf_g2)
    m0 = m_unit(0, e_tiles, m_streams, wf_m0)   # e1, e2
    wf_g0 = load_wchunks(io["w_g"][0], 8)
    g2[0]()                    # g2 stats
    g1 = gate_unit(1, e_tiles, m1, wf_g1)
    g2[1]()                    # g2 fin_a
    wf_f1 = prefetch_f1()
    g0 = gate_unit(0, e_tiles, m0, wf_g0)
    wf_f2 = load_wchunks(io["w_f2"], 4)
    g1[0]()
    g0[0]()
    g1[1]()                    # g1 fin_a
    g0[1]()                    # g0 fin_a
    l_tiles = [u[3] for u in lp_u]
    zt_tiles = [g0[3], g1[3], g2[3]]
    f1 = make_f1(l_tiles, zt_tiles, [g0[2], g1[2], g2[2]], wf_f1,
                 {0: g0[4], 1: g1[4], 2: g2[4]})
    f1[0]()
    f1[1]()

    # ---- f2 (final LN), pipelined per batch tile with f1's fin_b ----
    h_tiles = f1[3]
    o_tiles = [None, None]
    f2st = [None, None]
    f2hold = {}

    def f2_mm(bt):
        o_tiles[bt] = op_.tile([128, 4, 512], MF, name="o_sb")
        ps = [mm_ps.tile([128, 512], F32, name="mm", tag="mm")
              for _ in range(4)]
        for ci in range(4):
            for m in range(4):
                nc.tensor.matmul(ps[m][:], wf_f2[ci](m),
                                 h_tiles[bt][:, ci, :],
                                 start=(ci == 0), stop=(ci == 3))
        for m in range(4):
            nc.vector.tensor_scalar_add(o_tiles[bt][:, m, :], ps[m][:],
                                        b_f2[:, m:m + 1])

    def f2_stats(bt):
        f2st[bt] = st_ps.tile([1, 512], F32, name="st", tag="stat_ps")
        for m in range(4):
            sq = sqp.tile([128, 512], MF, name="sq")
            nc.gpsimd.tensor_mul(sq[:], o_tiles[bt][:, m, :],
                                 o_tiles[bt][:, m, :])
            nc.tensor.matmul(f2st[bt][:], ones_col[:], sq[:],
                             start=(m == 0), stop=(m == 3))

    def f2_fa():
        vr = []
        for bt in range(2):
            v = stf.tile([1, 512], F32, name="r32")
            nc.scalar.activation(v[0:1, :], f2st[bt][:], ACT.Identity,
                                 bias=eps1[:], scale=inv_d[:])
            vr.append(v)
        f2hold["y"] = nr_rsqrt_T(transpose_rows(vr), 2)

    def f2_fb(bt):
        istd = row_back(f2hold["y"], 2, bt)
        bc = bcast(istd)
        for m in range(4):
            z = zp.tile([128, 512], MF, name="z")
            nc.vector.tensor_mul(z[:], o_tiles[bt][:, m, :], bc[:])
            nc.scalar.activation(o_tiles[bt][:, m, :], z[:], ACT.Identity,
                                 bias=be_f2[:, m:m + 1],
                                 scale=g_f2[:, m:m + 1])
        nc.sync.dma_start(io["outT"][bt], o_tiles[bt][:])

    f1[2]((0,))                # -> h[bt0]
    f2_mm(0)
    f1[2]((1,))                # -> h[bt1]
    f2_mm(1)
    f2_stats(0)
    f2_stats(1)
    f2_fa()
    f2_fb(0)
    f2_fb(1)

    ctx.close()


def build_program():
    nc = bacc.Bacc("TRN2", target_bir_lowering=False, debug=False,
                   num_devices=NCORES)
    io = {}

    def din(name, shape, dtype=F32):
        io[name] = nc.dram_tensor(name, list(shape), dtype,
                                  kind="ExternalInput").ap()

    for s in range(3):
        din(f"xT{s}", (2, 4, 128, 2, 512), dtype=MM_DT)
        din(f"lT{s}", (FS[s], BL), dtype=MM_DT)
    din("w_hp", (3, 8, 128, 512), dtype=MM_DT)
    din("b_hp", (3, 128, 4))
    din("w_r", (4, 128, 512), dtype=MM_DT)
    din("b_r", (128, 4))
    din("w_m", (3, 8, 128, 512), dtype=MM_DT)
    din("b_m", (3, 128, 4))
    din("w_g", (3, 8, 128, 512), dtype=MM_DT)
    for s in range(3):
        din(f"w_lp{s}", (FS[s], 256), dtype=MM_DT)
    din("b_lp", (3, 128, 2))
    din("w_f1", (3, 4, 128, 512), dtype=MM_DT)
    din("negc_f1", (1, 3, 512), dtype=MM_DT)
    din("w_f1l", (3, 2, 128, 512), dtype=MM_DT)
    din("b_f1", (128, 4))
    din("w_f2", (4, 128, 512), dtype=MM_DT)
    din("b_f2", (128, 4))
    for name in ("g_hp", "be_hp", "g_n1", "be_n1", "b_g_half"):
        din(name, (3, 128, 4))
    for name in ("g_lp", "be_lp"):
        din(name, (3, 128, 2))
    for name in ("g_f1", "be_f1", "g_f2", "be_f2"):
        din(name, (128, 4))
    din("ones_col", (128, 1), dtype=MM_DT)
    din("ones_row", (1, 128), dtype=MM_DT)
    io["outT"] = nc.dram_tensor("outT", [2, 128, 4, 512], MM_DT,
                                kind="ExternalOutput").ap()

    with tile.TileContext(nc) as tc:
        emit_program(tc, io)
    nc.compile()
    return nc


def make_in_maps(inputs):
    fw = fold_weights(inputs)
    dev = device_arrays(fw)
    hidden = [np.asarray(inputs["verb_hidden"], np.float32),
              np.asarray(inputs["inst_hidden"], np.float32),
              np.asarray(inputs["target_hidden"], np.float32)]
    logits = [np.asarray(inputs["verb_logits"], np.float32),
              np.asarray(inputs["inst_logits"], np.float32),
              np.asarray(inputs["target_logits"], np.float32)]
    sig = [1.0 / (1.0 + np.exp(-np.asarray(l, F64))) for l in logits]
    in_maps = []
    for core in range(NCORES):
        rows = slice(core * BL, (core + 1) * BL)
        m = dict(dev)
        for s in range(3):
            # [bt, cp, p, cc, n] tiling: each [128,2,512] pair chunk is one
            # contiguous DMA read.
            xt = np.ascontiguousarray(hidden[s][rows].T)    # [HID, BL]
            xt = xt.reshape(4, 2, 128, 2, 512).transpose(3, 0, 2, 1, 4)
            m[f"xT{s}"] = _mf_np(xt)
            m[f"lT{s}"] = _mf_np(sig[s][rows].T)
        in_maps.append(m)
    return in_maps


_NC_CACHE = None


def _run(inputs, **spmd_kwargs):
    global _NC_CACHE
    if _NC_CACHE is None:
        _NC_CACHE = build_program()
    nc = _NC_CACHE
    in_maps = make_in_maps(inputs)
    res = run_bass_kernel_spmd(nc, in_maps, list(range(NCORES)),
                               **spmd_kwargs)
    out = np.empty((B, D), dtype=np.float32)
    for core in range(NCORES):
        o = np.asarray(res.results[core]["outT"], dtype=np.float32)
        feat = o.transpose(0, 2, 1, 3).reshape(2, 512, 512)   # [bt, f, n]
        out[core * BL:(core + 1) * BL] = np.concatenate(
            [feat[0], feat[1]], axis=1).T
    return out, res


def kernel(**inputs) -> np.ndarray:
    return _run(inputs)[0]


def kernel_profiled(inputs, tmpdir=None):
    """Returns (out, BassKernelResults) with an NTFF-based profile."""
    return _run(inputs, trace=True, tmpdir=tmpdir)


# revision 95
# speedup vs baseline: 1.2449x; 1.0283x over previous
"""Trainium2 Bass kernel for nn_AttentionModule_7146825580577.

Strategy (see spec sharding_hint): pure data parallel over the batch dim
(8192 rows -> 1024 rows per core, 8 cores), weights replicated.

Schedule: the two 512-column batch tiles of each core are interleaved
inside every layer unit so the Tensor engine always has independent
matmul work queued; weights are streamed from HBM once (each chunk feeds
both batch tiles, with DMAs prefetched a unit ahead since the Sync
engine issues them serially); all matmul operands are bf16. Every
engine executes its queue in emission order, so each LayerNorm "fin" is
split into fin_a (stat eviction + PE transposes + Newton-Raphson rsqrt)
emitted early, and fin_b (back-transposes + PE outer-product broadcast
+ normalize/activation) emitted just before the consuming matmuls.

Device math (per core), in feature-transposed layout (features on SBUF
partitions, batch on the free dim):

  - All LayerNorms whose input is an affine function of a previous
    activation use host-side column-centered weights, so mean(y) == 0 by
    construction and only sum(y^2) is needed on device (squares on
    GPSIMD, reduced over partitions by a ones-vector matmul on the PE).
  - seq_len==1 MHA reduces to out_proj(v_proj(kv)); both projections are
    fused on the host into a single 512x512 effective matrix. The self-
    attention residual (x + sa(x)) is folded into a single matmul with
    weights I + Wv@Wo.
  - The cross-attention pair average (a+b)/2 is a single concat-matmul,
    prescaled by 0.25 on the host so the gate combine is
    t = e + (tanh(pre/2 + b/2) + 1) * m_tilde (sigmoid via tanh keeps
    the scalar engine on a single activation table).
  - The n2 LayerNorm (after gating) is folded into the fus_W1 matmul:
    gamma scales fold into the weights, betas fold into the bias; the
    per-sample mean correction is a rank-1 (negc x mu*istd-row) matmul
    appended to the f1 accumulation group.
  - 1/sqrt(var+eps) uses the int32 bit trick + one Newton-Raphson step
    on PE-transposed [128, k] stat tiles (tiny DVE ops), transposed back
    and broadcast across partitions via a PE outer product.
"""
import os
import sys

sys.path.insert(0, "/opt/trn_rl_repo")

import numpy as np

import concourse.bass as bass
import concourse.tile as tile
from concourse import bacc, mybir
from concourse.bass import ts
from concourse.bass_utils import run_bass_kernel_spmd
from concourse.masks import make_identity

D = 512
HID = 1024
B = 8192
NCORES = 8
BL = B // NCORES          # rows per core
EPS = 1e-5
MAGIC = 0x5F3759DF
F32 = mybir.dt.float32
I32 = mybir.dt.int32
FS = [10, 6, 15]          # logit dims per stream
NR_ITERS = int(os.environ.get("KERNEL_NR_ITERS", "1"))
MM_DT = {
    "f32r": mybir.dt.float32r,
    "f32": mybir.dt.float32,
    "bf16": mybir.dt.bfloat16,
}[os.environ.get("KERNEL_MM_DTYPE", "bf16")]
MM_IS_BF16 = MM_DT == mybir.dt.bfloat16
MF = MM_DT

F64 = np.float64


# --------------------------------------------------------------------------
# Host-side weight folding
# --------------------------------------------------------------------------

def _center_cols(W, b):
    W = np.asarray(W, F64)
    b = np.asarray(b, F64)
    return W - W.mean(axis=1, keepdims=True), b - b.mean()


def fold_weights(inp):
    g = lambda k: np.asarray(inp[k], dtype=F64)
    out = {}

    w_hp, b_hp = [], []
    for s in range(3):
        W, b = _center_cols(g("hp_W")[s], g("hp_b")[s])
        w_hp.append(W)
        b_hp.append(b)
    out["w_hp"] = np.stack(w_hp)
    out["b_hp"] = np.stack(b_hp)
    out["g_hp"], out["be_hp"] = g("hp_g"), g("hp_be")

    mhaW, mhab = g("mha_in_W"), g("mha_in_b")
    moW, mob = g("mha_out_W"), g("mha_out_b")
    Wv0, bv0 = mhaW[0][:, 2 * D:], mhab[0][2 * D:]
    Wr, br = _center_cols(np.eye(D) + Wv0 @ moW[0], bv0 @ moW[0] + mob[0])
    out["w_r"], out["b_r"] = Wr, br
    out["g_n1"], out["be_n1"] = g("n1_g"), g("n1_be")

    Wj, bj = [None] * 4, [None] * 4
    for j in (1, 2, 3):
        Wv, bv = mhaW[j][:, 2 * D:], mhab[j][2 * D:]
        Wj[j] = Wv @ moW[j]
        bj[j] = bv @ moW[j] + mob[j]
    # m_verb uses (inst_e @ W1, target_e @ W2); m_inst (verb @ W1, target @ W3);
    # m_target (verb @ W2, inst @ W3)
    # m is stored pre-halved (0.25 = average 0.5 x sigmoid-via-tanh 0.5):
    # t = e + sigmoid(pre)*m_avg = (e + m_tilde) + tanh(pre/2)*m_tilde
    # with m_tilde = 0.5*m_avg.
    mods = [(1, 2), (1, 3), (2, 3)]
    w_m, b_m = [], []
    for s in range(3):
        ja, jb = mods[s]
        w_m.append(np.concatenate([0.25 * Wj[ja], 0.25 * Wj[jb]], axis=0))
        b_m.append(0.25 * (bj[ja] + bj[jb]))
    out["w_m"] = np.stack(w_m)
    out["b_m"] = np.stack(b_m)

    out["w_g"] = g("gate_W")
    # tanh trick: sigmoid(x + b) = 0.5*tanh(0.5*x + 0.5*b) + 0.5
    out["b_g_half"] = 0.5 * g("gate_b")

    w_lp, b_lp = [], []
    for s, key in enumerate(["verb", "inst", "target"]):
        W, b = _center_cols(g(f"lp_W_{key}"), g(f"lp_b_{key}"))
        w_lp.append(W)
        b_lp.append(b)
    out["w_lp"] = w_lp
    out["b_lp"] = np.stack(b_lp)
    out["g_lp"], out["be_lp"] = g("lp_g"), g("lp_be")

    W1 = g("fus_W1")
    g2, be2 = g("n2_g"), g("n2_be")
    A1, negc = [], []
    bias_total = g("fus_b1").copy()
    for s in range(3):
        blk = W1[s * D:(s + 1) * D]
        A = g2[s][:, None] * blk
        c = blk.T @ g2[s]
        A1.append(A - A.mean(axis=1, keepdims=True))
        negc.append(-(c - c.mean()))
        bias_total += be2[s] @ blk
    L1 = []
    for s in range(3):
        off = 3 * D + s * (D // 2)
        blk = W1[off: off + D // 2]
        L1.append(blk - blk.mean(axis=1, keepdims=True))
    out["w_f1"] = np.stack(A1)
    out["negc_f1"] = np.stack(negc)
    out["w_f1l"] = np.stack(L1)
    out["b_f1"] = bias_total - bias_total.mean()
    out["g_f1"], out["be_f1"] = g("fus_g1"), g("fus_ge1")

    W2c, b2c = _center_cols(g("fus_W2"), g("fus_b2"))
    out["w_f2"], out["b_f2"] = W2c, b2c
    out["g_f2"], out["be_f2"] = g("fus_g2"), g("fus_ge2")
    return out


def _vec_pp(v, nk):
    """[.., nk*128] feature vector -> ACT per-partition layout [.., 128, nk]."""
    v = np.asarray(v, np.float32)
    return np.ascontiguousarray(v.reshape(v.shape[:-1] + (nk, 128)).swapaxes(-1, -2))


def _mf_np(v):
    """Host array in the matmul dtype (bf16 or fp32)."""
    if MM_IS_BF16:
        import ml_dtypes
        return np.ascontiguousarray(np.asarray(v, np.float32).astype(
            ml_dtypes.bfloat16))
    return np.ascontiguousarray(np.asarray(v, np.float32))


def device_arrays(fw):
    """Folded weights -> dict of arrays matching the DRAM tensor decls."""
    f32 = _mf_np
    dev = {}
    dev["w_hp"] = f32(fw["w_hp"].reshape(3, 8, 128, 512))
    dev["b_hp"] = _vec_pp(fw["b_hp"], 4)
    dev["w_r"] = f32(fw["w_r"].reshape(4, 128, 512))
    dev["b_r"] = _vec_pp(fw["b_r"], 4)
    dev["w_m"] = f32(fw["w_m"].reshape(3, 8, 128, 512))
    dev["b_m"] = _vec_pp(fw["b_m"], 4)
    dev["w_g"] = f32(fw["w_g"].reshape(3, 8, 128, 512))
    for s in range(3):
        dev[f"w_lp{s}"] = f32(fw["w_lp"][s])
    dev["b_lp"] = _vec_pp(fw["b_lp"], 2)
    dev["w_f1"] = f32(fw["w_f1"].reshape(3, 4, 128, 512))
    dev["negc_f1"] = f32(fw["negc_f1"][None])
    dev["w_f1l"] = f32(fw["w_f1l"].reshape(3, 2, 128, 512))
    dev["b_f1"] = _vec_pp(fw["b_f1"], 4)
    dev["w_f2"] = f32(fw["w_f2"].reshape(4, 128, 512))
    dev["b_f2"] = _vec_pp(fw["b_f2"], 4)
    for name in ("g_hp", "be_hp", "g_n1", "be_n1"):
        dev[name] = _vec_pp(fw[name], 4)
    dev["b_g_half"] = _vec_pp(fw["b_g_half"], 4)
    dev["g_lp"] = _vec_pp(fw["g_lp"], 2)
    dev["be_lp"] = _vec_pp(fw["be_lp"], 2)
    for name in ("g_f1", "be_f1", "g_f2", "be_f2"):
        dev[name] = _vec_pp(fw[name], 4)
    dev["ones_col"] = _mf_np(np.ones((128, 1), np.float32))
    dev["ones_row"] = _mf_np(np.ones((1, 128), np.float32))
    return dev


# --------------------------------------------------------------------------
# Device program
# --------------------------------------------------------------------------

ACT = mybir.ActivationFunctionType
ALU = mybir.AluOpType


def emit_program(tc, io):
    nc = tc.nc
    from contextlib import ExitStack
    ctx = ExitStack()

    # ---------------- pools ----------------
    P = lambda name, bufs, space="SBUF": ctx.enter_context(
        tc.tile_pool(name=name, bufs=bufs, space=space))
    const = P("const", 1)
    wpool = P("wchunk", 11)     # [128,2,512] MF weight pair chunks
    #                             (f1 holds 9 pairs + f2 prefetch)
    xpool = P("xchunk", 8)      # [128,2,512] MF input pair chunks
    #                             (all 8 live across both m-halves of a unit)
    sqp = P("sq", 7)            # [128,512] MF squares (live until deferred
    #                             stats matmuls run, a unit later)
    thp = P("th", 2)            # [128,512] MF gate tanh tiles
    zp = P("z", 2)              # [128,512] MF z = y*istd tiles
    up = P("u", 2)              # [128,512] MF gate (th+1)*m tiles
    yhp = P("yh", 6)            # [128,4,512] MF hp outputs (alive hp->r)
    ep = P("e", 6)              # [128,4,512] MF n1 outputs (alive r->gate)
    mp = P("m", 4)              # [128,4,512] MF pair-average (alive m->gate)
    tp = P("t", 6)              # [128,4,512] MF gate t; normalized in place,
    #                             alive until f1 consumes it
    lpo = P("l", 6)             # [128,2,512] MF lp outputs (alive ->f1)
    hp_ = P("h", 2)             # [128,4,512] MF f1 outputs
    op_ = P("o", 2)             # [128,4,512] MF f2 outputs
    lsp = P("ls", 1)            # [Fs,1024] MF host-sigmoided logits
    stf = P("stats_f32", 6)     # [1,512] F32 stat rows
    stb = P("stats_mf", 8)      # [1,512] MF istd/wrow rows (gate wrow rows
    #                             stay live until f1's rank-1 matmuls)
    nrp = P("nr", 4)            # [128,8] F32 NR tiles
    bcp = P("bc_sb", 2)         # [128,512] MF broadcast rows
    mm_ps = P("mm_ps", 4, "PSUM")
    st_ps = P("st_ps", 4, "PSUM")

    # ---------------- DMA helpers (needed for the head prefetch) --------
    def wpair(dram_pair_ap):
        """Load two [128,512] k-chunks in one DMA -> [128,2,512] tile."""
        wc = wpool.tile([128, 2, 512], MF, name="wcp", tag="wcp")
        nc.sync.dma_start(wc[:], dram_pair_ap.rearrange("c p n -> p c n"))
        return wc

    def load_wchunks(dram_4d, nk):
        """nk k-chunks -> list of per-chunk lhsT accessors f(m)->[128,128]."""
        fns = []
        for c0 in range(0, nk, 2):
            wc = wpair(dram_4d[c0:c0 + 2])
            for cc in range(2):
                fns.append(lambda m, wc=wc, cc=cc: wc[:, cc, ts(m, 128)])
        return fns

    def load_x(s):
        xcs = []
        for bt in range(2):
            for cp in range(4):
                xc = xpool.tile([128, 2, 512], MF, name="xc")
                nc.sync.dma_start(xc[:], io[f"xT{s}"][bt, cp])
                xcs.append(xc)
        return xcs

    # The Sync engine issues DMAs in emission order at ~0.7us apiece, so
    # the first compute units' inputs must be first in the queue.
    x_pf = [load_x(0)]
    whp_pf = [load_wchunks(io["w_hp"][0], 8)]
    lsg = []
    for s in range(3):
        t = lsp.tile([FS[s], 1024], MF, name=f"lsg{s}")
        nc.sync.dma_start(t[:], io[f"lT{s}"])
        lsg.append(t)

    # ---------------- constants / resident weights ----------------
    ident = const.tile([128, 128], F32)
    make_identity(nc, ident)
    ones_col = const.tile([128, 1], MF)
    nc.sync.dma_start(ones_col[:], io["ones_col"])
    ones_row = const.tile([1, 128], MF)
    nc.sync.dma_start(ones_row[:], io["ones_row"])

    def fconst(value, name):
        t = const.tile([1, 1], F32, name=name)
        nc.gpsimd.memset(t[:], value)
        return t
    eps1 = fconst(EPS, "eps1")
    inv_d = fconst(1.0 / D, "inv_d")
    inv_d2 = fconst(2.0 / D, "inv_d2")

    def load(name, shape, rearr=None, dtype=F32):
        t = const.tile(shape, dtype, name=name)
        src = io[name]
        if rearr:
            src = src.rearrange(rearr)
        nc.sync.dma_start(t[:], src)
        return t

    b_hp = load("b_hp", [128, 3, 4], "s p c -> p s c")
    b_r = load("b_r", [128, 4])
    b_m = load("b_m", [128, 3, 4], "s p c -> p s c")
    b_lp = load("b_lp", [128, 3, 2], "s p c -> p s c")
    b_f1 = load("b_f1", [128, 4])
    b_f2 = load("b_f2", [128, 4])
    g_hp = load("g_hp", [128, 3, 4], "s p c -> p s c")
    be_hp = load("be_hp", [128, 3, 4], "s p c -> p s c")
    g_n1 = load("g_n1", [128, 3, 4], "s p c -> p s c")
    be_n1 = load("be_n1", [128, 3, 4], "s p c -> p s c")
    b_gh = load("b_g_half", [128, 3, 4], "s p c -> p s c")
    g_lp = load("g_lp", [128, 3, 2], "s p c -> p s c")
    be_lp = load("be_lp", [128, 3, 2], "s p c -> p s c")
    g_f1 = load("g_f1", [128, 4])
    be_f1 = load("be_f1", [128, 4])
    g_f2 = load("g_f2", [128, 4])
    be_f2 = load("be_f2", [128, 4])
    negc_t = load("negc_f1", [1, 3, 512], dtype=MF)
    w_lp = [load(f"w_lp{s}", [FS[s], 256], dtype=MF) for s in range(3)]
    # w_r is shared by all three r units: load once into the const pool.
    w_r_t = []
    for c0 in (0, 2):
        t = const.tile([128, 2, 512], MF, name=f"w_r{c0}")
        nc.sync.dma_start(t[:], io["w_r"][c0:c0 + 2].rearrange("c p n -> p c n"))
        w_r_t.append(t)
    wr_fns = [(lambda m, t=w_r_t[c // 2], cc=c % 2: t[:, cc, ts(m, 128)])
              for c in range(4)]

    # ---------------- helpers ----------------
    def mm_groups(srcs, n_m, evict_fn):
        """srcs: list of (lhsT_fn(m), rhs_fn(bt)). Emits matmuls in two
        m-halves; after each half's accumulation completes, evict_fn(bt, m,
        ps) is called. bt is innermost so consecutive matmuls share the
        stationary operand."""
        last = len(srcs) - 1
        for mh in range(0, n_m, 2):
            mis = range(mh, min(mh + 2, n_m))
            ps = {(m, bt): mm_ps.tile([128, 512], F32, name="mm", tag="mm")
                  for m in mis for bt in range(2)}
            for ci, (lf, rf) in enumerate(srcs):
                for m in mis:
                    for bt in range(2):
                        nc.tensor.matmul(ps[(m, bt)][:], lf(m), rf(bt),
                                         start=(ci == 0), stop=(ci == last))
            for bt in range(2):
                for m in mis:
                    evict_fn(bt, m, ps[(m, bt)])

    def transpose_rows(rows):
        """PE-transpose k [1,512] sbuf rows into one [128,4k] SBUF tile
        (via PSUM) so the per-sample scalar math runs on fat tiles."""
        k = len(rows)
        vT = st_ps.tile([128, 4 * k], F32, name="vT", tag="stat_ps")
        for c in range(4):
            for i, v in enumerate(rows):
                nc.tensor.transpose(vT[:, c * k + i:c * k + i + 1],
                                    v[0:1, ts(c, 128)], ident[0:1, 0:1])
        vs = nrp.tile([128, 4 * k], F32, name="nr_v")
        nc.vector.tensor_copy(vs[:], vT[:])
        return vs

    def nr_rsqrt_T(vs, k):
        """Newton-Raphson rsqrt of a transposed [128,4k] tile (GPSIMD)."""
        y = nrp.tile([128, 4 * k], F32, name="nr_y")
        t = nrp.tile([128, 4 * k], F32, name="nr_t")
        nc.vector.tensor_scalar(y[:].bitcast(I32), vs[:].bitcast(I32),
                                1, None, ALU.logical_shift_right)
        nc.vector.tensor_scalar(y[:].bitcast(I32), y[:].bitcast(I32),
                                -1, MAGIC, ALU.mult, ALU.add)
        for _ in range(NR_ITERS):
            nc.vector.tensor_mul(t[:], y[:], y[:])
            nc.vector.tensor_mul(t[:], t[:], vs[:])
            nc.vector.tensor_scalar(t[:], t[:], -0.5, 1.5, ALU.mult, ALU.add)
            nc.vector.tensor_mul(y[:], y[:], t[:])
        return y

    def row_back(y, k, i):
        """Transpose column set i of [128,4k] back to a [1,512] MF row."""
        rT = st_ps.tile([1, 512], F32, name="rT", tag="stat_ps")
        for c in range(4):
            nc.tensor.transpose(rT[0:1, ts(c, 128)],
                                y[:, c * k + i:c * k + i + 1], ident)
        row = stb.tile([1, 512], MF, name="r16")
        nc.vector.tensor_copy(row[:], rT[:])
        return row

    def bcast(row):
        """[1,512] row -> [128,512] MF tile via PE outer product."""
        bps = st_ps.tile([128, 512], F32, name="bc_ps", tag="stat_ps")
        nc.tensor.matmul(bps[:], ones_row[:], row[0:1, :],
                         start=True, stop=True)
        bc = bcp.tile([128, 512], MF, name="bc")
        nc.scalar.activation(bc[:], bps[:], ACT.Identity)
        return bc

    # ---------------- unit emitters ----------------
    # Each unit emits its matmuls+evictions inline and returns
    # (stats_fn, fin_fn) closures to be sequenced by the main schedule.

    def ln_unit(srcs, n_m, bias_cols, gam_cols, bet_cols, func, out_pool,
                dim, mm_emitter=None):
        """Generic matmul->LN->activation unit over both batch tiles.
        PSUM is evicted (bias added) straight into the unit's output tile;
        the final activation overwrites the same slice in place.
        Returns (stats_fn, fin_a_fn, fin_b_fn, outs)."""
        outs = [None, None]

        def evict(bt, m, ps):
            if outs[bt] is None:
                outs[bt] = out_pool.tile([128, n_m, 512], MF, name="out")
            nc.vector.tensor_scalar_add(outs[bt][:, m, :], ps[:],
                                        bias_cols[m])

        (mm_emitter or mm_groups)(srcs, n_m, evict)
        st = [None, None]

        def stats():
            for bt in range(2):
                st[bt] = st_ps.tile([1, 512], F32, name="st", tag="stat_ps")
                for m in range(n_m):
                    sq = sqp.tile([128, 512], MF, name="sq")
                    nc.gpsimd.tensor_mul(sq[:], outs[bt][:, m, :],
                                         outs[bt][:, m, :])
                    nc.tensor.matmul(st[bt][:], ones_col[:], sq[:],
                                     start=(m == 0), stop=(m == n_m - 1))

        hold = {}

        def fin_a():
            vr = []
            for bt in range(2):
                v = stf.tile([1, 512], F32, name="r32")
                nc.scalar.activation(v[0:1, :], st[bt][:], ACT.Identity,
                                     bias=eps1[:],
                                     scale=(inv_d if dim == D else inv_d2)[:])
                vr.append(v)
            hold["y"] = nr_rsqrt_T(transpose_rows(vr), 2)

        def fin_b(bts=(0, 1)):
            for bt in bts:
                istd = row_back(hold["y"], 2, bt)
                bc = bcast(istd)
                for m in range(n_m):
                    z = zp.tile([128, 512], MF, name="z")
                    nc.vector.tensor_mul(z[:], outs[bt][:, m, :], bc[:])
                    nc.scalar.activation(outs[bt][:, m, :], z[:], func,
                                         bias=bet_cols[m], scale=gam_cols[m])

        return stats, fin_a, fin_b, outs

    def m_unit(s, e_tiles, m_streams, wfns):
        """Pair-average matmul; eviction only (adds bias)."""
        sa, sb = m_streams[s]
        srcs = []
        for c in range(8):
            if c < 4:
                rf = (lambda c: (lambda bt: e_tiles[sa][bt][:, c, :]))(c)
            else:
                rf = (lambda c: (lambda bt: e_tiles[sb][bt][:, c - 4, :]))(c)
            srcs.append((wfns[c], rf))
        m_sb = [mp.tile([128, 4, 512], MF, name="m_sb") for _ in range(2)]

        def evict(bt, m, ps):
            nc.vector.tensor_scalar_add(m_sb[bt][:, m, :], ps[:],
                                        b_m[:, s, m:m + 1])

        mm_groups(srcs, 4, evict)
        return m_sb

    def gate_unit(s, e_tiles, m_sb, wfns):
        """Gate matmul -> tanh-sigmoid -> t = e + gate*m -> n2 stats.
        zt = t*istd_bc - (mu*istd)_bc is produced in fin."""
        srcs = []
        for c in range(8):
            if c < 4:
                rf = (lambda c: (lambda bt: e_tiles[s][bt][:, c, :]))(c)
            else:
                rf = (lambda c: (lambda bt: m_sb[bt][:, c - 4, :]))(c)
            srcs.append((wfns[c], rf))
        t_sb = [tp.tile([128, 4, 512], MF, name="t_sb") for _ in range(2)]

        def evict(bt, m, ps):
            # t = e + sigmoid(pre)*m_avg = e + (th+1)*m_tilde
            th = thp.tile([128, 512], MF, name="th")
            nc.scalar.activation(th[:], ps[:], ACT.Tanh,
                                 bias=b_gh[:, s, m:m + 1], scale=0.5)
            u = up.tile([128, 512], MF, name="u")
            nc.vector.scalar_tensor_tensor(u[:], th[:], 1.0,
                                           m_sb[bt][:, m, :],
                                           ALU.add, ALU.mult)
            nc.vector.tensor_add(t_sb[bt][:, m, :], u[:],
                                 e_tiles[s][bt][:, m, :])

        mm_groups(srcs, 4, evict)
        st_sum = [None, None]
        st_sq = [None, None]

        def stats():
            for bt in range(2):
                st_sum[bt] = st_ps.tile([1, 512], F32, name="st_sum",
                                        tag="stat_ps")
                for m in range(4):
                    nc.tensor.matmul(st_sum[bt][:], ones_col[:],
                                     t_sb[bt][:, m, :],
                                     start=(m == 0), stop=(m == 3))
            for bt in range(2):
                st_sq[bt] = st_ps.tile([1, 512], F32, name="st_sq",
                                       tag="stat_ps")
                for m in range(4):
                    sq = sqp.tile([128, 512], MF, name="sq")
                    nc.gpsimd.tensor_mul(sq[:], t_sb[bt][:, m, :],
                                         t_sb[bt][:, m, :])
                    nc.tensor.matmul(st_sq[bt][:], ones_col[:], sq[:],
                                     start=(m == 0), stop=(m == 3))

        hold = {}

        def fin_a():
            mu_rows, v_rows = [], []
            for bt in range(2):
                m_ = stf.tile([1, 512], F32, name="r32")
                nc.scalar.activation(m_[0:1, :], st_sum[bt][:], ACT.Identity,
                                     scale=inv_d[:])
                v = stf.tile([1, 512], F32, name="r32")
                nc.scalar.activation(v[0:1, :], st_sq[bt][:], ACT.Identity,
                                     bias=eps1[:], scale=inv_d[:])
                mu_rows.append(m_)
                v_rows.append(v)
            muS = transpose_rows(mu_rows)
            vS = transpose_rows(v_rows)
            musq = nrp.tile([128, 8], F32, name="nr_t")
            nc.vector.tensor_mul(musq[:], muS[:], muS[:])
            nc.vector.tensor_sub(vS[:], vS[:], musq[:])
            y = nr_rsqrt_T(vS, 2)
            wT = nrp.tile([128, 8], F32, name="nr_w")
            nc.vector.tensor_mul(wT[:], muS[:], y[:])
            hold["y"], hold["w"] = y, wT

        wrows = {}

        def fin_b(bts=(0, 1)):
            # zt = t*istd_bc in place; the -mu*istd mean correction is a
            # rank-1 negc matmul inside f1 (wrows are its rhs rows).
            for bt in bts:
                istd = row_back(hold["y"], 2, bt)
                wrows[bt] = row_back(hold["w"], 2, bt)
                bci = bcast(istd)
                for m in range(4):
                    nc.vector.tensor_mul(t_sb[bt][:, m, :],
                                         t_sb[bt][:, m, :], bci[:])

        return stats, fin_a, fin_b, t_sb, wrows

    # ---------------- unit constructors ----------------
    def make_lp(s):
        srcs = [(lambda m, s=s: w_lp[s][:, ts(m, 128)],
                 lambda bt, s=s: lsg[s][:, ts(bt, 512)])]
        return ln_unit(srcs, 2,
                       [b_lp[:, s, c:c + 1] for c in range(2)],
                       [g_lp[:, s, c:c + 1] for c in range(2)],
                       [be_lp[:, s, c:c + 1] for c in range(2)],
                       ACT.Gelu, lpo, D // 2)

    def make_hp(s, xcs, wfns):
        srcs = [(wfns[c],
                 (lambda c: (lambda bt: xcs[bt * 4 + c // 2][:, c % 2, :]))(c))
                for c in range(8)]
        return ln_unit(srcs, 4,
                       [b_hp[:, s, c:c + 1] for c in range(4)],
                       [g_hp[:, s, c:c + 1] for c in range(4)],
                       [be_hp[:, s, c:c + 1] for c in range(4)],
                       ACT.Gelu, yhp, D)

    def make_r(s, yh):
        srcs = [(wr_fns[c], (lambda c: (lambda bt: yh[bt][:, c, :]))(c))
                for c in range(4)]
        return ln_unit(srcs, 4,
                       [b_r[:, c:c + 1] for c in range(4)],
                       [g_n1[:, s, c:c + 1] for c in range(4)],
                       [be_n1[:, s, c:c + 1] for c in range(4)],
                       ACT.Identity, ep, D)

    def prefetch_f1():
        fns = []
        for s in range(3):
            fns.append(load_wchunks(io["w_f1l"][s], 2))
        for s in (2, 1, 0):
            fns.append(load_wchunks(io["w_f1"][s], 4))
        return fns

    def make_f1(l_tiles, zt_tiles, gate_fbs, wf, wrows_by_s):
        """f1 with the three gate fin_b's interleaved between chunk stages:
        l chunks first, then g2.fb, zt2 chunks, g1.fb, zt1, g0.fb, zt0,
        and the rank-1 mean-correction (negc x mu*istd rows) last."""
        srcs = []
        for s in range(3):
            for c in range(2):
                srcs.append((wf[s][c],
                             (lambda s, c: (lambda bt: l_tiles[s][bt][:, c, :]))(s, c)))
        for i, s in enumerate((2, 1, 0)):
            for c in range(4):
                srcs.append((wf[3 + i][c],
                             (lambda s, c: (lambda bt: zt_tiles[s][bt][:, c, :]))(s, c)))
        for s in (2, 1, 0):
            srcs.append(((lambda m, s=s: negc_t[0:1, s, ts(m, 128)]),
                         (lambda s=s: (lambda bt: wrows_by_s[s][bt][0:1, :]))()))
        fb_at = {0: gate_fbs[2], 6: gate_fbs[1], 10: gate_fbs[0]}

        def emitter(srcs, n_m, evict_fn):
            last = len(srcs) - 1
            for mh in range(0, n_m, 2):
                mis = range(mh, mh + 2)
                ps = {(m, bt): mm_ps.tile([128, 512], F32, name="mm",
                                          tag="mm")
                      for m in mis for bt in range(2)}
                for ci, (lf, rf) in enumerate(srcs):
                    if mh == 0 and ci in fb_at:
                        fb_at[ci]()
                    for m in mis:
                        for bt in range(2):
                            nc.tensor.matmul(ps[(m, bt)][:], lf(m), rf(bt),
                                             start=(ci == 0),
                                             stop=(ci == last))
                for bt in range(2):
                    for m in mis:
                        evict_fn(bt, m, ps[(m, bt)])

        return ln_unit(srcs, 4,
                       [b_f1[:, c:c + 1] for c in range(4)],
                       [g_f1[:, c:c + 1] for c in range(4)],
                       [be_f1[:, c:c + 1] for c in range(4)],
                       ACT.Gelu, hp_, D, mm_emitter=emitter)

    # ---------------- main schedule ----------------
    # Emission order == per-engine execution order (all engines run their
    # queues in order). Each unit's fin is split: fin_a (stat eviction +
    # transposes + NR chain) is emitted early so its latency runs under
    # later matmul blocks; fin_b (back-transposes + broadcast + normalize)
    # is emitted just before the consumer needs the result.
    m_streams = [(1, 2), (0, 2), (0, 1)]

    x_pf.append(load_x(1))
    whp_pf.append(load_wchunks(io["w_hp"][1], 8))
    hp0 = make_hp(0, x_pf[0], whp_pf[0])
    lp_u = [make_lp(s) for s in range(3)]
    x_pf.append(load_x(2))
    whp_pf.append(load_wchunks(io["w_hp"][2], 8))
    hp1 = make_hp(1, x_pf[1], whp_pf[1])
    hp0[0]()                   # hp0 stats
    for s in range(3):
        lp_u[s][0]()           # lp stats
    hp0[1]()                   # hp0 fin_a
    for s in range(3):
        lp_u[s][1]()           # lp fin_a
    hp2 = make_hp(2, x_pf[2], whp_pf[2])
    wf_m2 = load_wchunks(io["w_m"][2], 8)
    hp1[0]()
    hp0[2]()                   # hp0 fin_b -> yh0
    for s in range(3):
        lp_u[s][2]()           # lp fin_b -> l
    r0 = make_r(0, hp0[3])
    hp1[1]()
    hp2[0]()
    hp1[2]()                   # -> yh1
    r1 = make_r(1, hp1[3])
    wf_m1 = load_wchunks(io["w_m"][1], 8)
    r0[0]()
    hp2[1]()
    hp2[2]()                   # -> yh2
    r2 = make_r(2, hp2[3])
    wf_g2 = load_wchunks(io["w_g"][2], 8)
    r0[1]()
    r1[0]()
    r0[2]()                    # -> e0
    r2[0]()
    r1[1]()
    r2[1]()
    r1[2]()                    # -> e1
    e_tiles = [r0[3], r1[3], r2[3]]
    m2 = m_unit(2, e_tiles, m_streams, wf_m2)   # e0 (c0-3), e1 (c4-7)
    wf_m0 = load_wchunks(io["w_m"][0], 8)
    r2[2]()                    # -> e2 (NR ran under m2's matmuls)
    m1 = m_unit(1, e_tiles, m_streams, wf_m1)   # e0 (c0-3), e2 (c4-7)
    wf_g1 = load_wchunks(io["w_g"][1], 8)
    g2 = gate_unit(2, e_tiles, m2, wf_g2)
    m0 = m_unit(0, e_tiles, m_streams, wf_m0)   # e1, e2
    wf_g0 = load_wchunks(io["w_g"][0], 8)
    g2[0]()                    # g2 stats
    g1 = gate_unit(1, e_tiles, m1, wf_g1)
    g2[1]()                    # g2 fin_a
    wf_f1 = prefetch_f1()
    g0 = gate_unit(0, e_tiles, m0, wf_g0)
    wf_f2 = load_wchunks(io["w_f2"], 4)
    g1[0]()
    g0[0]()
    g1[1]()                    # g1 fin_a
    g0[1]()                    # g0 fin_a
    l_tiles = [u[3] for u in lp_u]
    zt_tiles = [g0[3], g1[3], g2[3]]
    f1 = make_f1(l_tiles, zt_tiles, [g0[2], g1[2], g2[2]], wf_f1,
                 {0: g0[4], 1: g1[4], 2: g2[4]})
    f1[0]()
    f1[1]()

    # ---- f2 (final LN), pipelined per batch tile with f1's fin_b ----
    h_tiles = f1[3]
    o_tiles = [None, None]
    f2st = [None, None]
    f2hold = {}

    def f2_mm(bt):
        o_tiles[bt] = op_.tile([128, 4, 512], MF, name="o_sb")
        ps = [mm_ps.tile([128, 512], F32, name="mm", tag="mm")
              for _ in range(4)]
        for ci in range(4):
            for m in range(4):
                nc.tensor.matmul(ps[m][:], wf_f2[ci](m),
                                 h_tiles[bt][:, ci, :],
                                 start=(ci == 0), stop=(ci == 3))
        for m in range(4):
            nc.vector.tensor_scalar_add(o_tiles[bt][:, m, :], ps[m][:],
                                        b_f2[:, m:m + 1])

    def f2_stats(bt):
        f2st[bt] = st_ps.tile([1, 512], F32, name="st", tag="stat_ps")
        for m in range(4):
            sq = sqp.tile([128, 512], MF, name="sq")
            nc.gpsimd.tensor_mul(sq[:], o_tiles[bt][:, m, :],
                                 o_tiles[bt][:, m, :])
            nc.tensor.matmul(f2st[bt][:], ones_col[:], sq[:],
                             start=(m == 0), stop=(m == 3))

    def f2_fa():
        vr = []
        for bt in range(2):
            v = stf.tile([1, 512], F32, name="r32")
            nc.scalar.activation(v[0:1, :], f2st[bt][:], ACT.Identity,
                                 bias=eps1[:], scale=inv_d[:])
            vr.append(v)
        f2hold["y"] = nr_rsqrt_T(transpose_rows(vr), 2)

    def f2_fb(bt):
        istd = row_back(f2hold["y"], 2, bt)
        bc = bcast(istd)
        for m in range(4):
            z = zp.tile([128, 512], MF, name="z")
            nc.vector.tensor_mul(z[:], o_tiles[bt][:, m, :], bc[:])
            nc.scalar.activation(o_tiles[bt][:, m, :], z[:], ACT.Identity,
                                 bias=be_f2[:, m:m + 1],
                                 scale=g_f2[:, m:m + 1])
        nc.sync.dma_start(io["outT"][bt], o_tiles[bt][:])

    f1[2]((0,))                # -> h[bt0]
    f2_mm(0)
    f1[2]((1,))                # -> h[bt1]
    f2_mm(1)
    f2_stats(0)
    f2_stats(1)
    f2_fa()
    f2_fb(0)
    f2_fb(1)

    ctx.close()


def build_program():
    nc = bacc.Bacc("TRN2", target_bir_lowering=False, debug=False,
                   num_devices=NCORES)
    io = {}

    def din(name, shape, dtype=F32):
        io[name] = nc.dram_tensor(name, list(shape), dtype,
                                  kind="ExternalInput").ap()

    for s in range(3):
        din(f"xT{s}", (2, 4, 128, 2, 512), dtype=MM_DT)
        din(f"lT{s}", (FS[s], BL), dtype=MM_DT)
    din("w_hp", (3, 8, 128, 512), dtype=MM_DT)
    din("b_hp", (3, 128, 4))
    din("w_r", (4, 128, 512), dtype=MM_DT)
    din("b_r", (128, 4))
    din("w_m", (3, 8, 128, 512), dtype=MM_DT)
    din("b_m", (3, 128, 4))
    din("w_g", (3, 8, 128, 512), dtype=MM_DT)
    for s in range(3):
        din(f"w_lp{s}", (FS[s], 256), dtype=MM_DT)
    din("b_lp", (3, 128, 2))
    din("w_f1", (3, 4, 128, 512), dtype=MM_DT)
    din("negc_f1", (1, 3, 512), dtype=MM_DT)
    din("w_f1l", (3, 2, 128, 512), dtype=MM_DT)
    din("b_f1", (128, 4))
    din("w_f2", (4, 128, 512), dtype=MM_DT)
    din("b_f2", (128, 4))
    for name in ("g_hp", "be_hp", "g_n1", "be_n1", "b_g_half"):
        din(name, (3, 128, 4))
    for name in ("g_lp", "be_lp"):
        din(name, (3, 128, 2))
    for name in ("g_f1", "be_f1", "g_f2", "be_f2"):
        din(name, (128, 4))
    din("ones_col", (128, 1), dtype=MM_DT)
    din("ones_row", (1, 128), dtype=MM_DT)
    io["outT"] = nc.dram_tensor("outT", [2, 128, 4, 512], MM_DT,
                                kind="ExternalOutput").ap()

    with tile.TileContext(nc) as tc:
        emit_program(tc, io)
    nc.compile()
    return nc


def make_in_maps(inputs):
    fw = fold_weights(inputs)
    dev = device_arrays(fw)
    hidden = [np.asarray(inputs["verb_hidden"], np.float32),
              np.asarray(inputs["inst_hidden"], np.float32),
              np.asarray(inputs["target_hidden"], np.float32)]
    logits = [np.asarray(inputs["verb_logits"], np.float32),
              np.asarray(inputs["inst_logits"], np.float32),
              np.asarray(inputs["target_logits"], np.float32)]
    sig = [1.0 / (1.0 + np.exp(-np.asarray(l, F64))) for l in logits]
    in_maps = []
    for core in range(NCORES):
        rows = slice(core * BL, (core + 1) * BL)
        m = dict(dev)
        for s in range(3):
            # [bt, cp, p, cc, n] tiling: each [128,2,512] pair chunk is one
            # contiguous DMA read.
            xt = np.ascontiguousarray(hidden[s][rows].T)    # [HID, BL]
            xt = xt.reshape(4, 2, 128, 2, 512).transpose(3, 0, 2, 1, 4)
            m[f"xT{s}"] = _mf_np(xt)
            m[f"lT{s}"] = _mf_np(sig[s][rows].T)
        in_maps.append(m)
    return in_maps


_NC_CACHE = None


def _run(inputs, **spmd_kwargs):
    global _NC_CACHE
    if _NC_CACHE is None:
        _NC_CACHE = build_program()
    nc = _NC_CACHE
    in_maps = make_in_maps(inputs)
    res = run_bass_kernel_spmd(nc, in_maps, list(range(NCORES)),
                               **spmd_kwargs)
    out = np.empty((B, D), dtype=np.float32)
    for core in range(NCORES):
        o = np.asarray(res.results[core]["outT"], dtype=np.float32)
        feat = o.transpose(0, 2, 1, 3).reshape(2, 512, 512)   # [bt, f, n]
        out[core * BL:(core + 1) * BL] = np.concatenate(
            [feat[0], feat[1]], axis=1).T
    return out, res


def kernel(**inputs) -> np.ndarray:
    return _run(inputs)[0]


def kernel_profiled(inputs, tmpdir=None):
    """Returns (out, BassKernelResults) with an NTFF-based profile."""
    return _run(inputs, trace=True, tmpdir=tmpdir)
